# revision 21
# baseline (speedup 1.0000x reference)
"""Sharded MHA-with-RoPE Trainium2 kernel (nn_CustomTorchMHASelf).

Contract: kernel(**inputs) takes the FULL unsharded inputs of the
reference (x [2,2048,2048], Wqkv_w [6144,2048], Wqkv_b [6144],
out_w [2048,2048], out_b [2048]) and returns the full [2,2048,2048]
fp32 output, running the compute on 8 NeuronCores.

Sharding: core = b*4 + g handles batch b and head-group g (4 of the 16
heads). Each core computes q/k/v projections for its heads, RoPE,
softmax attention, and its slice of the out-projection; the host sums
the 4 partial outputs per batch and adds out_b.

Device data plane is bf16 (fp32 PSUM accumulation); the host
pre-transposes x and the weight slices into the layouts the TensorE
wants (contraction dim on partitions everywhere).

Schedule: pass 1 computes K+RoPE and V for all tokens (the last block
also hides the attention prologue under its V-projection); pass 2 is a
flat software pipeline over (block, head) steps -- at step k the PE
stream interleaves att@V(k), scores(k+1), q-projection(k+2) and a
quarter of the previous block's out-projection (65 matmuls/step), so
the ScalarE exp stream (16/step) is never on the critical path.
Key device tricks:
  - DMA triggers cost ~600ns of issuing-engine queue time EACH
    regardless of size, so tensors move with ONE trigger per tile
    (16KB-per-partition contiguous descriptor runs) instead of 16;
    the startup-critical wk/x0 pair is split into eighth/quarter
    chunks issued alternately so the first K matmuls start ~10.5us
    in, and block 0's K-projection walks chunk-major (4 live PSUM
    accumulators) to consume chunks as they land;
  - rotate-half for RoPE is a PE matmul with a signed permutation
    matrix (SBUF-SBUF DMA swaps are slow and their DIRECT2D triggers
    serialize on the sync sequencer);
  - the softmax denominator is a bf16 tree-add into the attB tile on
    VectorE plus ONE ones-matmul per (head, block) instead of 16 full
    PE ones-matmuls; att is split into two tiles (attA/attB) so the
    tree's writes never alias tiles the PE still reads (the dep
    tracker is coarse); the ones-matmul+normalize are deferred one
    step so the PE never waits on the tree;
  - 1/denominator = Exp(-Ln(d)) on ScalarE (ln and exp share an
    activation table, so no table reloads) because DVE reciprocal is
    slow and custom-DVE ops don't compile on this toolchain;
  - deferred finishes are woven into matmul slots 2-5 of the next step
    (not appended after it) so their Ln/Exp never delays the next
    step's first exps, whose psS banks the score matmuls wait on;
  - warm-up matmuls on a memset tile (no DMA dependency) ramp the PE
    out of its 1.2GHz cold p-state while the first weight/x DMAs land,
    and filler matmuls after the first two K chunks keep the HAM
    activity window busy so the clock never re-throttles mid-startup;
  - output tiles are written bf16, one batched [P, E] DMA per token
    row-block; in the last step the tree is halved (level 1 written
    into a retired att tile), the PE itself does the final 8-way
    denominator reduction as an accumulating ones-matmul group, and
    the drain's final tile is split into column halves so the tail
    transfer is short.
"""

import math
import os
import sys
import types

import numpy as np
import ml_dtypes

import concourse.bass as bass
import concourse.mybir as mybir
import concourse.tile as tile
from concourse.bass import ds

F32 = mybir.dt.float32
BF16 = mybir.dt.bfloat16
Alu = mybir.AluOpType
Act = mybir.ActivationFunctionType
BF = ml_dtypes.bfloat16

S, E, HTOT, HL, D, P = 2048, 2048, 16, 4, 128, 128

# Filled with the profile exec time (ns) when MHA_TRACE=1; read by test.py.
LAST_EXEC_NS = None


def _install_axon_ntff_shim():
    """Provide antenv.axon_hooks so trace=True can reach the axon NTFF hook."""
    if "antenv.axon_hooks" in sys.modules:
        return
    mod = types.ModuleType("antenv.axon_hooks")
    holder = [None]
    mod.set_axon_ntff_profile_hook = lambda h: holder.__setitem__(0, h)
    mod.get_axon_ntff_profile_hook = lambda: holder[0]
    sys.modules["antenv.axon_hooks"] = mod
    try:
        import antenv
        antenv.axon_hooks = mod
    except ImportError:
        pass
    # boot() ran at interpreter start (sitecustomize), before this module
    # existed, so its NTFF-hook registration was silently skipped. Redo it.
    try:
        from trn_agent_boot.trn_boot import _ntff_profile_via_ctypes
        hook = _ntff_profile_via_ctypes("/opt/axon/libaxon_pjrt.so")
        if hook is not None:
            mod.set_axon_ntff_profile_hook(hook)
    except Exception:
        pass


def _split_multi_waits(nc):
    """Hoist extra sem-waits onto standalone NoOps (one wait per inst).

    This walrus build rejects any instruction carrying more than one
    sync-wait ("Too many sync wait commands"); Tile attaches one wait per
    outstanding semaphore to the consuming instruction. Splitting them
    across same-engine NoOps placed immediately before is equivalent:
    the engine executes serially, so all waits still precede the inst.
    """
    ctr = 0
    for fn in nc.m.functions:
        for blk in fn.blocks:
            out = []
            for inst in blk.instructions:
                si = getattr(inst, "sync_info", None)
                if si is not None and si.on_wait is not None \
                        and len(si.on_wait) > 1:
                    waits = list(si.on_wait)
                    si.on_wait = [waits[-1]]
                    for w in waits[:-1]:
                        ctr += 1
                        nop = mybir.InstNoOp(
                            name=f"I-wsplit-{ctr}", ins=[], outs=[])
                        nop.engine = inst.engine
                        nop.sync_info = mybir.SyncInfo(
                            on_wait=[w], on_update=[])
                        out.append(nop)
                out.append(inst)
            blk.instructions[:] = out


def _build_mha(nc: bass.Bass):
    """Emit the per-core MHA program (one shard) into `nc`."""
    EO = E // P            # contraction subtiles for the projections
    ST = 512               # free-dim tile (one PSUM bank of fp32)
    NS = S // ST
    SB = S // P
    JT = S // P            # key blocks per head
    ET = E // ST

    # packed layouts: [.., P, EO, ST] so each tensor is one long
    # per-partition-contiguous run -> ONE ~600ns DMA trigger moves it
    xP = nc.dram_tensor("xP", [NS, P, EO, ST], BF16, kind="ExternalInput")
    wkP = nc.dram_tensor("wkP", [P, EO, HL * D], BF16, kind="ExternalInput")
    wqP = nc.dram_tensor("wqP", [P, EO, HL * D], BF16, kind="ExternalInput")
    wvP = nc.dram_tensor("wvP", [P, EO, HL * D], BF16, kind="ExternalInput")
    # qkb | vb | ones | perm | cos | sin packed into one bf16 table;
    # the small head (biases + matrices) rides an early DMA slice, the
    # big cos/sin tail a later one
    TW = 2 * HL + HL * D + 2 * P + 2 * S
    tblP = nc.dram_tensor("tblP", [P, TW], BF16, kind="ExternalInput")
    owT = nc.dram_tensor("owT", [HL * D, E], BF16, kind="ExternalInput")
    out = nc.dram_tensor("out", [S, E], BF16, kind="ExternalOutput")

    isc = 1.0 / math.sqrt(D)

    from contextlib import ExitStack

    with tile.TileContext(nc) as tc, ExitStack() as stk:
        persist = stk.enter_context(tc.tile_pool(name="persist", bufs=1))
        kT_sb = persist.tile([P, HL, S], BF16)      # k post-RoPE [d, h, s]
        v_sb = persist.tile([P, SB, HL * D], BF16)  # v natural [s%128, s//128, hd]
        ctxT_sb = persist.tile([P, HL, S], BF16)    # [d, h, i]
        tbl_sb = persist.tile([P, TW], BF16)
        ow_sb = persist.tile([P, HL, E], BF16)
        wtile = persist.tile([P, P], BF16)          # memset warm-up operand

        QK0, VB0 = 0, 2 * HL
        ON0 = VB0 + HL * D
        PM0 = ON0 + P
        CS0, SN0 = PM0 + P, PM0 + P + S
        TSMALL = PM0 + P          # early slice: biases + ones + perm

        # tensor_scalar needs f32 scalars: widen the bf16 biases once
        qkbf = persist.tile([P, 2 * HL], F32)

        def qkb_ap(j):
            return qkbf[:, j, None]

        vb_ap = tbl_sb[:, ds(VB0, HL * D)]

        def cos_ap(s0):
            return tbl_sb[:, ds(CS0 + s0, ST)]

        def sin_ap(s0):
            return tbl_sb[:, ds(SN0 + s0, ST)]

        ones_ap = tbl_sb[:, ds(ON0, P)]
        perm_ap = tbl_sb[:, ds(PM0, P)]

        # x stream shared by both passes; rope temps likewise.  qb/rot are
        # still being read (by the rotate matmul / mults) when the next rope
        # starts, so they get extra bufs; t1/t2 are consumed immediately by
        # the in-order VectorE queue, so 1 buf suffices.
        xs = stk.enter_context(tc.tile_pool(name="xstream", bufs=2))
        rta = stk.enter_context(tc.tile_pool(name="ropetmpa", bufs=3))
        rtb = stk.enter_context(tc.tile_pool(name="ropetmpb", bufs=1))
        wqp = stk.enter_context(tc.tile_pool(name="wqpool", bufs=1))
        wq_sb = wqp.tile([P, EO, HL * D], BF16)

        psA = stk.enter_context(tc.tile_pool(name="psA", bufs=4, space="PSUM"))
        psS = stk.enter_context(tc.tile_pool(name="psS", bufs=2, space="PSUM"))
        psC = stk.enter_context(tc.tile_pool(name="psC", bufs=2, space="PSUM"))

        qp = stk.enter_context(tc.tile_pool(name="qpool", bufs=4))
        dp = stk.enter_context(tc.tile_pool(name="denp", bufs=1))
        oc = stk.enter_context(tc.tile_pool(name="ocopy", bufs=2))
        at0 = stk.enter_context(tc.tile_pool(name="att0p", bufs=1))

        # flat (block, head) schedule for the attention pass; blocks in
        # reverse order so the first one reuses pass 1's last x tile
        order = list(range(NS - 1, -1, -1))
        seq = [(i, h) for i in order for h in range(HL)]
        NK = len(seq)

        def blk(k):
            return seq[k][0]

        def rope_begin(ps, bias_ap):
            # qb = q + bias (bf16 so the rotate-half matmul runs full rate)
            qb = rta.tile([P, ST], BF16, tag="qb")
            nc.vector.tensor_scalar_add(qb[:], ps[:], bias_ap)
            return qb

        def rope_finish(qb, s0, out_ap):
            # rotate-half as a PE matmul with a signed permutation matrix
            # (cross-partition moves otherwise need a slow SBUF-SBUF DMA
            # whose trigger also serializes on the sync sequencer);
            # out = qb*cos + (perm.T @ qb)*sin.
            # rps lives in psC so the scores stream owns psS's two banks
            # outright (16 even allocations per step -> the exp reader is
            # always 2 full matmul-slots behind the next allocation)
            rps = psC.tile([P, ST], F32, tag="ct", name="rps")
            nc.tensor.matmul(rps[:], perm_ap, qb[:], start=True, stop=True)
            t1 = rtb.tile([P, ST], BF16, tag="t1")
            t2 = rtb.tile([P, ST], BF16, tag="t2")
            nc.vector.tensor_tensor(t1[:], qb[:], cos_ap(s0), Alu.mult)
            nc.vector.tensor_tensor(t2[:], rps[:], sin_ap(s0), Alu.mult)
            nc.vector.tensor_tensor(out_ap, t1[:], t2[:], Alu.add)

        def qproj_mm(k, psq, xt, eo):
            h = seq[k][1]
            nc.tensor.matmul(
                psq[:], wq_sb[:, eo, ds(h * D, D)], xt[:, eo, :],
                start=(eo == 0), stop=(eo == EO - 1))

        def qproj_fin(k, psq):
            qt = qp.tile([P, ST], BF16, tag="qt")
            qb = rope_begin(psq, qkb_ap(HL + seq[k][1]))
            return qt, (qb, blk(k) * ST, qt[:])

        def scores_mm(k, attab, qt, jb):
            h = seq[k][1]
            ps = psS.tile([P, ST], F32, tag="sc")
            nc.tensor.matmul(
                ps[:], kT_sb[:, h, ds(jb * P, P)], qt[:],
                start=True, stop=True)
            dst = attab[jb // 8][:, jb % 8, :]
            nc.scalar.activation(dst, ps[:], Act.Exp, scale=isc)

        # PE warm-up on a memset tile (no DMA dependency): the HAM clock
        # gate needs ~3.4us of sustained PE activity to release the cold
        # 1.2GHz p-state, and the first real matmul data lands ~10.5us in.
        nc.gpsimd.memset(wtile[:], 1.0)
        wsink = persist.tile([P, 1], F32)
        wps = psS.tile([P, ST], F32, tag="sc", name="warmps")
        NWARM = 44
        for w in range(NWARM):
            nc.tensor.matmul(
                wps[:, :P], wtile[:], wtile[:],
                start=(w == 0), stop=(w == NWARM - 1))
        nc.vector.tensor_copy(wsink[:], wps[:, :1])

        # ---- pass 1: K projection + RoPE, V projection ----
        # The last block additionally hides the attention pass's prologue
        # (q heads 0/1 + scores/exp for head 0) under its V-projection.
        xt_last = None
        att0 = None
        qtiles = []
        with tc.tile_pool(name="p1w", bufs=1) as p1:
            wk_sb = p1.tile([P, EO, HL * D], BF16)
            wv_sb = p1.tile([P, EO, HL * D], BF16)
            # Startup DMA plan.  Sync carries the startup-critical wk/x0
            # stream as interleaved eighth/quarter chunks (each trigger is
            # ~600ns of queue time, each chunk 0.7-2.9us of transfer);
            # Scalar (idle all of pass 1, first free ~8.8us after its
            # activation-table load) carries everything else in whole-tile
            # triggers, ordered by first use.
            # Each HWDGE queue streams ~200 B/ns, so the two startup-
            # critical tensors (wk, x0) ride DIFFERENT engines' queues in
            # matched chunks; everything else follows in first-use order.
            xt0 = xs.tile([P, EO, ST], BF16, tag="xt", name="xt0")
            xt1 = xs.tile([P, EO, ST], BF16, tag="xt", name="xt1")
            CH0 = [(0, 2), (2, 8), (8, 16)]
            nc.sync.dma_start(wk_sb[:, 0:2, :], wkP[:, 0:2, :])
            nc.scalar.dma_start(xt0[:, 0:2, :], xP[0][:, 0:2, :])
            nc.sync.dma_start(wk_sb[:, 2:8, :], wkP[:, 2:8, :])
            nc.scalar.dma_start(tbl_sb[:, :TSMALL], tblP[:, :TSMALL])
            nc.vector.tensor_copy(qkbf[:], tbl_sb[:, ds(QK0, 2 * HL)])
            nc.sync.dma_start(wk_sb[:, 8:16, :], wkP[:, 8:16, :])
            nc.scalar.dma_start(xt0[:, 2:8, :], xP[0][:, 2:8, :])
            nc.scalar.dma_start(xt0[:, 8:16, :], xP[0][:, 8:16, :])
            nc.sync.dma_start(wv_sb[:, 0:8, :], wvP[:, 0:8, :])
            nc.sync.dma_start(wv_sb[:, 8:16, :], wvP[:, 8:16, :])
            nc.scalar.dma_start(tbl_sb[:, TSMALL:], tblP[:, TSMALL:])
            nc.scalar.dma_start(xt1[:, 0:8, :], xP[1][:, 0:8, :])
            nc.scalar.dma_start(xt1[:, 8:16, :], xP[1][:, 8:16, :])

            for i in range(NS):
                if i == 0:
                    xt = xt0
                elif i == 1:
                    xt = xt1
                else:
                    xt = xs.tile([P, EO, ST], BF16, tag="xt")
                    nc.sync.dma_start(xt[:, 0:8, :], xP[i][:, 0:8, :])
                    nc.sync.dma_start(xt[:, 8:16, :], xP[i][:, 8:16, :])
                    if i == NS - 1:
                        # prefetch pass-2 weights behind this block's x:
                        # wq feeds the q-projections later in this block,
                        # ow the out-projection a block later
                        nc.sync.dma_start(wq_sb[:, 0:8, :], wqP[:, 0:8, :])
                        nc.sync.dma_start(wq_sb[:, 8:16, :], wqP[:, 8:16, :])
                        nc.sync.dma_start(
                            ow_sb[:],
                            owT[:].rearrange("(h p) e -> p h e", p=P))
                sl = ds(i * ST, ST)
                # K-projection: block 0 walks chunk-major with one live
                # PSUM accumulator per head so the PE consumes wk/x0
                # chunks as they land; later blocks' x is fully resident.
                chunks = CH0 if i == 0 else [(0, EO)]
                kps = [psA.tile([P, ST], F32, tag="acc", name=f"kps{jb}")
                       for jb in range(HL)]
                for ci, (a, b) in enumerate(chunks):
                    for jb in range(HL):
                        for eo in range(a, b):
                            nc.tensor.matmul(
                                kps[jb][:], wk_sb[:, eo, ds(jb * D, D)],
                                xt[:, eo, :],
                                start=(eo == 0), stop=(eo == EO - 1))
                    if i == 0 and ci < 2:
                        # keep the HAM clock-gate's activity window busy
                        # while the next wk/x0 chunk lands (psS is untouched
                        # through block 0's K, so wps is still the scores
                        # pool's most recent allocation)
                        for _ in range(34 + 6 * ci):
                            nc.tensor.matmul(
                                wps[:, :P], wtile[:], wtile[:],
                                start=True, stop=True)
                kropes = []
                for jb in range(HL):
                    qb = rope_begin(kps[jb], qkb_ap(jb))
                    kropes.append((qb, i * ST, kT_sb[:, jb, sl]))
                if i < NS - 1:
                    for sbl in range(ST // P):
                        sb = i * (ST // P) + sbl
                        ps = psS.tile([P, ST], F32, tag="sc")
                        for eo in range(EO):
                            nc.tensor.matmul(
                                ps[:, : HL * D], xt[:, eo, ds(sbl * P, P)],
                                wv_sb[:, eo, :],
                                start=(eo == 0), stop=(eo == EO - 1))
                        nc.vector.tensor_tensor(
                            v_sb[:, sb, :], ps[:, : HL * D], vb_ap, Alu.add)
                    for kr in kropes:
                        rope_finish(*kr)
                else:
                    for kr in kropes:
                        rope_finish(*kr)
                    # q-projections for the first two attention heads, then
                    # V-projection interleaved with scores/exp for head 0
                    psq0 = psA.tile([P, ST], F32, tag="acc")
                    for eo in range(EO):
                        qproj_mm(0, psq0, xt, eo)
                    qt0, rf0 = qproj_fin(0, psq0)
                    psq1 = psA.tile([P, ST], F32, tag="acc")
                    for eo in range(EO):
                        qproj_mm(1, psq1, xt, eo)
                    qt1, rf1 = qproj_fin(1, psq1)
                    qtiles.extend([qt0, qt1])
                    rope_finish(*rf0)
                    att0 = (at0.tile([P, 8, ST], BF16, name="att0A"),
                            at0.tile([P, 8, ST], BF16, name="att0B"))
                    sc_jb = 0
                    vps = None
                    for vi in range(4 * EO):
                        sbl, eo = vi // EO, vi % EO
                        if eo == 0:
                            vps = psA.tile([P, ST], F32, tag="acc")
                        nc.tensor.matmul(
                            vps[:, : HL * D], xt[:, eo, ds(sbl * P, P)],
                            wv_sb[:, eo, :],
                            start=(eo == 0), stop=(eo == EO - 1))
                        if eo == EO - 1:
                            nc.vector.tensor_tensor(
                                v_sb[:, i * (ST // P) + sbl, :],
                                vps[:, : HL * D], vb_ap, Alu.add)
                        if vi >= EO and (vi - EO) % 3 == 0 and sc_jb < JT:
                            scores_mm(0, att0, qt0, sc_jb)
                            sc_jb += 1
                        if vi == 40:
                            rope_finish(*rf1)
                if i == NS - 1:
                    xt_last = xt

        # ---- pass 2: flat software pipeline over (block, head) steps ----
        # Blocks run in reverse so the first one reuses pass 1's last x
        # tile.  At step k: att@V + denominator tree for head k, scores+exp
        # for head k+1, Q-projection for head k+2, one quarter of the
        # PREVIOUS block's out-projection, and the deferred denominator
        # ones-matmul + normalize for head k-1 -- all interleaved so the PE
        # stream (65 matmuls/step) hides the exp stream (16/step).
        with tc.tile_pool(name="attpa", bufs=3) as abA, \
             tc.tile_pool(name="attpb", bufs=3) as abB:

            xts = {order[0]: xt_last}

            def cblock_mm(ci, jb, pst, drain=False):
                # one of the 16x4 out-projection matmuls for token block ci;
                # jb runs 0..63 across the block's four steps.  Each row
                # block's four 512-col quarters collect in one oc tile and
                # leave in a single [P, E] DMA.
                tile_i, ho = jb // HL, jb % HL
                sb_loc, et = tile_i // ET, tile_i % ET
                sb = ci * (ST // P) + sb_loc
                if ho == 0:
                    pst[0] = psC.tile([P, ST], F32, tag="ct", name="ct")
                nc.tensor.matmul(
                    pst[0][:], ctxT_sb[:, ho, ds(sb * P, P)],
                    ow_sb[:, ho, ds(et * ST, ST)],
                    start=(ho == 0), stop=(ho == HL - 1))
                if ho == HL - 1:
                    if et == 0:
                        pst[1] = oc.tile([P, ET, ST], BF16, tag="ot",
                                         name="ot")
                    nc.vector.tensor_copy(pst[1][:, et, :], pst[0][:])
                    if drain and sb_loc == ST // P - 1:
                        # final row block: one small DMA per quarter right
                        # after its copy, alternating trigger engines, so
                        # the post-matmul tail transfer is short
                        eng = nc.scalar if (et % 2) else nc.sync
                        eng.dma_start(
                            out[ds(sb * P, P), ds(et * ST, ST)],
                            pst[1][:, et, :])
                    elif et == ET - 1:
                        if not drain:
                            nc.sync.dma_start(
                                out[ds(sb * P, P), :], pst[1][:])
                        else:
                            eng = nc.scalar if (sb_loc % 2) else nc.sync
                            eng.dma_start(out[ds(sb * P, P), :], pst[1][:])

            # denominator ones-matmul + normalize for head k; deferred one
            # step so the PE reaches the ones-matmul well after the VectorE
            # tree produced attB[:, 0, :].  1/d = Exp(-Ln(d)) on ScalarE
            # (ln/exp share an activation table: no reloads).  Split into
            # pieces so the last two steps can weave them into the matmul
            # stream instead of serializing after it.
            def finish_mm(attab):
                psd = psC.tile([P, ST], F32, tag="ct", name="psd")
                nc.tensor.matmul(
                    psd[:], ones_ap, attab[1][:, 0, :],
                    start=True, stop=True)
                return psd

            def finish_ln(psd):
                lnd = dp.tile([P, ST], F32, tag="lnd")
                nc.scalar.activation(lnd[:], psd[:], Act.Ln)
                return lnd

            def finish_exp(lnd):
                rec = dp.tile([P, ST], F32, tag="rec")
                nc.scalar.activation(rec[:], lnd[:], Act.Exp, scale=-1.0)
                return rec

            def finish_tt(k, psc, rec):
                nc.vector.tensor_tensor(
                    ctxT_sb[:, seq[k][1], ds(blk(k) * ST, ST)],
                    psc[:], rec[:], Alu.mult)

            def finish(k, attab, psc):
                finish_tt(k, psc, finish_exp(finish_ln(finish_mm(attab))))

            atts = [att0]

            cpst = [None, None]
            pending = None
            for k in range(NK):
                i, h = seq[k]
                if h == 0 and k + 4 < NK:
                    # prefetch the x tile for the NEXT block now; the DMA
                    # has a whole block (~55us) to land
                    nxt = blk(k + 4)
                    xtn = xs.tile([P, EO, ST], BF16, tag="xt")
                    nc.sync.dma_start(xtn[:], xP[nxt][:])
                    xts[nxt] = xtn
                att = atts[k]
                attA, attB = att
                ci_prev = blk(k - 4) if k >= 4 else None
                if k + 1 < NK:
                    attn = (abA.tile([P, 8, ST], BF16, tag="attA", name="attA"),
                            abB.tile([P, 8, ST], BF16, tag="attB", name="attB"))
                    atts.append(attn)
                else:
                    attn = None
                if k + 2 < NK:
                    psq = psA.tile([P, ST], F32, tag="acc")
                else:
                    psq = None
                psc = psA.tile([P, ST], F32, tag="acc")
                last = (k == NK - 1)
                # block-closing steps reduce the tree two slots early so
                # the finish-NOW chain lands sooner in the next step
                tree_at = [7, 9, 11, 13] if (h == HL - 1 and not last) \
                    else [7, 11, 13, 15]
                held = []
                psd15 = None
                for idx in range(JT):
                    if attn is not None:
                        scores_mm(k + 1, attn, qtiles[k + 1], idx)
                    jb = (idx + 8) % JT     # att@V: B half first
                    avs = (attA, attB)[jb // 8][:, jb % 8, :]
                    nc.tensor.matmul(
                        psc[:], v_sb[:, jb, ds(h * D, D)], avs,
                        start=(idx == 0), stop=(idx == JT - 1))
                    if psq is not None:
                        qproj_mm(k + 2, psq, xts[blk(k + 2)], idx)
                    if ci_prev is not None:
                        if k % 4 != 0:
                            if last and idx >= 4:
                                # hold the tail of the previous block's
                                # out-projection so it can hide the final
                                # normalize's Ln/Exp latency (below)
                                held.append((k % 4) * JT + idx)
                            else:
                                cblock_mm(ci_prev, (k % 4) * JT + idx, cpst)
                        elif idx >= 8:
                            # block-boundary step: the previous block's ctx
                            # normalize lands ~1us in, so weave its out-
                            # projection into the back half, two per slot
                            cblock_mm(ci_prev, (idx - 8) * 2, cpst)
                            cblock_mm(ci_prev, (idx - 8) * 2 + 1, cpst)
                    if not last:
                        if pending is not None and idx == 2:
                            pk, patt, ppsc = pending
                            p_psd = finish_mm(patt)
                        elif pending is not None and idx == 3:
                            p_lnd = finish_ln(p_psd)
                        elif pending is not None and idx == 4:
                            p_rec = finish_exp(p_lnd)
                        elif pending is not None and idx == 5:
                            finish_tt(pk, ppsc, p_rec)
                            pending = None
                        if h == HL - 1:
                            # block-closing step: own-head ones-matmul/Ln
                            # woven into idx 14-15 so ctx is final ~1us
                            # into the next (boundary) step
                            if idx == 14:
                                now_psd = finish_mm(att)
                            elif idx == 15:
                                now_lnd = finish_ln(now_psd)
                        # denominator tree levels woven into the matmul
                        # stream; they only ever write attB, whose att@V
                        # reads all finished at idx 7
                        if idx == tree_at[0]:
                            nc.vector.tensor_tensor(
                                attB[:], attB[:], attA[:], Alu.add)
                        elif idx == tree_at[1]:
                            nc.vector.tensor_tensor(
                                attB[:, 0:4, :], attB[:, 0:4, :],
                                attB[:, 4:8, :], Alu.add)
                        elif idx == tree_at[2]:
                            nc.vector.tensor_tensor(
                                attB[:, 0:2, :], attB[:, 0:2, :],
                                attB[:, 2:4, :], Alu.add)
                        elif idx == tree_at[3]:
                            nc.vector.tensor_tensor(
                                attB[:, 0, :], attB[:, 0, :], attB[:, 1, :],
                                Alu.add)
                    else:
                        # LAST step: nothing interleaves after it, so the
                        # usual deferred chains would serialize behind the
                        # matmul stream.  Weave the pending head's finish
                        # into idx 0-3, halve the tree (level 1 only, the
                        # first half written into step NK-2's retired attB
                        # so the coarse dep-tracker sees no write to a tile
                        # the PE still reads), and let the PE itself do the
                        # final 8-way reduction as an accumulating
                        # ones-matmul group in idx 8-15.
                        attB_prev = atts[NK - 2][1]
                        if idx == 0:
                            pk, patt, ppsc = pending
                            p_psd = finish_mm(patt)
                        elif idx == 1:
                            p_lnd = finish_ln(p_psd)
                        elif idx == 2:
                            p_rec = finish_exp(p_lnd)
                        elif idx == 3:
                            finish_tt(pk, ppsc, p_rec)
                            pending = None
                        elif idx == 4:
                            nc.vector.tensor_tensor(
                                attB_prev[:, 0:4, :], attB[:, 0:4, :],
                                attA[:, 0:4, :], Alu.add)
                        elif idx == 7:
                            nc.vector.tensor_tensor(
                                attB[:, 4:8, :], attB[:, 4:8, :],
                                attA[:, 4:8, :], Alu.add)
                        if idx >= 8:
                            s = idx - 8
                            src = attB_prev[:, s, :] if s < 4 \
                                else attB[:, s, :]
                            if s == 0:
                                psd15 = psA.tile([P, ST], F32, tag="acc",
                                                 name="psd15")
                            nc.tensor.matmul(
                                psd15[:], ones_ap, src,
                                start=(s == 0), stop=(s == 7))
                if psq is not None:
                    qt, rf = qproj_fin(k + 2, psq)
                    qtiles.append(qt)
                else:
                    rf = None
                if pending is not None:
                    finish(*pending)
                    pending = None
                if h == HL - 1:
                    # last head of the block: finish NOW so the next
                    # block's interleaved out-projection reads final ctx
                    if last:
                        finish_tt(k, psc, finish_exp(finish_ln(psd15)))
                    else:
                        finish_tt(k, psc, finish_exp(now_lnd))
                    for jb in held:
                        cblock_mm(ci_prev, jb, cpst)
                else:
                    pending = (k, att, psc)
                if rf is not None:
                    rope_finish(*rf)

            # the last block's out-projection has no next block to hide in
            cpst = [None, None]
            for jb in range(4 * JT - 4):
                cblock_mm(blk(NK - 1), jb, cpst, drain=True)
            # final tile: accumulate the two column halves as separate
            # groups so the first half's copy+DMA overlaps the second
            # half's matmuls and the tail transfer is only [P, 256]
            ci = blk(NK - 1)
            sb = ci * (ST // P) + 3
            for hf in range(2):
                psf = psC.tile([P, ST // 2], F32, tag="ct", name="ctf")
                for ho in range(HL):
                    nc.tensor.matmul(
                        psf[:], ctxT_sb[:, ho, ds(sb * P, P)],
                        ow_sb[:, ho, ds(3 * ST + hf * (ST // 2), ST // 2)],
                        start=(ho == 0), stop=(ho == HL - 1))
                otf = oc.tile([P, ST // 2], BF16, tag="otf", name="otf")
                nc.vector.tensor_copy(otf[:], psf[:])
                eng = nc.scalar if hf else nc.sync
                eng.dma_start(
                    out[ds(sb * P, P), ds(3 * ST + hf * (ST // 2), ST // 2)],
                    otf[:])

    return nc


def _rope_tables():
    inv_freq = 1.0 / (10000.0 ** (np.arange(0, D, 2, dtype=np.float32) / D))
    t = np.arange(S, dtype=np.float32)
    freqs = np.einsum("s,f->sf", t, inv_freq)
    emb = np.concatenate([freqs, freqs], axis=-1)
    cosT = np.cos(emb).astype(np.float32).T.copy()
    sinT = np.sin(emb).astype(np.float32).T.copy()
    # rotate-half sign lives in the on-device permutation matrix
    return cosT.astype(BF), np.ascontiguousarray(sinT).astype(BF)


def _core_inputs(x, Wqkv_w, Wqkv_b, out_w, b, g, shared, xT_bf):
    # k-head columns first, then q-head columns (matches kernel layout)
    k_cols, q_cols, kb_rows, qb_rows = [], [], [], []
    for hl in range(HL):
        h = g * HL + hl
        q_cols.append(Wqkv_w[h * D:(h + 1) * D, :].T)
        k_cols.append(Wqkv_w[E + h * D:E + (h + 1) * D, :].T)
        qb_rows.append(Wqkv_b[h * D:(h + 1) * D])
        kb_rows.append(Wqkv_b[E + h * D:E + (h + 1) * D])
    def pack(wT):
        # [E, HL*D] -> [P, EO, HL*D]: per-partition contiguous rows so
        # the whole tensor moves in one (or few) large-descriptor DMAs
        return np.ascontiguousarray(
            wT.reshape(E // P, P, HL * D).transpose(1, 0, 2)).astype(BF)

    wkP = pack(np.concatenate(k_cols, axis=1))
    wqP = pack(np.concatenate(q_cols, axis=1))
    qkbT = np.stack(kb_rows + qb_rows).astype(np.float32).T    # [D, 2HL]
    v0 = 2 * E + g * HL * D
    wvP = pack(Wqkv_w[v0:v0 + HL * D, :].T)
    vb = Wqkv_b[v0:v0 + HL * D].astype(np.float32)
    owT = np.ascontiguousarray(
        out_w[:, g * HL * D:(g + 1) * HL * D].T).astype(BF)
    cosT, sinT, ones, perm = shared
    # qkb | vb | ones | perm | cos | sin (kernel's tbl layout); biases in
    # bf16 cost ~0.4% of their 0.01-scale values -- negligible
    tblP = np.ascontiguousarray(np.concatenate(
        [qkbT, np.broadcast_to(vb[None, :], (P, HL * D)), ones, perm,
         cosT, sinT], axis=1).astype(np.float32)).astype(BF)
    return {"xP": xT_bf, "wkP": wkP, "wqP": wqP, "wvP": wvP,
            "tblP": tblP, "owT": owT}


def kernel(x, Wqkv_w, Wqkv_b, out_w, out_b):
    global LAST_EXEC_NS
    _install_axon_ntff_shim()
    from concourse.bass_utils import run_bass_kernel_spmd

    x = np.asarray(x, dtype=np.float32)
    Wqkv_w = np.asarray(Wqkv_w, dtype=np.float32)
    Wqkv_b = np.asarray(Wqkv_b, dtype=np.float32)
    out_w = np.asarray(out_w, dtype=np.float32)
    out_b = np.asarray(out_b, dtype=np.float32)

    cosT, sinT = _rope_tables()
    # rotate-half permutation: out[d] = -q[d+64] (d<64), +q[d-64] (d>=64)
    perm = np.zeros((P, P), dtype=np.float32)
    for d in range(D // 2):
        perm[d + D // 2, d] = -1.0
        perm[d, d + D // 2] = 1.0
    shared = (cosT.astype(np.float32), sinT.astype(np.float32),
              np.ones((P, P), np.float32), perm)
    # x packed as [NS, P, EO, ST]: xP[i, p, eo, s] = x[b, i*ST+s, eo*P+p]
    NS, EO, ST = S // 512, E // P, 512
    xT_bf = [np.ascontiguousarray(
        x[b].reshape(NS, ST, EO, P).transpose(0, 3, 2, 1)).astype(BF)
        for b in range(2)]
    in_maps = []
    for core in range(8):
        b, g = core // 4, core % 4
        in_maps.append(
            _core_inputs(x, Wqkv_w, Wqkv_b, out_w, b, g, shared, xT_bf[b]))

    nc = bass.Bass()
    _build_mha(nc)
    _split_multi_waits(nc)

    trace = bool(os.environ.get("MHA_TRACE"))
    if trace:
        # dev-only profiling path; skip the S3 artifact upload
        import concourse.bass_utils as _bu
        _bu.upload_artifacts = lambda tmpdir: tmpdir
    res = run_bass_kernel_spmd(
        nc, in_maps, core_ids=list(range(8)), trace=trace)
    if trace:
        LAST_EXEC_NS = res.exec_time_ns

    out = np.empty((2, S, E), dtype=np.float32)
    for b in range(2):
        acc = res.results[b * 4 + 0]["out"].astype(np.float32)
        for g in range(1, 4):
            acc += res.results[b * 4 + g]["out"].astype(np.float32)
        out[b] = acc + out_b[None, :]
    return out


# revision 23
# speedup vs baseline: 1.0042x; 1.0042x over previous
"""Sharded MHA-with-RoPE Trainium2 kernel (nn_CustomTorchMHASelf).

Contract: kernel(**inputs) takes the FULL unsharded inputs of the
reference (x [2,2048,2048], Wqkv_w [6144,2048], Wqkv_b [6144],
out_w [2048,2048], out_b [2048]) and returns the full [2,2048,2048]
fp32 output, running the compute on 8 NeuronCores.

Sharding: core = b*4 + g handles batch b and head-group g (4 of the 16
heads). Each core computes q/k/v projections for its heads, RoPE,
softmax attention, and its slice of the out-projection; the host sums
the 4 partial outputs per batch and adds out_b.

Device data plane is bf16 (fp32 PSUM accumulation); the host
pre-transposes x and the weight slices into the layouts the TensorE
wants (contraction dim on partitions everywhere).

Schedule: pass 1 computes K+RoPE and V for all tokens (the last block
also hides the attention prologue under its V-projection); pass 2 is a
flat software pipeline over (block, head) steps -- at step k the PE
stream interleaves att@V(k), scores(k+1), q-projection(k+2) and a
quarter of the previous block's out-projection (65 matmuls/step), so
the ScalarE exp stream (16/step) is never on the critical path.
Key device tricks:
  - DMA triggers cost ~600ns of issuing-engine queue time EACH
    regardless of size, so tensors move with ONE trigger per tile
    (16KB-per-partition contiguous descriptor runs) instead of 16;
    the startup-critical wk/x0 pair is split into eighth/quarter
    chunks issued alternately so the first K matmuls start ~10.5us
    in, and block 0's K-projection walks chunk-major (4 live PSUM
    accumulators) to consume chunks as they land;
  - rotate-half for RoPE is a PE matmul with a signed permutation
    matrix (SBUF-SBUF DMA swaps are slow and their DIRECT2D triggers
    serialize on the sync sequencer);
  - the softmax denominator is a bf16 tree-add into the attB tile on
    VectorE plus ONE ones-matmul per (head, block) instead of 16 full
    PE ones-matmuls; att is split into two tiles (attA/attB) so the
    tree's writes never alias tiles the PE still reads (the dep
    tracker is coarse); the ones-matmul+normalize are deferred one
    step so the PE never waits on the tree;
  - 1/denominator = Exp(-Ln(d)) on ScalarE (ln and exp share an
    activation table, so no table reloads) because DVE reciprocal is
    slow and custom-DVE ops don't compile on this toolchain;
  - deferred finishes are woven into matmul slots 2-5 of the next step
    (not appended after it) so their Ln/Exp never delays the next
    step's first exps, whose psS banks the score matmuls wait on;
  - warm-up matmuls on a memset tile (no DMA dependency) ramp the PE
    out of its 1.2GHz cold p-state while the first weight/x DMAs land,
    and filler matmuls after the first two K chunks keep the HAM
    activity window busy so the clock never re-throttles mid-startup;
  - output tiles are written bf16, one batched [P, E] DMA per token
    row-block; in the last step the tree is halved (level 1 written
    into a retired att tile), the PE itself does the final 8-way
    denominator reduction as an accumulating ones-matmul group, and
    the drain's final tile is split into column halves so the tail
    transfer is short.
"""

import math
import os
import sys
import types

import numpy as np
import ml_dtypes

import concourse.bass as bass
import concourse.mybir as mybir
import concourse.tile as tile
from concourse.bass import ds

F32 = mybir.dt.float32
BF16 = mybir.dt.bfloat16
Alu = mybir.AluOpType
Act = mybir.ActivationFunctionType
BF = ml_dtypes.bfloat16

S, E, HTOT, HL, D, P = 2048, 2048, 16, 4, 128, 128

# Filled with the profile exec time (ns) when MHA_TRACE=1; read by test.py.
LAST_EXEC_NS = None


def _install_axon_ntff_shim():
    """Provide antenv.axon_hooks so trace=True can reach the axon NTFF hook."""
    if "antenv.axon_hooks" in sys.modules:
        return
    mod = types.ModuleType("antenv.axon_hooks")
    holder = [None]
    mod.set_axon_ntff_profile_hook = lambda h: holder.__setitem__(0, h)
    mod.get_axon_ntff_profile_hook = lambda: holder[0]
    sys.modules["antenv.axon_hooks"] = mod
    try:
        import antenv
        antenv.axon_hooks = mod
    except ImportError:
        pass
    # boot() ran at interpreter start (sitecustomize), before this module
    # existed, so its NTFF-hook registration was silently skipped. Redo it.
    try:
        from trn_agent_boot.trn_boot import _ntff_profile_via_ctypes
        hook = _ntff_profile_via_ctypes("/opt/axon/libaxon_pjrt.so")
        if hook is not None:
            mod.set_axon_ntff_profile_hook(hook)
    except Exception:
        pass


def _split_multi_waits(nc):
    """Hoist extra sem-waits onto standalone NoOps (one wait per inst).

    This walrus build rejects any instruction carrying more than one
    sync-wait ("Too many sync wait commands"); Tile attaches one wait per
    outstanding semaphore to the consuming instruction. Splitting them
    across same-engine NoOps placed immediately before is equivalent:
    the engine executes serially, so all waits still precede the inst.
    """
    ctr = 0
    for fn in nc.m.functions:
        for blk in fn.blocks:
            out = []
            for inst in blk.instructions:
                si = getattr(inst, "sync_info", None)
                if si is not None and si.on_wait is not None \
                        and len(si.on_wait) > 1:
                    waits = list(si.on_wait)
                    si.on_wait = [waits[-1]]
                    for w in waits[:-1]:
                        ctr += 1
                        nop = mybir.InstNoOp(
                            name=f"I-wsplit-{ctr}", ins=[], outs=[])
                        nop.engine = inst.engine
                        nop.sync_info = mybir.SyncInfo(
                            on_wait=[w], on_update=[])
                        out.append(nop)
                out.append(inst)
            blk.instructions[:] = out


def _build_mha(nc: bass.Bass):
    """Emit the per-core MHA program (one shard) into `nc`."""
    EO = E // P            # contraction subtiles for the projections
    ST = 512               # free-dim tile (one PSUM bank of fp32)
    NS = S // ST
    SB = S // P
    JT = S // P            # key blocks per head
    ET = E // ST

    # packed layouts: [.., P, EO, ST] so each tensor is one long
    # per-partition-contiguous run -> ONE ~600ns DMA trigger moves it
    xP = nc.dram_tensor("xP", [NS, P, EO, ST], BF16, kind="ExternalInput")
    wkP = nc.dram_tensor("wkP", [P, EO, HL * D], BF16, kind="ExternalInput")
    wqP = nc.dram_tensor("wqP", [P, EO, HL * D], BF16, kind="ExternalInput")
    wvP = nc.dram_tensor("wvP", [P, EO, HL * D], BF16, kind="ExternalInput")
    # qkb | vb | ones | perm | cos | sin packed into one bf16 table;
    # the small head (biases + matrices) rides an early DMA slice, the
    # big cos/sin tail a later one
    TW = 2 * HL + HL * D + 2 * P + 2 * S
    tblP = nc.dram_tensor("tblP", [P, TW], BF16, kind="ExternalInput")
    owT = nc.dram_tensor("owT", [HL * D, E], BF16, kind="ExternalInput")
    out = nc.dram_tensor("out", [S, E], BF16, kind="ExternalOutput")

    isc = 1.0 / math.sqrt(D)

    from contextlib import ExitStack

    with tile.TileContext(nc) as tc, ExitStack() as stk:
        persist = stk.enter_context(tc.tile_pool(name="persist", bufs=1))
        kT_sb = persist.tile([P, HL, S], BF16)      # k post-RoPE [d, h, s]
        v_sb = persist.tile([P, SB, HL * D], BF16)  # v natural [s%128, s//128, hd]
        ctxT_sb = persist.tile([P, HL, S], BF16)    # [d, h, i]
        tbl_sb = persist.tile([P, TW], BF16)
        ow_sb = persist.tile([P, HL, E], BF16)
        wtile = persist.tile([P, P], BF16)          # memset warm-up operand

        QK0, VB0 = 0, 2 * HL
        ON0 = VB0 + HL * D
        PM0 = ON0 + P
        CS0, SN0 = PM0 + P, PM0 + P + S
        TSMALL = PM0 + P          # early slice: biases + ones + perm

        # tensor_scalar needs f32 scalars: widen the bf16 biases once
        qkbf = persist.tile([P, 2 * HL], F32)

        def qkb_ap(j):
            return qkbf[:, j, None]

        vb_ap = tbl_sb[:, ds(VB0, HL * D)]

        def cos_ap(s0):
            return tbl_sb[:, ds(CS0 + s0, ST)]

        def sin_ap(s0):
            return tbl_sb[:, ds(SN0 + s0, ST)]

        ones_ap = tbl_sb[:, ds(ON0, P)]
        perm_ap = tbl_sb[:, ds(PM0, P)]

        # x stream shared by both passes; rope temps likewise.  qb/rot are
        # still being read (by the rotate matmul / mults) when the next rope
        # starts, so they get extra bufs; t1/t2 are consumed immediately by
        # the in-order VectorE queue, so 1 buf suffices.
        xs = stk.enter_context(tc.tile_pool(name="xstream", bufs=2))
        rta = stk.enter_context(tc.tile_pool(name="ropetmpa", bufs=3))
        rtb = stk.enter_context(tc.tile_pool(name="ropetmpb", bufs=1))
        wqp = stk.enter_context(tc.tile_pool(name="wqpool", bufs=1))
        wq_sb = wqp.tile([P, EO, HL * D], BF16)

        psA = stk.enter_context(tc.tile_pool(name="psA", bufs=4, space="PSUM"))
        psS = stk.enter_context(tc.tile_pool(name="psS", bufs=2, space="PSUM"))
        psC = stk.enter_context(tc.tile_pool(name="psC", bufs=2, space="PSUM"))

        qp = stk.enter_context(tc.tile_pool(name="qpool", bufs=4))
        dp = stk.enter_context(tc.tile_pool(name="denp", bufs=1))
        oc = stk.enter_context(tc.tile_pool(name="ocopy", bufs=2))
        at0 = stk.enter_context(tc.tile_pool(name="att0p", bufs=1))

        # flat (block, head) schedule for the attention pass; blocks in
        # reverse order so the first one reuses pass 1's last x tile
        order = list(range(NS - 1, -1, -1))
        seq = [(i, h) for i in order for h in range(HL)]
        NK = len(seq)

        def blk(k):
            return seq[k][0]

        def rope_begin(ps, bias_ap):
            # qb = q + bias (bf16 so the rotate-half matmul runs full rate)
            qb = rta.tile([P, ST], BF16, tag="qb")
            nc.vector.tensor_scalar_add(qb[:], ps[:], bias_ap)
            return qb

        def rope_finish(qb, s0, out_ap):
            # rotate-half as a PE matmul with a signed permutation matrix
            # (cross-partition moves otherwise need a slow SBUF-SBUF DMA
            # whose trigger also serializes on the sync sequencer);
            # out = qb*cos + (perm.T @ qb)*sin.
            # rps lives in psC so the scores stream owns psS's two banks
            # outright (16 even allocations per step -> the exp reader is
            # always 2 full matmul-slots behind the next allocation)
            rps = psC.tile([P, ST], F32, tag="ct", name="rps")
            nc.tensor.matmul(rps[:], perm_ap, qb[:], start=True, stop=True)
            t1 = rtb.tile([P, ST], BF16, tag="t1")
            t2 = rtb.tile([P, ST], BF16, tag="t2")
            nc.vector.tensor_tensor(t1[:], qb[:], cos_ap(s0), Alu.mult)
            nc.vector.tensor_tensor(t2[:], rps[:], sin_ap(s0), Alu.mult)
            nc.vector.tensor_tensor(out_ap, t1[:], t2[:], Alu.add)

        def qproj_mm(k, psq, xt, eo):
            h = seq[k][1]
            nc.tensor.matmul(
                psq[:], wq_sb[:, eo, ds(h * D, D)], xt[:, eo, :],
                start=(eo == 0), stop=(eo == EO - 1))

        def qproj_fin(k, psq):
            qt = qp.tile([P, ST], BF16, tag="qt")
            qb = rope_begin(psq, qkb_ap(HL + seq[k][1]))
            return qt, (qb, blk(k) * ST, qt[:])

        def scores_mm(k, attab, qt, jb):
            h = seq[k][1]
            ps = psS.tile([P, ST], F32, tag="sc")
            nc.tensor.matmul(
                ps[:], kT_sb[:, h, ds(jb * P, P)], qt[:],
                start=True, stop=True)
            dst = attab[jb // 8][:, jb % 8, :]
            nc.scalar.activation(dst, ps[:], Act.Exp, scale=isc)

        # PE warm-up on a memset tile (no DMA dependency): the HAM clock
        # gate needs ~3.4us of sustained PE activity to release the cold
        # 1.2GHz p-state, and the first real matmul data lands ~10.5us in.
        nc.gpsimd.memset(wtile[:], 1.0)
        wsink = persist.tile([P, 1], F32)
        wps = psS.tile([P, ST], F32, tag="sc", name="warmps")
        NWARM = 44
        for w in range(NWARM):
            nc.tensor.matmul(
                wps[:, :P], wtile[:], wtile[:],
                start=(w == 0), stop=(w == NWARM - 1))
        nc.vector.tensor_copy(wsink[:], wps[:, :1])

        # ---- pass 1: K projection + RoPE, V projection ----
        # The last block additionally hides the attention pass's prologue
        # (q heads 0/1 + scores/exp for head 0) under its V-projection.
        xt_last = None
        att0 = None
        qtiles = []
        with tc.tile_pool(name="p1w", bufs=1) as p1:
            wk_sb = p1.tile([P, EO, HL * D], BF16)
            wv_sb = p1.tile([P, EO, HL * D], BF16)
            # Startup DMA plan.  Sync carries the startup-critical wk/x0
            # stream as interleaved eighth/quarter chunks (each trigger is
            # ~600ns of queue time, each chunk 0.7-2.9us of transfer);
            # Scalar (idle all of pass 1, first free ~8.8us after its
            # activation-table load) carries everything else in whole-tile
            # triggers, ordered by first use.
            # Each HWDGE queue streams ~200 B/ns, so the two startup-
            # critical tensors (wk, x0) ride DIFFERENT engines' queues in
            # matched chunks; everything else follows in first-use order.
            xt0 = xs.tile([P, EO, ST], BF16, tag="xt", name="xt0")
            xt1 = xs.tile([P, EO, ST], BF16, tag="xt", name="xt1")
            CH0 = [(0, 2), (2, 8), (8, 16)]
            nc.sync.dma_start(wk_sb[:, 0:2, :], wkP[:, 0:2, :])
            nc.scalar.dma_start(xt0[:, 0:2, :], xP[0][:, 0:2, :])
            nc.sync.dma_start(wk_sb[:, 2:8, :], wkP[:, 2:8, :])
            nc.scalar.dma_start(tbl_sb[:, :TSMALL], tblP[:, :TSMALL])
            nc.vector.tensor_copy(qkbf[:], tbl_sb[:, ds(QK0, 2 * HL)])
            nc.sync.dma_start(wk_sb[:, 8:16, :], wkP[:, 8:16, :])
            nc.scalar.dma_start(xt0[:, 2:8, :], xP[0][:, 2:8, :])
            nc.scalar.dma_start(xt0[:, 8:16, :], xP[0][:, 8:16, :])
            nc.sync.dma_start(wv_sb[:, 0:8, :], wvP[:, 0:8, :])
            nc.sync.dma_start(wv_sb[:, 8:16, :], wvP[:, 8:16, :])
            nc.scalar.dma_start(tbl_sb[:, TSMALL:], tblP[:, TSMALL:])
            nc.scalar.dma_start(xt1[:, 0:8, :], xP[1][:, 0:8, :])
            nc.scalar.dma_start(xt1[:, 8:16, :], xP[1][:, 8:16, :])

            for i in range(NS):
                if i == 0:
                    xt = xt0
                elif i == 1:
                    xt = xt1
                else:
                    xt = xs.tile([P, EO, ST], BF16, tag="xt")
                    nc.sync.dma_start(xt[:, 0:8, :], xP[i][:, 0:8, :])
                    nc.sync.dma_start(xt[:, 8:16, :], xP[i][:, 8:16, :])
                    if i == NS - 1:
                        # prefetch pass-2 weights behind this block's x:
                        # wq feeds the q-projections later in this block,
                        # ow the out-projection a block later
                        nc.sync.dma_start(wq_sb[:, 0:8, :], wqP[:, 0:8, :])
                        nc.sync.dma_start(wq_sb[:, 8:16, :], wqP[:, 8:16, :])
                        nc.sync.dma_start(
                            ow_sb[:],
                            owT[:].rearrange("(h p) e -> p h e", p=P))
                sl = ds(i * ST, ST)
                # K-projection: block 0 walks chunk-major with one live
                # PSUM accumulator per head so the PE consumes wk/x0
                # chunks as they land; later blocks' x is fully resident.
                chunks = CH0 if i == 0 else [(0, EO)]
                kps = [psA.tile([P, ST], F32, tag="acc", name=f"kps{jb}")
                       for jb in range(HL)]
                for ci, (a, b) in enumerate(chunks):
                    for jb in range(HL):
                        for eo in range(a, b):
                            nc.tensor.matmul(
                                kps[jb][:], wk_sb[:, eo, ds(jb * D, D)],
                                xt[:, eo, :],
                                start=(eo == 0), stop=(eo == EO - 1))
                    if i == 0 and ci < 2:
                        # keep the HAM clock-gate's activity window busy
                        # while the next wk/x0 chunk lands (psS is untouched
                        # through block 0's K, so wps is still the scores
                        # pool's most recent allocation)
                        for _ in range(34 + 6 * ci):
                            nc.tensor.matmul(
                                wps[:, :P], wtile[:], wtile[:],
                                start=True, stop=True)
                kropes = []
                for jb in range(HL):
                    qb = rope_begin(kps[jb], qkb_ap(jb))
                    kropes.append((qb, i * ST, kT_sb[:, jb, sl]))
                if i < NS - 1:
                    for sbl in range(ST // P):
                        sb = i * (ST // P) + sbl
                        ps = psS.tile([P, ST], F32, tag="sc")
                        for eo in range(EO):
                            nc.tensor.matmul(
                                ps[:, : HL * D], xt[:, eo, ds(sbl * P, P)],
                                wv_sb[:, eo, :],
                                start=(eo == 0), stop=(eo == EO - 1))
                        nc.vector.tensor_tensor(
                            v_sb[:, sb, :], ps[:, : HL * D], vb_ap, Alu.add)
                    for kr in kropes:
                        rope_finish(*kr)
                else:
                    for kr in kropes:
                        rope_finish(*kr)
                    # q-projections for the first two attention heads, then
                    # V-projection interleaved with scores/exp for head 0
                    psq0 = psA.tile([P, ST], F32, tag="acc")
                    for eo in range(EO):
                        qproj_mm(0, psq0, xt, eo)
                    qt0, rf0 = qproj_fin(0, psq0)
                    psq1 = psA.tile([P, ST], F32, tag="acc")
                    for eo in range(EO):
                        qproj_mm(1, psq1, xt, eo)
                    qt1, rf1 = qproj_fin(1, psq1)
                    qtiles.extend([qt0, qt1])
                    rope_finish(*rf0)
                    att0 = (at0.tile([P, 8, ST], BF16, name="att0A"),
                            at0.tile([P, 8, ST], BF16, name="att0B"))
                    sc_jb = 0
                    vps = None
                    for vi in range(4 * EO):
                        sbl, eo = vi // EO, vi % EO
                        if eo == 0:
                            vps = psA.tile([P, ST], F32, tag="acc")
                        nc.tensor.matmul(
                            vps[:, : HL * D], xt[:, eo, ds(sbl * P, P)],
                            wv_sb[:, eo, :],
                            start=(eo == 0), stop=(eo == EO - 1))
                        if eo == EO - 1:
                            nc.vector.tensor_tensor(
                                v_sb[:, i * (ST // P) + sbl, :],
                                vps[:, : HL * D], vb_ap, Alu.add)
                        if vi >= EO and (vi - EO) % 3 == 0 and sc_jb < JT:
                            scores_mm(0, att0, qt0, sc_jb)
                            sc_jb += 1
                        if vi == 40:
                            rope_finish(*rf1)
                if i == NS - 1:
                    xt_last = xt

        # ---- pass 2: flat software pipeline over (block, head) steps ----
        # Blocks run in reverse so the first one reuses pass 1's last x
        # tile.  At step k: att@V + denominator tree for head k, scores+exp
        # for head k+1, Q-projection for head k+2, one quarter of the
        # PREVIOUS block's out-projection, and the deferred denominator
        # ones-matmul + normalize for head k-1 -- all interleaved so the PE
        # stream (65 matmuls/step) hides the exp stream (16/step).
        with tc.tile_pool(name="attpa", bufs=3) as abA, \
             tc.tile_pool(name="attpb", bufs=3) as abB:

            xts = {order[0]: xt_last}

            def cblock_mm(ci, jb, pst, drain=False):
                # one of the 16x4 out-projection matmuls for token block ci;
                # jb runs 0..63 across the block's four steps.  Each row
                # block's four 512-col quarters collect in one oc tile and
                # leave in a single [P, E] DMA.
                tile_i, ho = jb // HL, jb % HL
                sb_loc, et = tile_i // ET, tile_i % ET
                sb = ci * (ST // P) + sb_loc
                if ho == 0:
                    pst[0] = psC.tile([P, ST], F32, tag="ct", name="ct")
                nc.tensor.matmul(
                    pst[0][:], ctxT_sb[:, ho, ds(sb * P, P)],
                    ow_sb[:, ho, ds(et * ST, ST)],
                    start=(ho == 0), stop=(ho == HL - 1))
                if ho == HL - 1:
                    if et == 0:
                        pst[1] = oc.tile([P, ET, ST], BF16, tag="ot",
                                         name="ot")
                    nc.vector.tensor_copy(pst[1][:, et, :], pst[0][:])
                    if drain and sb_loc == ST // P - 1:
                        # final row block: one small DMA per quarter right
                        # after its copy, alternating trigger engines, so
                        # the post-matmul tail transfer is short
                        eng = nc.scalar if (et % 2) else nc.sync
                        eng.dma_start(
                            out[ds(sb * P, P), ds(et * ST, ST)],
                            pst[1][:, et, :])
                    elif et == ET - 1:
                        if not drain:
                            nc.sync.dma_start(
                                out[ds(sb * P, P), :], pst[1][:])
                        else:
                            eng = nc.scalar if (sb_loc % 2) else nc.sync
                            eng.dma_start(out[ds(sb * P, P), :], pst[1][:])

            # denominator ones-matmul + normalize for head k; deferred one
            # step so the PE reaches the ones-matmul well after the VectorE
            # tree produced attB[:, 0, :].  1/d = Exp(-Ln(d)) on ScalarE
            # (ln/exp share an activation table: no reloads).  Split into
            # pieces so the last two steps can weave them into the matmul
            # stream instead of serializing after it.
            def finish_mm(attab):
                psd = psC.tile([P, ST], F32, tag="ct", name="psd")
                nc.tensor.matmul(
                    psd[:], ones_ap, attab[1][:, 0, :],
                    start=True, stop=True)
                return psd

            def finish_ln(psd):
                lnd = dp.tile([P, ST], F32, tag="lnd")
                nc.scalar.activation(lnd[:], psd[:], Act.Ln)
                return lnd

            def finish_exp(lnd):
                rec = dp.tile([P, ST], F32, tag="rec")
                nc.scalar.activation(rec[:], lnd[:], Act.Exp, scale=-1.0)
                return rec

            def finish_tt(k, psc, rec):
                nc.vector.tensor_tensor(
                    ctxT_sb[:, seq[k][1], ds(blk(k) * ST, ST)],
                    psc[:], rec[:], Alu.mult)

            def finish(k, attab, psc):
                finish_tt(k, psc, finish_exp(finish_ln(finish_mm(attab))))

            atts = [att0]

            cpst = [None, None]
            pending = None
            for k in range(NK):
                i, h = seq[k]
                if h == 0 and k + 4 < NK:
                    # prefetch the x tile for the NEXT block now; the DMA
                    # has a whole block (~55us) to land
                    nxt = blk(k + 4)
                    xtn = xs.tile([P, EO, ST], BF16, tag="xt")
                    nc.sync.dma_start(xtn[:], xP[nxt][:])
                    xts[nxt] = xtn
                att = atts[k]
                attA, attB = att
                ci_prev = blk(k - 4) if k >= 4 else None
                if k + 1 < NK:
                    attn = (abA.tile([P, 8, ST], BF16, tag="attA", name="attA"),
                            abB.tile([P, 8, ST], BF16, tag="attB", name="attB"))
                    atts.append(attn)
                else:
                    attn = None
                if k + 2 < NK:
                    psq = psA.tile([P, ST], F32, tag="acc")
                else:
                    psq = None
                psc = psA.tile([P, ST], F32, tag="acc")
                last = (k == NK - 1)
                # block-closing steps reduce the tree two slots early so
                # the finish-NOW chain lands sooner in the next step
                tree_at = [7, 9, 11, 13] if (h == HL - 1 and not last) \
                    else [7, 11, 13, 15]
                held = []
                psd15 = None
                for idx in range(JT):
                    if attn is not None:
                        scores_mm(k + 1, attn, qtiles[k + 1], idx)
                    jb = (idx + 8) % JT     # att@V: B half first
                    avs = (attA, attB)[jb // 8][:, jb % 8, :]
                    nc.tensor.matmul(
                        psc[:], v_sb[:, jb, ds(h * D, D)], avs,
                        start=(idx == 0), stop=(idx == JT - 1))
                    if psq is not None:
                        if h == HL - 1:
                            qproj_mm(k + 2, psq, xts[blk(k + 2)], idx)
                        else:
                            # compress: eo 14/15 ride slots 12/13 so the
                            # accumulation stops two slots early and the
                            # rope chain (idx 13/15 below) drains inside
                            # the step instead of at its boundary
                            if idx <= 13:
                                qproj_mm(k + 2, psq, xts[blk(k + 2)], idx)
                            if 12 <= idx <= 13:
                                qproj_mm(k + 2, psq, xts[blk(k + 2)],
                                         idx + 2)
                    if ci_prev is not None:
                        if k % 4 != 0:
                            if last and idx >= 4:
                                # hold the tail of the previous block's
                                # out-projection so it can hide the final
                                # normalize's Ln/Exp latency (below)
                                held.append((k % 4) * JT + idx)
                            else:
                                cblock_mm(ci_prev, (k % 4) * JT + idx, cpst)
                        elif idx >= 8:
                            # block-boundary step: the previous block's ctx
                            # normalize lands ~1us in, so weave its out-
                            # projection into the back half, two per slot
                            cblock_mm(ci_prev, (idx - 8) * 2, cpst)
                            cblock_mm(ci_prev, (idx - 8) * 2 + 1, cpst)
                    if not last:
                        if psq is not None and h < HL - 1:
                            if idx == 13:
                                qt, rf_early = qproj_fin(k + 2, psq)
                                qtiles.append(qt)
                            elif idx == 15:
                                rope_finish(*rf_early)
                        if pending is not None and idx == 2:
                            pk, patt, ppsc = pending
                            p_psd = finish_mm(patt)
                        elif pending is not None and idx == 3:
                            p_lnd = finish_ln(p_psd)
                        elif pending is not None and idx == 4:
                            p_rec = finish_exp(p_lnd)
                        elif pending is not None and idx == 5:
                            finish_tt(pk, ppsc, p_rec)
                            pending = None
                        if h == HL - 1:
                            # block-closing step: own-head ones-matmul/Ln
                            # woven into idx 14-15 so ctx is final ~1us
                            # into the next (boundary) step
                            if idx == 14:
                                now_psd = finish_mm(att)
                            elif idx == 15:
                                now_lnd = finish_ln(now_psd)
                        # denominator tree levels woven into the matmul
                        # stream; they only ever write attB, whose att@V
                        # reads all finished at idx 7
                        if idx == tree_at[0]:
                            nc.vector.tensor_tensor(
                                attB[:], attB[:], attA[:], Alu.add)
                        elif idx == tree_at[1]:
                            nc.vector.tensor_tensor(
                                attB[:, 0:4, :], attB[:, 0:4, :],
                                attB[:, 4:8, :], Alu.add)
                        elif idx == tree_at[2]:
                            nc.vector.tensor_tensor(
                                attB[:, 0:2, :], attB[:, 0:2, :],
                                attB[:, 2:4, :], Alu.add)
                        elif idx == tree_at[3]:
                            nc.vector.tensor_tensor(
                                attB[:, 0, :], attB[:, 0, :], attB[:, 1, :],
                                Alu.add)
                    else:
                        # LAST step: nothing interleaves after it, so the
                        # usual deferred chains would serialize behind the
                        # matmul stream.  Weave the pending head's finish
                        # into idx 0-3, halve the tree (level 1 only, the
                        # first half written into step NK-2's retired attB
                        # so the coarse dep-tracker sees no write to a tile
                        # the PE still reads), and let the PE itself do the
                        # final 8-way reduction as an accumulating
                        # ones-matmul group in idx 8-15.
                        attB_prev = atts[NK - 2][1]
                        if idx == 0:
                            pk, patt, ppsc = pending
                            p_psd = finish_mm(patt)
                        elif idx == 1:
                            p_lnd = finish_ln(p_psd)
                        elif idx == 2:
                            p_rec = finish_exp(p_lnd)
                        elif idx == 3:
                            finish_tt(pk, ppsc, p_rec)
                            pending = None
                        elif idx == 4:
                            nc.vector.tensor_tensor(
                                attB_prev[:, 0:4, :], attB[:, 0:4, :],
                                attA[:, 0:4, :], Alu.add)
                        elif idx == 7:
                            nc.vector.tensor_tensor(
                                attB[:, 4:8, :], attB[:, 4:8, :],
                                attA[:, 4:8, :], Alu.add)
                        if idx >= 8:
                            s = idx - 8
                            src = attB_prev[:, s, :] if s < 4 \
                                else attB[:, s, :]
                            if s == 0:
                                psd15 = psA.tile([P, ST], F32, tag="acc",
                                                 name="psd15")
                            nc.tensor.matmul(
                                psd15[:], ones_ap, src,
                                start=(s == 0), stop=(s == 7))
                if psq is not None and (h == HL - 1 or last):
                    qt, rf = qproj_fin(k + 2, psq)
                    qtiles.append(qt)
                else:
                    rf = None
                if pending is not None:
                    finish(*pending)
                    pending = None
                if h == HL - 1:
                    # last head of the block: finish NOW so the next
                    # block's interleaved out-projection reads final ctx
                    if last:
                        finish_tt(k, psc, finish_exp(finish_ln(psd15)))
                    else:
                        finish_tt(k, psc, finish_exp(now_lnd))
                    for jb in held:
                        cblock_mm(ci_prev, jb, cpst)
                else:
                    pending = (k, att, psc)
                if rf is not None:
                    rope_finish(*rf)

            # the last block's out-projection has no next block to hide in
            cpst = [None, None]
            for jb in range(4 * JT - 4):
                cblock_mm(blk(NK - 1), jb, cpst, drain=True)
            # final tile: accumulate the two column halves as separate
            # groups so the first half's copy+DMA overlaps the second
            # half's matmuls and the tail transfer is only [P, 256]
            ci = blk(NK - 1)
            sb = ci * (ST // P) + 3
            for hf in range(2):
                psf = psC.tile([P, ST // 2], F32, tag="ct", name="ctf")
                for ho in range(HL):
                    nc.tensor.matmul(
                        psf[:], ctxT_sb[:, ho, ds(sb * P, P)],
                        ow_sb[:, ho, ds(3 * ST + hf * (ST // 2), ST // 2)],
                        start=(ho == 0), stop=(ho == HL - 1))
                otf = oc.tile([P, ST // 2], BF16, tag="otf", name="otf")
                nc.vector.tensor_copy(otf[:], psf[:])
                eng = nc.scalar if hf else nc.sync
                eng.dma_start(
                    out[ds(sb * P, P), ds(3 * ST + hf * (ST // 2), ST // 2)],
                    otf[:])

    return nc


def _rope_tables():
    inv_freq = 1.0 / (10000.0 ** (np.arange(0, D, 2, dtype=np.float32) / D))
    t = np.arange(S, dtype=np.float32)
    freqs = np.einsum("s,f->sf", t, inv_freq)
    emb = np.concatenate([freqs, freqs], axis=-1)
    cosT = np.cos(emb).astype(np.float32).T.copy()
    sinT = np.sin(emb).astype(np.float32).T.copy()
    # rotate-half sign lives in the on-device permutation matrix
    return cosT.astype(BF), np.ascontiguousarray(sinT).astype(BF)


def _core_inputs(x, Wqkv_w, Wqkv_b, out_w, b, g, shared, xT_bf):
    # k-head columns first, then q-head columns (matches kernel layout)
    k_cols, q_cols, kb_rows, qb_rows = [], [], [], []
    for hl in range(HL):
        h = g * HL + hl
        q_cols.append(Wqkv_w[h * D:(h + 1) * D, :].T)
        k_cols.append(Wqkv_w[E + h * D:E + (h + 1) * D, :].T)
        qb_rows.append(Wqkv_b[h * D:(h + 1) * D])
        kb_rows.append(Wqkv_b[E + h * D:E + (h + 1) * D])
    def pack(wT):
        # [E, HL*D] -> [P, EO, HL*D]: per-partition contiguous rows so
        # the whole tensor moves in one (or few) large-descriptor DMAs
        return np.ascontiguousarray(
            wT.reshape(E // P, P, HL * D).transpose(1, 0, 2)).astype(BF)

    wkP = pack(np.concatenate(k_cols, axis=1))
    wqP = pack(np.concatenate(q_cols, axis=1))
    qkbT = np.stack(kb_rows + qb_rows).astype(np.float32).T    # [D, 2HL]
    v0 = 2 * E + g * HL * D
    wvP = pack(Wqkv_w[v0:v0 + HL * D, :].T)
    vb = Wqkv_b[v0:v0 + HL * D].astype(np.float32)
    owT = np.ascontiguousarray(
        out_w[:, g * HL * D:(g + 1) * HL * D].T).astype(BF)
    cosT, sinT, ones, perm = shared
    # qkb | vb | ones | perm | cos | sin (kernel's tbl layout); biases in
    # bf16 cost ~0.4% of their 0.01-scale values -- negligible
    tblP = np.ascontiguousarray(np.concatenate(
        [qkbT, np.broadcast_to(vb[None, :], (P, HL * D)), ones, perm,
         cosT, sinT], axis=1).astype(np.float32)).astype(BF)
    return {"xP": xT_bf, "wkP": wkP, "wqP": wqP, "wvP": wvP,
            "tblP": tblP, "owT": owT}


def kernel(x, Wqkv_w, Wqkv_b, out_w, out_b):
    global LAST_EXEC_NS
    _install_axon_ntff_shim()
    from concourse.bass_utils import run_bass_kernel_spmd

    x = np.asarray(x, dtype=np.float32)
    Wqkv_w = np.asarray(Wqkv_w, dtype=np.float32)
    Wqkv_b = np.asarray(Wqkv_b, dtype=np.float32)
    out_w = np.asarray(out_w, dtype=np.float32)
    out_b = np.asarray(out_b, dtype=np.float32)

    cosT, sinT = _rope_tables()
    # rotate-half permutation: out[d] = -q[d+64] (d<64), +q[d-64] (d>=64)
    perm = np.zeros((P, P), dtype=np.float32)
    for d in range(D // 2):
        perm[d + D // 2, d] = -1.0
        perm[d, d + D // 2] = 1.0
    shared = (cosT.astype(np.float32), sinT.astype(np.float32),
              np.ones((P, P), np.float32), perm)
    # x packed as [NS, P, EO, ST]: xP[i, p, eo, s] = x[b, i*ST+s, eo*P+p]
    NS, EO, ST = S // 512, E // P, 512
    xT_bf = [np.ascontiguousarray(
        x[b].reshape(NS, ST, EO, P).transpose(0, 3, 2, 1)).astype(BF)
        for b in range(2)]
    in_maps = []
    for core in range(8):
        b, g = core // 4, core % 4
        in_maps.append(
            _core_inputs(x, Wqkv_w, Wqkv_b, out_w, b, g, shared, xT_bf[b]))

    nc = bass.Bass()
    _build_mha(nc)
    _split_multi_waits(nc)

    trace = bool(os.environ.get("MHA_TRACE"))
    if trace:
        # dev-only profiling path; skip the S3 artifact upload
        import concourse.bass_utils as _bu
        _bu.upload_artifacts = lambda tmpdir: tmpdir
    res = run_bass_kernel_spmd(
        nc, in_maps, core_ids=list(range(8)), trace=trace)
    if trace:
        LAST_EXEC_NS = res.exec_time_ns

    out = np.empty((2, S, E), dtype=np.float32)
    for b in range(2):
        acc = res.results[b * 4 + 0]["out"].astype(np.float32)
        for g in range(1, 4):
            acc += res.results[b * 4 + g]["out"].astype(np.float32)
        out[b] = acc + out_b[None, :]
    return out


# revision 24
# speedup vs baseline: 1.0063x; 1.0021x over previous
"""Sharded MHA-with-RoPE Trainium2 kernel (nn_CustomTorchMHASelf).

Contract: kernel(**inputs) takes the FULL unsharded inputs of the
reference (x [2,2048,2048], Wqkv_w [6144,2048], Wqkv_b [6144],
out_w [2048,2048], out_b [2048]) and returns the full [2,2048,2048]
fp32 output, running the compute on 8 NeuronCores.

Sharding: core = b*4 + g handles batch b and head-group g (4 of the 16
heads). Each core computes q/k/v projections for its heads, RoPE,
softmax attention, and its slice of the out-projection; the host sums
the 4 partial outputs per batch and adds out_b.

Device data plane is bf16 (fp32 PSUM accumulation); the host
pre-transposes x and the weight slices into the layouts the TensorE
wants (contraction dim on partitions everywhere).

Schedule: pass 1 computes K+RoPE and V for all tokens (the last block
also hides the attention prologue under its V-projection); pass 2 is a
flat software pipeline over (block, head) steps -- at step k the PE
stream interleaves att@V(k), scores(k+1), q-projection(k+2) and a
quarter of the previous block's out-projection (65 matmuls/step), so
the ScalarE exp stream (16/step) is never on the critical path.
Key device tricks:
  - DMA triggers cost ~600ns of issuing-engine queue time EACH
    regardless of size, so tensors move with ONE trigger per tile
    (16KB-per-partition contiguous descriptor runs) instead of 16;
    the startup-critical wk/x0 pair is split into eighth/quarter
    chunks issued alternately so the first K matmuls start ~10.5us
    in, and block 0's K-projection walks chunk-major (4 live PSUM
    accumulators) to consume chunks as they land;
  - rotate-half for RoPE is a PE matmul with a signed permutation
    matrix (SBUF-SBUF DMA swaps are slow and their DIRECT2D triggers
    serialize on the sync sequencer);
  - the softmax denominator is a bf16 tree-add into the attB tile on
    VectorE plus ONE ones-matmul per (head, block) instead of 16 full
    PE ones-matmuls; att is split into two tiles (attA/attB) so the
    tree's writes never alias tiles the PE still reads (the dep
    tracker is coarse); the ones-matmul+normalize are deferred one
    step so the PE never waits on the tree;
  - 1/denominator = Exp(-Ln(d)) on ScalarE (ln and exp share an
    activation table, so no table reloads) because DVE reciprocal is
    slow and custom-DVE ops don't compile on this toolchain;
  - deferred finishes are woven into matmul slots 2-5 of the next step
    (not appended after it) so their Ln/Exp never delays the next
    step's first exps, whose psS banks the score matmuls wait on;
  - warm-up matmuls on a memset tile (no DMA dependency) ramp the PE
    out of its 1.2GHz cold p-state while the first weight/x DMAs land,
    and filler matmuls after the first two K chunks keep the HAM
    activity window busy so the clock never re-throttles mid-startup;
  - output tiles are written bf16, one batched [P, E] DMA per token
    row-block; in the last step the tree is halved (level 1 written
    into a retired att tile), the PE itself does the final 8-way
    denominator reduction as an accumulating ones-matmul group, and
    the drain's final tile is split into column halves so the tail
    transfer is short.
"""

import math
import os
import sys
import types

import numpy as np
import ml_dtypes

import concourse.bass as bass
import concourse.mybir as mybir
import concourse.tile as tile
from concourse.bass import ds

F32 = mybir.dt.float32
BF16 = mybir.dt.bfloat16
Alu = mybir.AluOpType
Act = mybir.ActivationFunctionType
BF = ml_dtypes.bfloat16

S, E, HTOT, HL, D, P = 2048, 2048, 16, 4, 128, 128

# Filled with the profile exec time (ns) when MHA_TRACE=1; read by test.py.
LAST_EXEC_NS = None


def _install_axon_ntff_shim():
    """Provide antenv.axon_hooks so trace=True can reach the axon NTFF hook."""
    if "antenv.axon_hooks" in sys.modules:
        return
    mod = types.ModuleType("antenv.axon_hooks")
    holder = [None]
    mod.set_axon_ntff_profile_hook = lambda h: holder.__setitem__(0, h)
    mod.get_axon_ntff_profile_hook = lambda: holder[0]
    sys.modules["antenv.axon_hooks"] = mod
    try:
        import antenv
        antenv.axon_hooks = mod
    except ImportError:
        pass
    # boot() ran at interpreter start (sitecustomize), before this module
    # existed, so its NTFF-hook registration was silently skipped. Redo it.
    try:
        from trn_agent_boot.trn_boot import _ntff_profile_via_ctypes
        hook = _ntff_profile_via_ctypes("/opt/axon/libaxon_pjrt.so")
        if hook is not None:
            mod.set_axon_ntff_profile_hook(hook)
    except Exception:
        pass


def _split_multi_waits(nc):
    """Hoist extra sem-waits onto standalone NoOps (one wait per inst).

    This walrus build rejects any instruction carrying more than one
    sync-wait ("Too many sync wait commands"); Tile attaches one wait per
    outstanding semaphore to the consuming instruction. Splitting them
    across same-engine NoOps placed immediately before is equivalent:
    the engine executes serially, so all waits still precede the inst.
    """
    ctr = 0
    for fn in nc.m.functions:
        for blk in fn.blocks:
            out = []
            for inst in blk.instructions:
                si = getattr(inst, "sync_info", None)
                if si is not None and si.on_wait is not None \
                        and len(si.on_wait) > 1:
                    waits = list(si.on_wait)
                    si.on_wait = [waits[-1]]
                    for w in waits[:-1]:
                        ctr += 1
                        nop = mybir.InstNoOp(
                            name=f"I-wsplit-{ctr}", ins=[], outs=[])
                        nop.engine = inst.engine
                        nop.sync_info = mybir.SyncInfo(
                            on_wait=[w], on_update=[])
                        out.append(nop)
                out.append(inst)
            blk.instructions[:] = out


def _build_mha(nc: bass.Bass):
    """Emit the per-core MHA program (one shard) into `nc`."""
    EO = E // P            # contraction subtiles for the projections
    ST = 512               # free-dim tile (one PSUM bank of fp32)
    NS = S // ST
    SB = S // P
    JT = S // P            # key blocks per head
    ET = E // ST

    # packed layouts: [.., P, EO, ST] so each tensor is one long
    # per-partition-contiguous run -> ONE ~600ns DMA trigger moves it
    xP = nc.dram_tensor("xP", [NS, P, EO, ST], BF16, kind="ExternalInput")
    wkP = nc.dram_tensor("wkP", [P, EO, HL * D], BF16, kind="ExternalInput")
    wqP = nc.dram_tensor("wqP", [P, EO, HL * D], BF16, kind="ExternalInput")
    wvP = nc.dram_tensor("wvP", [P, EO, HL * D], BF16, kind="ExternalInput")
    # qkb | vb | ones | perm | cos | sin packed into one bf16 table;
    # the small head (biases + matrices) rides an early DMA slice, the
    # big cos/sin tail a later one
    TW = 2 * HL + HL * D + 2 * P + 2 * S
    tblP = nc.dram_tensor("tblP", [P, TW], BF16, kind="ExternalInput")
    owT = nc.dram_tensor("owT", [HL * D, E], BF16, kind="ExternalInput")
    out = nc.dram_tensor("out", [S, E], BF16, kind="ExternalOutput")

    isc = 1.0 / math.sqrt(D)

    from contextlib import ExitStack

    with tile.TileContext(nc) as tc, ExitStack() as stk:
        persist = stk.enter_context(tc.tile_pool(name="persist", bufs=1))
        kT_sb = persist.tile([P, HL, S], BF16)      # k post-RoPE [d, h, s]
        v_sb = persist.tile([P, SB, HL * D], BF16)  # v natural [s%128, s//128, hd]
        ctxT_sb = persist.tile([P, HL, S], BF16)    # [d, h, i]
        tbl_sb = persist.tile([P, TW], BF16)
        ow_sb = persist.tile([P, HL, E], BF16)
        wtile = persist.tile([P, P], BF16)          # memset warm-up operand

        QK0, VB0 = 0, 2 * HL
        ON0 = VB0 + HL * D
        PM0 = ON0 + P
        CS0, SN0 = PM0 + P, PM0 + P + S
        TSMALL = PM0 + P          # early slice: biases + ones + perm

        # tensor_scalar needs f32 scalars: widen the bf16 biases once
        qkbf = persist.tile([P, 2 * HL], F32)

        def qkb_ap(j):
            return qkbf[:, j, None]

        vb_ap = tbl_sb[:, ds(VB0, HL * D)]

        def cos_ap(s0):
            return tbl_sb[:, ds(CS0 + s0, ST)]

        def sin_ap(s0):
            return tbl_sb[:, ds(SN0 + s0, ST)]

        ones_ap = tbl_sb[:, ds(ON0, P)]
        perm_ap = tbl_sb[:, ds(PM0, P)]

        # x stream shared by both passes; rope temps likewise.  qb/rot are
        # still being read (by the rotate matmul / mults) when the next rope
        # starts, so they get extra bufs; t1/t2 are consumed immediately by
        # the in-order VectorE queue, so 1 buf suffices.
        xs = stk.enter_context(tc.tile_pool(name="xstream", bufs=2))
        rta = stk.enter_context(tc.tile_pool(name="ropetmpa", bufs=3))
        rtb = stk.enter_context(tc.tile_pool(name="ropetmpb", bufs=1))
        wqp = stk.enter_context(tc.tile_pool(name="wqpool", bufs=1))
        wq_sb = wqp.tile([P, EO, HL * D], BF16)

        psA = stk.enter_context(tc.tile_pool(name="psA", bufs=4, space="PSUM"))
        psS = stk.enter_context(tc.tile_pool(name="psS", bufs=2, space="PSUM"))
        psC = stk.enter_context(tc.tile_pool(name="psC", bufs=2, space="PSUM"))

        qp = stk.enter_context(tc.tile_pool(name="qpool", bufs=4))
        dp = stk.enter_context(tc.tile_pool(name="denp", bufs=1))
        oc = stk.enter_context(tc.tile_pool(name="ocopy", bufs=2))
        at0 = stk.enter_context(tc.tile_pool(name="att0p", bufs=1))

        # flat (block, head) schedule for the attention pass; blocks in
        # reverse order so the first one reuses pass 1's last x tile
        order = list(range(NS - 1, -1, -1))
        seq = [(i, h) for i in order for h in range(HL)]
        NK = len(seq)

        def blk(k):
            return seq[k][0]

        def rope_begin(ps, bias_ap):
            # qb = q + bias (bf16 so the rotate-half matmul runs full rate)
            qb = rta.tile([P, ST], BF16, tag="qb")
            nc.vector.tensor_scalar_add(qb[:], ps[:], bias_ap)
            return qb

        def rope_finish(qb, s0, out_ap):
            # rotate-half as a PE matmul with a signed permutation matrix
            # (cross-partition moves otherwise need a slow SBUF-SBUF DMA
            # whose trigger also serializes on the sync sequencer);
            # out = qb*cos + (perm.T @ qb)*sin.
            # rps lives in psC so the scores stream owns psS's two banks
            # outright (16 even allocations per step -> the exp reader is
            # always 2 full matmul-slots behind the next allocation)
            rps = psC.tile([P, ST], F32, tag="ct", name="rps")
            nc.tensor.matmul(rps[:], perm_ap, qb[:], start=True, stop=True)
            t1 = rtb.tile([P, ST], BF16, tag="t1")
            t2 = rtb.tile([P, ST], BF16, tag="t2")
            nc.vector.tensor_tensor(t1[:], qb[:], cos_ap(s0), Alu.mult)
            nc.vector.tensor_tensor(t2[:], rps[:], sin_ap(s0), Alu.mult)
            nc.vector.tensor_tensor(out_ap, t1[:], t2[:], Alu.add)

        def qproj_mm(k, psq, xt, eo):
            h = seq[k][1]
            nc.tensor.matmul(
                psq[:], wq_sb[:, eo, ds(h * D, D)], xt[:, eo, :],
                start=(eo == 0), stop=(eo == EO - 1))

        def qproj_fin(k, psq):
            qt = qp.tile([P, ST], BF16, tag="qt")
            qb = rope_begin(psq, qkb_ap(HL + seq[k][1]))
            return qt, (qb, blk(k) * ST, qt[:])

        def scores_mm(k, attab, qt, jb):
            h = seq[k][1]
            ps = psS.tile([P, ST], F32, tag="sc")
            nc.tensor.matmul(
                ps[:], kT_sb[:, h, ds(jb * P, P)], qt[:],
                start=True, stop=True)
            dst = attab[jb // 8][:, jb % 8, :]
            nc.scalar.activation(dst, ps[:], Act.Exp, scale=isc)

        # PE warm-up on a memset tile (no DMA dependency): the HAM clock
        # gate needs ~3.4us of sustained PE activity to release the cold
        # 1.2GHz p-state, and the first real matmul data lands ~10.5us in.
        nc.gpsimd.memset(wtile[:], 1.0)
        wsink = persist.tile([P, 1], F32)
        wps = psS.tile([P, ST], F32, tag="sc", name="warmps")
        NWARM = 44
        for w in range(NWARM):
            nc.tensor.matmul(
                wps[:, :P], wtile[:], wtile[:],
                start=(w == 0), stop=(w == NWARM - 1))
        nc.vector.tensor_copy(wsink[:], wps[:, :1])

        # ---- pass 1: K projection + RoPE, V projection ----
        # The last block additionally hides the attention pass's prologue
        # (q heads 0/1 + scores/exp for head 0) under its V-projection.
        xt_last = None
        att0 = None
        qtiles = []
        with tc.tile_pool(name="p1w", bufs=1) as p1:
            wk_sb = p1.tile([P, EO, HL * D], BF16)
            wv_sb = p1.tile([P, EO, HL * D], BF16)
            # Startup DMA plan.  Sync carries the startup-critical wk/x0
            # stream as interleaved eighth/quarter chunks (each trigger is
            # ~600ns of queue time, each chunk 0.7-2.9us of transfer);
            # Scalar (idle all of pass 1, first free ~8.8us after its
            # activation-table load) carries everything else in whole-tile
            # triggers, ordered by first use.
            # Each HWDGE queue streams ~200 B/ns, so the two startup-
            # critical tensors (wk, x0) ride DIFFERENT engines' queues in
            # matched chunks; everything else follows in first-use order.
            xt0 = xs.tile([P, EO, ST], BF16, tag="xt", name="xt0")
            xt1 = xs.tile([P, EO, ST], BF16, tag="xt", name="xt1")
            CH0 = [(0, 2), (2, 8), (8, 16)]
            nc.sync.dma_start(wk_sb[:, 0:2, :], wkP[:, 0:2, :])
            nc.scalar.dma_start(xt0[:, 0:2, :], xP[0][:, 0:2, :])
            nc.sync.dma_start(wk_sb[:, 2:8, :], wkP[:, 2:8, :])
            nc.scalar.dma_start(tbl_sb[:, :TSMALL], tblP[:, :TSMALL])
            nc.vector.tensor_copy(qkbf[:], tbl_sb[:, ds(QK0, 2 * HL)])
            nc.sync.dma_start(wk_sb[:, 8:16, :], wkP[:, 8:16, :])
            nc.scalar.dma_start(xt0[:, 2:8, :], xP[0][:, 2:8, :])
            nc.scalar.dma_start(xt0[:, 8:16, :], xP[0][:, 8:16, :])
            nc.sync.dma_start(wv_sb[:, 0:8, :], wvP[:, 0:8, :])
            nc.sync.dma_start(wv_sb[:, 8:16, :], wvP[:, 8:16, :])
            nc.scalar.dma_start(tbl_sb[:, TSMALL:], tblP[:, TSMALL:])
            nc.scalar.dma_start(xt1[:, 0:8, :], xP[1][:, 0:8, :])
            nc.scalar.dma_start(xt1[:, 8:16, :], xP[1][:, 8:16, :])

            for i in range(NS):
                if i == 0:
                    xt = xt0
                elif i == 1:
                    xt = xt1
                else:
                    xt = xs.tile([P, EO, ST], BF16, tag="xt")
                    nc.sync.dma_start(xt[:, 0:8, :], xP[i][:, 0:8, :])
                    nc.sync.dma_start(xt[:, 8:16, :], xP[i][:, 8:16, :])
                    if i == NS - 1:
                        # prefetch pass-2 weights behind this block's x:
                        # wq feeds the q-projections later in this block,
                        # ow the out-projection a block later
                        nc.sync.dma_start(wq_sb[:, 0:8, :], wqP[:, 0:8, :])
                        nc.sync.dma_start(wq_sb[:, 8:16, :], wqP[:, 8:16, :])
                        nc.sync.dma_start(
                            ow_sb[:],
                            owT[:].rearrange("(h p) e -> p h e", p=P))
                sl = ds(i * ST, ST)
                # K-projection: block 0 walks chunk-major with one live
                # PSUM accumulator per head so the PE consumes wk/x0
                # chunks as they land; later blocks' x is fully resident.
                chunks = CH0 if i == 0 else [(0, EO)]
                kps = [psA.tile([P, ST], F32, tag="acc", name=f"kps{jb}")
                       for jb in range(HL)]
                for ci, (a, b) in enumerate(chunks):
                    for jb in range(HL):
                        for eo in range(a, b):
                            nc.tensor.matmul(
                                kps[jb][:], wk_sb[:, eo, ds(jb * D, D)],
                                xt[:, eo, :],
                                start=(eo == 0), stop=(eo == EO - 1))
                    if i == 0 and ci < 2:
                        # keep the HAM clock-gate's activity window busy
                        # while the next wk/x0 chunk lands (psS is untouched
                        # through block 0's K, so wps is still the scores
                        # pool's most recent allocation)
                        for _ in range(34 + 6 * ci):
                            nc.tensor.matmul(
                                wps[:, :P], wtile[:], wtile[:],
                                start=True, stop=True)
                kropes = []
                for jb in range(HL):
                    qb = rope_begin(kps[jb], qkb_ap(jb))
                    kropes.append((qb, i * ST, kT_sb[:, jb, sl]))
                if i < NS - 1:
                    for sbl in range(ST // P):
                        sb = i * (ST // P) + sbl
                        ps = psS.tile([P, ST], F32, tag="sc")
                        for eo in range(EO):
                            nc.tensor.matmul(
                                ps[:, : HL * D], xt[:, eo, ds(sbl * P, P)],
                                wv_sb[:, eo, :],
                                start=(eo == 0), stop=(eo == EO - 1))
                        nc.vector.tensor_tensor(
                            v_sb[:, sb, :], ps[:, : HL * D], vb_ap, Alu.add)
                    for kr in kropes:
                        rope_finish(*kr)
                else:
                    for kr in kropes:
                        rope_finish(*kr)
                    # q-projections for the first two attention heads, then
                    # V-projection interleaved with scores/exp for head 0
                    psq0 = psA.tile([P, ST], F32, tag="acc")
                    for eo in range(EO):
                        qproj_mm(0, psq0, xt, eo)
                    qt0, rf0 = qproj_fin(0, psq0)
                    psq1 = psA.tile([P, ST], F32, tag="acc")
                    for eo in range(EO):
                        qproj_mm(1, psq1, xt, eo)
                    qt1, rf1 = qproj_fin(1, psq1)
                    qtiles.extend([qt0, qt1])
                    rope_finish(*rf0)
                    att0 = (at0.tile([P, 8, ST], BF16, name="att0A"),
                            at0.tile([P, 8, ST], BF16, name="att0B"))
                    sc_jb = 0
                    vps = None
                    for vi in range(4 * EO):
                        sbl, eo = vi // EO, vi % EO
                        if eo == 0:
                            vps = psA.tile([P, ST], F32, tag="acc")
                        nc.tensor.matmul(
                            vps[:, : HL * D], xt[:, eo, ds(sbl * P, P)],
                            wv_sb[:, eo, :],
                            start=(eo == 0), stop=(eo == EO - 1))
                        if eo == EO - 1:
                            nc.vector.tensor_tensor(
                                v_sb[:, i * (ST // P) + sbl, :],
                                vps[:, : HL * D], vb_ap, Alu.add)
                        if vi >= EO and (vi - EO) % 3 == 0 and sc_jb < JT:
                            scores_mm(0, att0, qt0, sc_jb)
                            sc_jb += 1
                        if vi == 40:
                            rope_finish(*rf1)
                if i == NS - 1:
                    xt_last = xt

        # ---- pass 2: flat software pipeline over (block, head) steps ----
        # Blocks run in reverse so the first one reuses pass 1's last x
        # tile.  At step k: att@V + denominator tree for head k, scores+exp
        # for head k+1, Q-projection for head k+2, one quarter of the
        # PREVIOUS block's out-projection, and the deferred denominator
        # ones-matmul + normalize for head k-1 -- all interleaved so the PE
        # stream (65 matmuls/step) hides the exp stream (16/step).
        with tc.tile_pool(name="attpa", bufs=3) as abA, \
             tc.tile_pool(name="attpb", bufs=3) as abB:

            xts = {order[0]: xt_last}

            def cblock_mm(ci, jb, pst, drain=False):
                # one of the 16x4 out-projection matmuls for token block ci;
                # jb runs 0..63 across the block's four steps.  Each row
                # block's four 512-col quarters collect in one oc tile and
                # leave in a single [P, E] DMA.
                tile_i, ho = jb // HL, jb % HL
                sb_loc, et = tile_i // ET, tile_i % ET
                sb = ci * (ST // P) + sb_loc
                if ho == 0:
                    pst[0] = psC.tile([P, ST], F32, tag="ct", name="ct")
                nc.tensor.matmul(
                    pst[0][:], ctxT_sb[:, ho, ds(sb * P, P)],
                    ow_sb[:, ho, ds(et * ST, ST)],
                    start=(ho == 0), stop=(ho == HL - 1))
                if ho == HL - 1:
                    if et == 0:
                        pst[1] = oc.tile([P, ET, ST], BF16, tag="ot",
                                         name="ot")
                    nc.vector.tensor_copy(pst[1][:, et, :], pst[0][:])
                    if drain and sb_loc == ST // P - 1:
                        # final row block: one small DMA per quarter right
                        # after its copy, alternating trigger engines, so
                        # the post-matmul tail transfer is short
                        eng = nc.scalar if (et % 2) else nc.sync
                        eng.dma_start(
                            out[ds(sb * P, P), ds(et * ST, ST)],
                            pst[1][:, et, :])
                    elif et == ET - 1:
                        if not drain:
                            nc.sync.dma_start(
                                out[ds(sb * P, P), :], pst[1][:])
                        else:
                            eng = nc.scalar if (sb_loc % 2) else nc.sync
                            eng.dma_start(out[ds(sb * P, P), :], pst[1][:])

            # denominator ones-matmul + normalize for head k; deferred one
            # step so the PE reaches the ones-matmul well after the VectorE
            # tree produced attB[:, 0, :].  1/d = Exp(-Ln(d)) on ScalarE
            # (ln/exp share an activation table: no reloads).  Split into
            # pieces so the last two steps can weave them into the matmul
            # stream instead of serializing after it.
            def finish_mm(attab):
                psd = psC.tile([P, ST], F32, tag="ct", name="psd")
                nc.tensor.matmul(
                    psd[:], ones_ap, attab[1][:, 0, :],
                    start=True, stop=True)
                return psd

            def finish_ln(psd):
                lnd = dp.tile([P, ST], F32, tag="lnd")
                nc.scalar.activation(lnd[:], psd[:], Act.Ln)
                return lnd

            def finish_exp(lnd):
                rec = dp.tile([P, ST], F32, tag="rec")
                nc.scalar.activation(rec[:], lnd[:], Act.Exp, scale=-1.0)
                return rec

            def finish_tt(k, psc, rec):
                nc.vector.tensor_tensor(
                    ctxT_sb[:, seq[k][1], ds(blk(k) * ST, ST)],
                    psc[:], rec[:], Alu.mult)

            def finish(k, attab, psc):
                finish_tt(k, psc, finish_exp(finish_ln(finish_mm(attab))))

            atts = [att0]

            cpst = [None, None]
            pending = None
            for k in range(NK):
                i, h = seq[k]
                if h == 0 and k + 4 < NK:
                    # prefetch the x tile for the NEXT block now; the DMA
                    # has a whole block (~55us) to land
                    nxt = blk(k + 4)
                    xtn = xs.tile([P, EO, ST], BF16, tag="xt")
                    nc.sync.dma_start(xtn[:], xP[nxt][:])
                    xts[nxt] = xtn
                att = atts[k]
                attA, attB = att
                ci_prev = blk(k - 4) if k >= 4 else None
                if k + 1 < NK:
                    attn = (abA.tile([P, 8, ST], BF16, tag="attA", name="attA"),
                            abB.tile([P, 8, ST], BF16, tag="attB", name="attB"))
                    atts.append(attn)
                else:
                    attn = None
                if k + 2 < NK:
                    psq = psA.tile([P, ST], F32, tag="acc")
                else:
                    psq = None
                psc = psA.tile([P, ST], F32, tag="acc")
                last = (k == NK - 1)
                # block-closing steps reduce the tree two slots early so
                # the finish-NOW chain lands sooner in the next step
                # tree two slots early everywhere: the deferred ones-
                # matmul reads attB[:, 0, :] at idx 2 of the NEXT step,
                # and level 4 at idx 15 + DVE queue lag just misses that
                tree_at = [7, 9, 11, 13]
                held = []
                psd15 = None
                for idx in range(JT):
                    if attn is not None:
                        scores_mm(k + 1, attn, qtiles[k + 1], idx)
                    jb = (idx + 8) % JT     # att@V: B half first
                    avs = (attA, attB)[jb // 8][:, jb % 8, :]
                    nc.tensor.matmul(
                        psc[:], v_sb[:, jb, ds(h * D, D)], avs,
                        start=(idx == 0), stop=(idx == JT - 1))
                    if psq is not None:
                        if h == HL - 1:
                            qproj_mm(k + 2, psq, xts[blk(k + 2)], idx)
                        else:
                            # compress: eo 14/15 ride slots 12/13 so the
                            # accumulation stops two slots early and the
                            # rope chain (idx 13/15 below) drains inside
                            # the step instead of at its boundary
                            if idx <= 13:
                                qproj_mm(k + 2, psq, xts[blk(k + 2)], idx)
                            if 12 <= idx <= 13:
                                qproj_mm(k + 2, psq, xts[blk(k + 2)],
                                         idx + 2)
                    if ci_prev is not None:
                        if k % 4 != 0:
                            if last and idx >= 4:
                                # hold the tail of the previous block's
                                # out-projection so it can hide the final
                                # normalize's Ln/Exp latency (below)
                                held.append((k % 4) * JT + idx)
                            else:
                                cblock_mm(ci_prev, (k % 4) * JT + idx, cpst)
                        elif idx >= 8:
                            # block-boundary step: the previous block's ctx
                            # normalize lands ~1us in, so weave its out-
                            # projection into the back half, two per slot
                            cblock_mm(ci_prev, (idx - 8) * 2, cpst)
                            cblock_mm(ci_prev, (idx - 8) * 2 + 1, cpst)
                    if not last:
                        if psq is not None and h < HL - 1:
                            if idx == 13:
                                qt, rf_early = qproj_fin(k + 2, psq)
                                qtiles.append(qt)
                            elif idx == 15:
                                rope_finish(*rf_early)
                        if pending is not None and idx == 2:
                            pk, patt, ppsc = pending
                            p_psd = finish_mm(patt)
                        elif pending is not None and idx == 3:
                            p_lnd = finish_ln(p_psd)
                        elif pending is not None and idx == 4:
                            p_rec = finish_exp(p_lnd)
                        elif pending is not None and idx == 5:
                            finish_tt(pk, ppsc, p_rec)
                            pending = None
                        if h == HL - 1:
                            # block-closing step: own-head ones-matmul/Ln
                            # woven into idx 14-15 so ctx is final ~1us
                            # into the next (boundary) step
                            if idx == 14:
                                now_psd = finish_mm(att)
                            elif idx == 15:
                                now_lnd = finish_ln(now_psd)
                        # denominator tree levels woven into the matmul
                        # stream; they only ever write attB, whose att@V
                        # reads all finished at idx 7
                        if idx == tree_at[0]:
                            nc.vector.tensor_tensor(
                                attB[:], attB[:], attA[:], Alu.add)
                        elif idx == tree_at[1]:
                            nc.vector.tensor_tensor(
                                attB[:, 0:4, :], attB[:, 0:4, :],
                                attB[:, 4:8, :], Alu.add)
                        elif idx == tree_at[2]:
                            nc.vector.tensor_tensor(
                                attB[:, 0:2, :], attB[:, 0:2, :],
                                attB[:, 2:4, :], Alu.add)
                        elif idx == tree_at[3]:
                            nc.vector.tensor_tensor(
                                attB[:, 0, :], attB[:, 0, :], attB[:, 1, :],
                                Alu.add)
                    else:
                        # LAST step: nothing interleaves after it, so the
                        # usual deferred chains would serialize behind the
                        # matmul stream.  Weave the pending head's finish
                        # into idx 0-3, halve the tree (level 1 only, the
                        # first half written into step NK-2's retired attB
                        # so the coarse dep-tracker sees no write to a tile
                        # the PE still reads), and let the PE itself do the
                        # final 8-way reduction as an accumulating
                        # ones-matmul group in idx 8-15.
                        attB_prev = atts[NK - 2][1]
                        if idx == 0:
                            pk, patt, ppsc = pending
                            p_psd = finish_mm(patt)
                        elif idx == 1:
                            p_lnd = finish_ln(p_psd)
                        elif idx == 2:
                            p_rec = finish_exp(p_lnd)
                        elif idx == 3:
                            finish_tt(pk, ppsc, p_rec)
                            pending = None
                        elif idx == 4:
                            nc.vector.tensor_tensor(
                                attB_prev[:, 0:4, :], attB[:, 0:4, :],
                                attA[:, 0:4, :], Alu.add)
                        elif idx == 7:
                            nc.vector.tensor_tensor(
                                attB[:, 4:8, :], attB[:, 4:8, :],
                                attA[:, 4:8, :], Alu.add)
                        if idx >= 8:
                            s = idx - 8
                            src = attB_prev[:, s, :] if s < 4 \
                                else attB[:, s, :]
                            if s == 0:
                                psd15 = psA.tile([P, ST], F32, tag="acc",
                                                 name="psd15")
                            nc.tensor.matmul(
                                psd15[:], ones_ap, src,
                                start=(s == 0), stop=(s == 7))
                if psq is not None and (h == HL - 1 or last):
                    qt, rf = qproj_fin(k + 2, psq)
                    qtiles.append(qt)
                else:
                    rf = None
                if pending is not None:
                    finish(*pending)
                    pending = None
                if h == HL - 1:
                    # last head of the block: finish NOW so the next
                    # block's interleaved out-projection reads final ctx
                    if last:
                        finish_tt(k, psc, finish_exp(finish_ln(psd15)))
                    else:
                        finish_tt(k, psc, finish_exp(now_lnd))
                    for jb in held:
                        cblock_mm(ci_prev, jb, cpst)
                else:
                    pending = (k, att, psc)
                if rf is not None:
                    rope_finish(*rf)

            # the last block's out-projection has no next block to hide in
            cpst = [None, None]
            for jb in range(4 * JT - 4):
                cblock_mm(blk(NK - 1), jb, cpst, drain=True)
            # final tile: accumulate the two column halves as separate
            # groups so the first half's copy+DMA overlaps the second
            # half's matmuls and the tail transfer is only [P, 256]
            ci = blk(NK - 1)
            sb = ci * (ST // P) + 3
            for hf in range(2):
                psf = psC.tile([P, ST // 2], F32, tag="ct", name="ctf")
                for ho in range(HL):
                    nc.tensor.matmul(
                        psf[:], ctxT_sb[:, ho, ds(sb * P, P)],
                        ow_sb[:, ho, ds(3 * ST + hf * (ST // 2), ST // 2)],
                        start=(ho == 0), stop=(ho == HL - 1))
                otf = oc.tile([P, ST // 2], BF16, tag="otf", name="otf")
                nc.vector.tensor_copy(otf[:], psf[:])
                eng = nc.scalar if hf else nc.sync
                eng.dma_start(
                    out[ds(sb * P, P), ds(3 * ST + hf * (ST // 2), ST // 2)],
                    otf[:])

    return nc


def _rope_tables():
    inv_freq = 1.0 / (10000.0 ** (np.arange(0, D, 2, dtype=np.float32) / D))
    t = np.arange(S, dtype=np.float32)
    freqs = np.einsum("s,f->sf", t, inv_freq)
    emb = np.concatenate([freqs, freqs], axis=-1)
    cosT = np.cos(emb).astype(np.float32).T.copy()
    sinT = np.sin(emb).astype(np.float32).T.copy()
    # rotate-half sign lives in the on-device permutation matrix
    return cosT.astype(BF), np.ascontiguousarray(sinT).astype(BF)


def _core_inputs(x, Wqkv_w, Wqkv_b, out_w, b, g, shared, xT_bf):
    # k-head columns first, then q-head columns (matches kernel layout)
    k_cols, q_cols, kb_rows, qb_rows = [], [], [], []
    for hl in range(HL):
        h = g * HL + hl
        q_cols.append(Wqkv_w[h * D:(h + 1) * D, :].T)
        k_cols.append(Wqkv_w[E + h * D:E + (h + 1) * D, :].T)
        qb_rows.append(Wqkv_b[h * D:(h + 1) * D])
        kb_rows.append(Wqkv_b[E + h * D:E + (h + 1) * D])
    def pack(wT):
        # [E, HL*D] -> [P, EO, HL*D]: per-partition contiguous rows so
        # the whole tensor moves in one (or few) large-descriptor DMAs
        return np.ascontiguousarray(
            wT.reshape(E // P, P, HL * D).transpose(1, 0, 2)).astype(BF)

    wkP = pack(np.concatenate(k_cols, axis=1))
    wqP = pack(np.concatenate(q_cols, axis=1))
    qkbT = np.stack(kb_rows + qb_rows).astype(np.float32).T    # [D, 2HL]
    v0 = 2 * E + g * HL * D
    wvP = pack(Wqkv_w[v0:v0 + HL * D, :].T)
    vb = Wqkv_b[v0:v0 + HL * D].astype(np.float32)
    owT = np.ascontiguousarray(
        out_w[:, g * HL * D:(g + 1) * HL * D].T).astype(BF)
    cosT, sinT, ones, perm = shared
    # qkb | vb | ones | perm | cos | sin (kernel's tbl layout); biases in
    # bf16 cost ~0.4% of their 0.01-scale values -- negligible
    tblP = np.ascontiguousarray(np.concatenate(
        [qkbT, np.broadcast_to(vb[None, :], (P, HL * D)), ones, perm,
         cosT, sinT], axis=1).astype(np.float32)).astype(BF)
    return {"xP": xT_bf, "wkP": wkP, "wqP": wqP, "wvP": wvP,
            "tblP": tblP, "owT": owT}


def kernel(x, Wqkv_w, Wqkv_b, out_w, out_b):
    global LAST_EXEC_NS
    _install_axon_ntff_shim()
    from concourse.bass_utils import run_bass_kernel_spmd

    x = np.asarray(x, dtype=np.float32)
    Wqkv_w = np.asarray(Wqkv_w, dtype=np.float32)
    Wqkv_b = np.asarray(Wqkv_b, dtype=np.float32)
    out_w = np.asarray(out_w, dtype=np.float32)
    out_b = np.asarray(out_b, dtype=np.float32)

    cosT, sinT = _rope_tables()
    # rotate-half permutation: out[d] = -q[d+64] (d<64), +q[d-64] (d>=64)
    perm = np.zeros((P, P), dtype=np.float32)
    for d in range(D // 2):
        perm[d + D // 2, d] = -1.0
        perm[d, d + D // 2] = 1.0
    shared = (cosT.astype(np.float32), sinT.astype(np.float32),
              np.ones((P, P), np.float32), perm)
    # x packed as [NS, P, EO, ST]: xP[i, p, eo, s] = x[b, i*ST+s, eo*P+p]
    NS, EO, ST = S // 512, E // P, 512
    xT_bf = [np.ascontiguousarray(
        x[b].reshape(NS, ST, EO, P).transpose(0, 3, 2, 1)).astype(BF)
        for b in range(2)]
    in_maps = []
    for core in range(8):
        b, g = core // 4, core % 4
        in_maps.append(
            _core_inputs(x, Wqkv_w, Wqkv_b, out_w, b, g, shared, xT_bf[b]))

    nc = bass.Bass()
    _build_mha(nc)
    _split_multi_waits(nc)

    trace = bool(os.environ.get("MHA_TRACE"))
    if trace:
        # dev-only profiling path; skip the S3 artifact upload
        import concourse.bass_utils as _bu
        _bu.upload_artifacts = lambda tmpdir: tmpdir
    res = run_bass_kernel_spmd(
        nc, in_maps, core_ids=list(range(8)), trace=trace)
    if trace:
        LAST_EXEC_NS = res.exec_time_ns

    out = np.empty((2, S, E), dtype=np.float32)
    for b in range(2):
        acc = res.results[b * 4 + 0]["out"].astype(np.float32)
        for g in range(1, 4):
            acc += res.results[b * 4 + g]["out"].astype(np.float32)
        out[b] = acc + out_b[None, :]
    return out


# revision 25
# speedup vs baseline: 1.0093x; 1.0029x over previous
"""Sharded MHA-with-RoPE Trainium2 kernel (nn_CustomTorchMHASelf).

Contract: kernel(**inputs) takes the FULL unsharded inputs of the
reference (x [2,2048,2048], Wqkv_w [6144,2048], Wqkv_b [6144],
out_w [2048,2048], out_b [2048]) and returns the full [2,2048,2048]
fp32 output, running the compute on 8 NeuronCores.

Sharding: core = b*4 + g handles batch b and head-group g (4 of the 16
heads). Each core computes q/k/v projections for its heads, RoPE,
softmax attention, and its slice of the out-projection; the host sums
the 4 partial outputs per batch and adds out_b.

Device data plane is bf16 (fp32 PSUM accumulation); the host
pre-transposes x and the weight slices into the layouts the TensorE
wants (contraction dim on partitions everywhere).

Schedule: pass 1 computes K+RoPE and V for all tokens (the last block
also hides the attention prologue under its V-projection); pass 2 is a
flat software pipeline over (block, head) steps -- at step k the PE
stream interleaves att@V(k), scores(k+1), q-projection(k+2) and a
quarter of the previous block's out-projection (65 matmuls/step), so
the ScalarE exp stream (16/step) is never on the critical path.
Key device tricks:
  - DMA triggers cost ~600ns of issuing-engine queue time EACH
    regardless of size, so tensors move with ONE trigger per tile
    (16KB-per-partition contiguous descriptor runs) instead of 16;
    the startup-critical wk/x0 pair is split into eighth/quarter
    chunks issued alternately so the first K matmuls start ~10.5us
    in, and block 0's K-projection walks chunk-major (4 live PSUM
    accumulators) to consume chunks as they land;
  - rotate-half for RoPE is a PE matmul with a signed permutation
    matrix (SBUF-SBUF DMA swaps are slow and their DIRECT2D triggers
    serialize on the sync sequencer);
  - the softmax denominator is a bf16 tree-add into the attB tile on
    VectorE plus ONE ones-matmul per (head, block) instead of 16 full
    PE ones-matmuls; att is split into two tiles (attA/attB) so the
    tree's writes never alias tiles the PE still reads (the dep
    tracker is coarse); the ones-matmul+normalize are deferred one
    step so the PE never waits on the tree;
  - 1/denominator = Exp(-Ln(d)) on ScalarE (ln and exp share an
    activation table, so no table reloads) because DVE reciprocal is
    slow and custom-DVE ops don't compile on this toolchain;
  - deferred finishes are woven into matmul slots 2-5 of the next step
    (not appended after it) so their Ln/Exp never delays the next
    step's first exps, whose psS banks the score matmuls wait on;
  - warm-up matmuls on a memset tile (no DMA dependency) ramp the PE
    out of its 1.2GHz cold p-state while the first weight/x DMAs land,
    and filler matmuls after the first two K chunks keep the HAM
    activity window busy so the clock never re-throttles mid-startup;
  - output tiles are written bf16, one batched [P, E] DMA per token
    row-block; in the last step the tree is halved (level 1 written
    into a retired att tile), the PE itself does the final 8-way
    denominator reduction as an accumulating ones-matmul group, and
    the drain's final tile is split into column halves so the tail
    transfer is short.
"""

import math
import os
import sys
import types

import numpy as np
import ml_dtypes

import concourse.bass as bass
import concourse.mybir as mybir
import concourse.tile as tile
from concourse.bass import ds

F32 = mybir.dt.float32
BF16 = mybir.dt.bfloat16
Alu = mybir.AluOpType
Act = mybir.ActivationFunctionType
BF = ml_dtypes.bfloat16

S, E, HTOT, HL, D, P = 2048, 2048, 16, 4, 128, 128

# Filled with the profile exec time (ns) when MHA_TRACE=1; read by test.py.
LAST_EXEC_NS = None


def _install_axon_ntff_shim():
    """Provide antenv.axon_hooks so trace=True can reach the axon NTFF hook."""
    if "antenv.axon_hooks" in sys.modules:
        return
    mod = types.ModuleType("antenv.axon_hooks")
    holder = [None]
    mod.set_axon_ntff_profile_hook = lambda h: holder.__setitem__(0, h)
    mod.get_axon_ntff_profile_hook = lambda: holder[0]
    sys.modules["antenv.axon_hooks"] = mod
    try:
        import antenv
        antenv.axon_hooks = mod
    except ImportError:
        pass
    # boot() ran at interpreter start (sitecustomize), before this module
    # existed, so its NTFF-hook registration was silently skipped. Redo it.
    try:
        from trn_agent_boot.trn_boot import _ntff_profile_via_ctypes
        hook = _ntff_profile_via_ctypes("/opt/axon/libaxon_pjrt.so")
        if hook is not None:
            mod.set_axon_ntff_profile_hook(hook)
    except Exception:
        pass


def _split_multi_waits(nc):
    """Hoist extra sem-waits onto standalone NoOps (one wait per inst).

    This walrus build rejects any instruction carrying more than one
    sync-wait ("Too many sync wait commands"); Tile attaches one wait per
    outstanding semaphore to the consuming instruction. Splitting them
    across same-engine NoOps placed immediately before is equivalent:
    the engine executes serially, so all waits still precede the inst.
    """
    ctr = 0
    for fn in nc.m.functions:
        for blk in fn.blocks:
            out = []
            for inst in blk.instructions:
                si = getattr(inst, "sync_info", None)
                if si is not None and si.on_wait is not None \
                        and len(si.on_wait) > 1:
                    waits = list(si.on_wait)
                    si.on_wait = [waits[-1]]
                    for w in waits[:-1]:
                        ctr += 1
                        nop = mybir.InstNoOp(
                            name=f"I-wsplit-{ctr}", ins=[], outs=[])
                        nop.engine = inst.engine
                        nop.sync_info = mybir.SyncInfo(
                            on_wait=[w], on_update=[])
                        out.append(nop)
                out.append(inst)
            blk.instructions[:] = out


def _build_mha(nc: bass.Bass):
    """Emit the per-core MHA program (one shard) into `nc`."""
    EO = E // P            # contraction subtiles for the projections
    ST = 512               # free-dim tile (one PSUM bank of fp32)
    NS = S // ST
    SB = S // P
    JT = S // P            # key blocks per head
    ET = E // ST

    # packed layouts: [.., P, EO, ST] so each tensor is one long
    # per-partition-contiguous run -> ONE ~600ns DMA trigger moves it
    xP = nc.dram_tensor("xP", [NS, P, EO, ST], BF16, kind="ExternalInput")
    wkP = nc.dram_tensor("wkP", [P, EO, HL * D], BF16, kind="ExternalInput")
    wqP = nc.dram_tensor("wqP", [P, EO, HL * D], BF16, kind="ExternalInput")
    wvP = nc.dram_tensor("wvP", [P, EO, HL * D], BF16, kind="ExternalInput")
    # qkb | vb | ones | perm | cos | sin packed into one bf16 table;
    # the small head (biases + matrices) rides an early DMA slice, the
    # big cos/sin tail a later one
    TW = 2 * HL + HL * D + 2 * P + 2 * S
    tblP = nc.dram_tensor("tblP", [P, TW], BF16, kind="ExternalInput")
    owT = nc.dram_tensor("owT", [HL * D, E], BF16, kind="ExternalInput")
    out = nc.dram_tensor("out", [S, E], BF16, kind="ExternalOutput")

    isc = 1.0 / math.sqrt(D)

    from contextlib import ExitStack

    with tile.TileContext(nc) as tc, ExitStack() as stk:
        persist = stk.enter_context(tc.tile_pool(name="persist", bufs=1))
        kT_sb = persist.tile([P, HL, S], BF16)      # k post-RoPE [d, h, s]
        v_sb = persist.tile([P, SB, HL * D], BF16)  # v natural [s%128, s//128, hd]
        ctxT_sb = persist.tile([P, HL, S], BF16)    # [d, h, i]
        tbl_sb = persist.tile([P, TW], BF16)
        ow_sb = persist.tile([P, HL, E], BF16)
        wtile = persist.tile([P, P], BF16)          # memset warm-up operand

        QK0, VB0 = 0, 2 * HL
        ON0 = VB0 + HL * D
        PM0 = ON0 + P
        CS0, SN0 = PM0 + P, PM0 + P + S
        TSMALL = PM0 + P          # early slice: biases + ones + perm

        # tensor_scalar needs f32 scalars: widen the bf16 biases once
        qkbf = persist.tile([P, 2 * HL], F32)

        def qkb_ap(j):
            return qkbf[:, j, None]

        vb_ap = tbl_sb[:, ds(VB0, HL * D)]

        def cos_ap(s0):
            return tbl_sb[:, ds(CS0 + s0, ST)]

        def sin_ap(s0):
            return tbl_sb[:, ds(SN0 + s0, ST)]

        ones_ap = tbl_sb[:, ds(ON0, P)]
        perm_ap = tbl_sb[:, ds(PM0, P)]

        # x stream shared by both passes; rope temps likewise.  qb/rot are
        # still being read (by the rotate matmul / mults) when the next rope
        # starts, so they get extra bufs; t1/t2 are consumed immediately by
        # the in-order VectorE queue, so 1 buf suffices.
        xs = stk.enter_context(tc.tile_pool(name="xstream", bufs=2))
        rta = stk.enter_context(tc.tile_pool(name="ropetmpa", bufs=3))
        rtb = stk.enter_context(tc.tile_pool(name="ropetmpb", bufs=1))
        wqp = stk.enter_context(tc.tile_pool(name="wqpool", bufs=1))
        wq_sb = wqp.tile([P, EO, HL * D], BF16)

        psA = stk.enter_context(tc.tile_pool(name="psA", bufs=4, space="PSUM"))
        psS = stk.enter_context(tc.tile_pool(name="psS", bufs=2, space="PSUM"))
        psC = stk.enter_context(tc.tile_pool(name="psC", bufs=2, space="PSUM"))

        qp = stk.enter_context(tc.tile_pool(name="qpool", bufs=4))
        dp = stk.enter_context(tc.tile_pool(name="denp", bufs=1))
        oc = stk.enter_context(tc.tile_pool(name="ocopy", bufs=2))
        at0 = stk.enter_context(tc.tile_pool(name="att0p", bufs=1))

        # flat (block, head) schedule for the attention pass; blocks in
        # reverse order so the first one reuses pass 1's last x tile
        order = list(range(NS - 1, -1, -1))
        seq = [(i, h) for i in order for h in range(HL)]
        NK = len(seq)

        def blk(k):
            return seq[k][0]

        def rope_begin(ps, bias_ap):
            # qb = q + bias (bf16 so the rotate-half matmul runs full rate)
            qb = rta.tile([P, ST], BF16, tag="qb")
            nc.vector.tensor_scalar_add(qb[:], ps[:], bias_ap)
            return qb

        def rope_finish(qb, s0, out_ap):
            # rotate-half as a PE matmul with a signed permutation matrix
            # (cross-partition moves otherwise need a slow SBUF-SBUF DMA
            # whose trigger also serializes on the sync sequencer);
            # out = qb*cos + (perm.T @ qb)*sin.
            # rps lives in psC so the scores stream owns psS's two banks
            # outright (16 even allocations per step -> the exp reader is
            # always 2 full matmul-slots behind the next allocation)
            rps = psC.tile([P, ST], F32, tag="ct", name="rps")
            nc.tensor.matmul(rps[:], perm_ap, qb[:], start=True, stop=True)
            t1 = rtb.tile([P, ST], BF16, tag="t1")
            t2 = rtb.tile([P, ST], BF16, tag="t2")
            nc.vector.tensor_tensor(t1[:], qb[:], cos_ap(s0), Alu.mult)
            nc.vector.tensor_tensor(t2[:], rps[:], sin_ap(s0), Alu.mult)
            nc.vector.tensor_tensor(out_ap, t1[:], t2[:], Alu.add)

        def qproj_mm(k, psq, xt, eo):
            h = seq[k][1]
            nc.tensor.matmul(
                psq[:], wq_sb[:, eo, ds(h * D, D)], xt[:, eo, :],
                start=(eo == 0), stop=(eo == EO - 1))

        def qproj_fin(k, psq):
            qt = qp.tile([P, ST], BF16, tag="qt")
            qb = rope_begin(psq, qkb_ap(HL + seq[k][1]))
            return qt, (qb, blk(k) * ST, qt[:])

        def scores_mm(k, attab, qt, jb):
            h = seq[k][1]
            ps = psS.tile([P, ST], F32, tag="sc")
            nc.tensor.matmul(
                ps[:], kT_sb[:, h, ds(jb * P, P)], qt[:],
                start=True, stop=True)
            dst = attab[jb // 8][:, jb % 8, :]
            nc.scalar.activation(dst, ps[:], Act.Exp, scale=isc)

        # PE warm-up on a memset tile (no DMA dependency): the HAM clock
        # gate needs ~3.4us of sustained PE activity to release the cold
        # 1.2GHz p-state, and the first real matmul data lands ~10.5us in.
        nc.gpsimd.memset(wtile[:], 1.0)
        wsink = persist.tile([P, 1], F32)
        wps = psS.tile([P, ST], F32, tag="sc", name="warmps")
        NWARM = 44
        for w in range(NWARM):
            nc.tensor.matmul(
                wps[:, :P], wtile[:], wtile[:],
                start=(w == 0), stop=(w == NWARM - 1))
        nc.vector.tensor_copy(wsink[:], wps[:, :1])

        # ---- pass 1: K projection + RoPE, V projection ----
        # The last block additionally hides the attention pass's prologue
        # (q heads 0/1 + scores/exp for head 0) under its V-projection.
        xt_last = None
        att0 = None
        qtiles = []
        with tc.tile_pool(name="p1w", bufs=1) as p1:
            wk_sb = p1.tile([P, EO, HL * D], BF16)
            wv_sb = p1.tile([P, EO, HL * D], BF16)
            # Startup DMA plan.  Sync carries the startup-critical wk/x0
            # stream as interleaved eighth/quarter chunks (each trigger is
            # ~600ns of queue time, each chunk 0.7-2.9us of transfer);
            # Scalar (idle all of pass 1, first free ~8.8us after its
            # activation-table load) carries everything else in whole-tile
            # triggers, ordered by first use.
            # Each HWDGE queue streams ~200 B/ns, so the two startup-
            # critical tensors (wk, x0) ride DIFFERENT engines' queues in
            # matched chunks; everything else follows in first-use order.
            xt0 = xs.tile([P, EO, ST], BF16, tag="xt", name="xt0")
            xt1 = xs.tile([P, EO, ST], BF16, tag="xt", name="xt1")
            CH0 = [(0, 2), (2, 8), (8, 16)]
            nc.sync.dma_start(wk_sb[:, 0:2, :], wkP[:, 0:2, :])
            nc.scalar.dma_start(xt0[:, 0:2, :], xP[0][:, 0:2, :])
            nc.sync.dma_start(wk_sb[:, 2:8, :], wkP[:, 2:8, :])
            nc.scalar.dma_start(tbl_sb[:, :TSMALL], tblP[:, :TSMALL])
            nc.vector.tensor_copy(qkbf[:], tbl_sb[:, ds(QK0, 2 * HL)])
            nc.sync.dma_start(wk_sb[:, 8:16, :], wkP[:, 8:16, :])
            nc.scalar.dma_start(xt0[:, 2:8, :], xP[0][:, 2:8, :])
            nc.scalar.dma_start(xt0[:, 8:16, :], xP[0][:, 8:16, :])
            nc.sync.dma_start(wv_sb[:, 0:8, :], wvP[:, 0:8, :])
            nc.sync.dma_start(wv_sb[:, 8:16, :], wvP[:, 8:16, :])
            nc.scalar.dma_start(tbl_sb[:, TSMALL:], tblP[:, TSMALL:])
            nc.scalar.dma_start(xt1[:, 0:8, :], xP[1][:, 0:8, :])
            nc.scalar.dma_start(xt1[:, 8:16, :], xP[1][:, 8:16, :])

            for i in range(NS):
                if i == 0:
                    xt = xt0
                elif i == 1:
                    xt = xt1
                else:
                    xt = xs.tile([P, EO, ST], BF16, tag="xt")
                    nc.sync.dma_start(xt[:, 0:8, :], xP[i][:, 0:8, :])
                    nc.sync.dma_start(xt[:, 8:16, :], xP[i][:, 8:16, :])
                    if i == NS - 1:
                        # prefetch pass-2 weights behind this block's x:
                        # wq feeds the q-projections later in this block,
                        # ow the out-projection a block later
                        nc.sync.dma_start(wq_sb[:, 0:8, :], wqP[:, 0:8, :])
                        nc.sync.dma_start(wq_sb[:, 8:16, :], wqP[:, 8:16, :])
                        nc.sync.dma_start(
                            ow_sb[:],
                            owT[:].rearrange("(h p) e -> p h e", p=P))
                sl = ds(i * ST, ST)
                # K-projection: block 0 walks chunk-major with one live
                # PSUM accumulator per head so the PE consumes wk/x0
                # chunks as they land; later blocks' x is fully resident.
                chunks = CH0 if i == 0 else [(0, EO)]
                kps = [psA.tile([P, ST], F32, tag="acc", name=f"kps{jb}")
                       for jb in range(HL)]
                for ci, (a, b) in enumerate(chunks):
                    for jb in range(HL):
                        for eo in range(a, b):
                            nc.tensor.matmul(
                                kps[jb][:], wk_sb[:, eo, ds(jb * D, D)],
                                xt[:, eo, :],
                                start=(eo == 0), stop=(eo == EO - 1))
                    if i == 0 and ci < 2:
                        # keep the HAM clock-gate's activity window busy
                        # while the next wk/x0 chunk lands (psS is untouched
                        # through block 0's K, so wps is still the scores
                        # pool's most recent allocation)
                        for _ in range(34 + 6 * ci):
                            nc.tensor.matmul(
                                wps[:, :P], wtile[:], wtile[:],
                                start=True, stop=True)
                kropes = []
                for jb in range(HL):
                    qb = rope_begin(kps[jb], qkb_ap(jb))
                    kropes.append((qb, i * ST, kT_sb[:, jb, sl]))
                if i < NS - 1:
                    for sbl in range(ST // P):
                        sb = i * (ST // P) + sbl
                        ps = psS.tile([P, ST], F32, tag="sc")
                        for eo in range(EO):
                            nc.tensor.matmul(
                                ps[:, : HL * D], xt[:, eo, ds(sbl * P, P)],
                                wv_sb[:, eo, :],
                                start=(eo == 0), stop=(eo == EO - 1))
                        nc.vector.tensor_tensor(
                            v_sb[:, sb, :], ps[:, : HL * D], vb_ap, Alu.add)
                    for kr in kropes:
                        rope_finish(*kr)
                else:
                    for kr in kropes:
                        rope_finish(*kr)
                    # q-projections for the first two attention heads, then
                    # V-projection interleaved with scores/exp for head 0
                    psq0 = psA.tile([P, ST], F32, tag="acc")
                    for eo in range(EO):
                        qproj_mm(0, psq0, xt, eo)
                    qt0, rf0 = qproj_fin(0, psq0)
                    psq1 = psA.tile([P, ST], F32, tag="acc")
                    for eo in range(EO):
                        qproj_mm(1, psq1, xt, eo)
                    qt1, rf1 = qproj_fin(1, psq1)
                    qtiles.extend([qt0, qt1])
                    rope_finish(*rf0)
                    att0 = (at0.tile([P, 8, ST], BF16, name="att0A"),
                            at0.tile([P, 8, ST], BF16, name="att0B"))
                    sc_jb = 0
                    vps = None
                    for vi in range(4 * EO):
                        sbl, eo = vi // EO, vi % EO
                        if eo == 0:
                            vps = psA.tile([P, ST], F32, tag="acc")
                        nc.tensor.matmul(
                            vps[:, : HL * D], xt[:, eo, ds(sbl * P, P)],
                            wv_sb[:, eo, :],
                            start=(eo == 0), stop=(eo == EO - 1))
                        if eo == EO - 1:
                            nc.vector.tensor_tensor(
                                v_sb[:, i * (ST // P) + sbl, :],
                                vps[:, : HL * D], vb_ap, Alu.add)
                        if vi >= EO and (vi - EO) % 3 == 0 and sc_jb < JT:
                            scores_mm(0, att0, qt0, sc_jb)
                            sc_jb += 1
                        if vi == 40:
                            rope_finish(*rf1)
                if i == NS - 1:
                    xt_last = xt

        # ---- pass 2: flat software pipeline over (block, head) steps ----
        # Blocks run in reverse so the first one reuses pass 1's last x
        # tile.  At step k: att@V + denominator tree for head k, scores+exp
        # for head k+1, Q-projection for head k+2, one quarter of the
        # PREVIOUS block's out-projection, and the deferred denominator
        # ones-matmul + normalize for head k-1 -- all interleaved so the PE
        # stream (65 matmuls/step) hides the exp stream (16/step).
        with tc.tile_pool(name="attpa", bufs=3) as abA, \
             tc.tile_pool(name="attpb", bufs=3) as abB:

            xts = {order[0]: xt_last}

            def cblock_mm(ci, jb, pst, drain=False):
                # one of the 16x4 out-projection matmuls for token block ci;
                # jb runs 0..63 across the block's four steps.  Each row
                # block's four 512-col quarters collect in one oc tile and
                # leave in a single [P, E] DMA.
                tile_i, ho = jb // HL, jb % HL
                sb_loc, et = tile_i // ET, tile_i % ET
                sb = ci * (ST // P) + sb_loc
                if ho == 0:
                    pst[0] = psC.tile([P, ST], F32, tag="ct", name="ct")
                nc.tensor.matmul(
                    pst[0][:], ctxT_sb[:, ho, ds(sb * P, P)],
                    ow_sb[:, ho, ds(et * ST, ST)],
                    start=(ho == 0), stop=(ho == HL - 1))
                if ho == HL - 1:
                    if et == 0:
                        pst[1] = oc.tile([P, ET, ST], BF16, tag="ot",
                                         name="ot")
                    nc.vector.tensor_copy(pst[1][:, et, :], pst[0][:])
                    if drain and sb_loc == ST // P - 1:
                        # final row block: one small DMA per quarter right
                        # after its copy, alternating trigger engines, so
                        # the post-matmul tail transfer is short
                        eng = nc.scalar if (et % 2) else nc.sync
                        eng.dma_start(
                            out[ds(sb * P, P), ds(et * ST, ST)],
                            pst[1][:, et, :])
                    elif et == ET - 1:
                        if not drain:
                            nc.sync.dma_start(
                                out[ds(sb * P, P), :], pst[1][:])
                        else:
                            eng = nc.scalar if (sb_loc % 2) else nc.sync
                            eng.dma_start(out[ds(sb * P, P), :], pst[1][:])

            # denominator ones-matmul + normalize for head k; deferred one
            # step so the PE reaches the ones-matmul well after the VectorE
            # tree produced attB[:, 0, :].  1/d = Exp(-Ln(d)) on ScalarE
            # (ln/exp share an activation table: no reloads).  Split into
            # pieces so the last two steps can weave them into the matmul
            # stream instead of serializing after it.
            def finish_mm(attab):
                psd = psC.tile([P, ST], F32, tag="ct", name="psd")
                nc.tensor.matmul(
                    psd[:], ones_ap, attab[1][:, 0, :],
                    start=True, stop=True)
                return psd

            def finish_ln(psd):
                lnd = dp.tile([P, ST], F32, tag="lnd")
                nc.scalar.activation(lnd[:], psd[:], Act.Ln)
                return lnd

            def finish_exp(lnd):
                rec = dp.tile([P, ST], F32, tag="rec")
                nc.scalar.activation(rec[:], lnd[:], Act.Exp, scale=-1.0)
                return rec

            def finish_tt(k, psc, rec):
                nc.vector.tensor_tensor(
                    ctxT_sb[:, seq[k][1], ds(blk(k) * ST, ST)],
                    psc[:], rec[:], Alu.mult)

            def finish(k, attab, psc):
                finish_tt(k, psc, finish_exp(finish_ln(finish_mm(attab))))

            atts = [att0]

            cpst = [None, None]
            pending = None
            for k in range(NK):
                i, h = seq[k]
                if h == 0 and k + 4 < NK:
                    # prefetch the x tile for the NEXT block now; the DMA
                    # has a whole block (~55us) to land
                    nxt = blk(k + 4)
                    xtn = xs.tile([P, EO, ST], BF16, tag="xt")
                    nc.sync.dma_start(xtn[:], xP[nxt][:])
                    xts[nxt] = xtn
                att = atts[k]
                attA, attB = att
                ci_prev = blk(k - 4) if k >= 4 else None
                if k + 1 < NK:
                    attn = (abA.tile([P, 8, ST], BF16, tag="attA", name="attA"),
                            abB.tile([P, 8, ST], BF16, tag="attB", name="attB"))
                    atts.append(attn)
                else:
                    attn = None
                if k + 2 < NK:
                    psq = psA.tile([P, ST], F32, tag="acc")
                else:
                    psq = None
                psc = psA.tile([P, ST], F32, tag="acc")
                last = (k == NK - 1)
                # block-closing steps reduce the tree two slots early so
                # the finish-NOW chain lands sooner in the next step
                # tree two slots early everywhere: the deferred ones-
                # matmul reads attB[:, 0, :] at idx 2 of the NEXT step,
                # and level 4 at idx 15 + DVE queue lag just misses that
                tree_at = [7, 9, 11, 13]
                held = []
                psd15 = None
                for idx in range(JT):
                    if attn is not None:
                        scores_mm(k + 1, attn, qtiles[k + 1], idx)
                    jb = (idx + 8) % JT     # att@V: B half first
                    avs = (attA, attB)[jb // 8][:, jb % 8, :]
                    nc.tensor.matmul(
                        psc[:], v_sb[:, jb, ds(h * D, D)], avs,
                        start=(idx == 0), stop=(idx == JT - 1))
                    if psq is not None:
                        if h == HL - 1:
                            qproj_mm(k + 2, psq, xts[blk(k + 2)], idx)
                        else:
                            # compress: eo 14/15 ride slots 12/13 so the
                            # accumulation stops two slots early and the
                            # rope chain (idx 13/15 below) drains inside
                            # the step instead of at its boundary
                            if idx <= 13:
                                qproj_mm(k + 2, psq, xts[blk(k + 2)], idx)
                            if 12 <= idx <= 13:
                                qproj_mm(k + 2, psq, xts[blk(k + 2)],
                                         idx + 2)
                    if not last and pending is not None:
                        # pending finish at idx 0-3, the ones-matmul BEFORE
                        # this slot's cblock matmul: its psC tile then sits
                        # AHEAD of cblock group 1 in the bank ring and its
                        # reader is the fast Ln at idx 1, so every cblock
                        # group alloc gets >=4 slots of margin over its
                        # predecessor's copy (g2 no longer waits g1's CAST)
                        if idx == 0:
                            pk, patt, ppsc = pending
                            p_psd = finish_mm(patt)
                        elif idx == 1:
                            p_lnd = finish_ln(p_psd)
                        elif idx == 2:
                            p_rec = finish_exp(p_lnd)
                        elif idx == 3:
                            finish_tt(pk, ppsc, p_rec)
                            pending = None
                    if ci_prev is not None:
                        if k % 4 != 0:
                            if last and idx >= 4:
                                # hold the tail of the previous block's
                                # out-projection so it can hide the final
                                # normalize's Ln/Exp latency (below)
                                held.append((k % 4) * JT + idx)
                            else:
                                cblock_mm(ci_prev, (k % 4) * JT + idx, cpst)
                        elif idx >= 8:
                            # block-boundary step: the previous block's ctx
                            # normalize lands ~1us in, so weave its out-
                            # projection into the back half, two per slot
                            cblock_mm(ci_prev, (idx - 8) * 2, cpst)
                            cblock_mm(ci_prev, (idx - 8) * 2 + 1, cpst)
                    if not last:
                        if psq is not None and h < HL - 1:
                            if idx == 13:
                                qt, rf_early = qproj_fin(k + 2, psq)
                                qtiles.append(qt)
                            elif idx == 15:
                                rope_finish(*rf_early)
                        if h == HL - 1:
                            # block-closing step: own-head ones-matmul/Ln
                            # woven into idx 14-15 so ctx is final ~1us
                            # into the next (boundary) step
                            if idx == 14:
                                now_psd = finish_mm(att)
                            elif idx == 15:
                                now_lnd = finish_ln(now_psd)
                        # denominator tree levels woven into the matmul
                        # stream; they only ever write attB, whose att@V
                        # reads all finished at idx 7
                        if idx == tree_at[0]:
                            nc.vector.tensor_tensor(
                                attB[:], attB[:], attA[:], Alu.add)
                        elif idx == tree_at[1]:
                            nc.vector.tensor_tensor(
                                attB[:, 0:4, :], attB[:, 0:4, :],
                                attB[:, 4:8, :], Alu.add)
                        elif idx == tree_at[2]:
                            nc.vector.tensor_tensor(
                                attB[:, 0:2, :], attB[:, 0:2, :],
                                attB[:, 2:4, :], Alu.add)
                        elif idx == tree_at[3]:
                            nc.vector.tensor_tensor(
                                attB[:, 0, :], attB[:, 0, :], attB[:, 1, :],
                                Alu.add)
                    else:
                        # LAST step: nothing interleaves after it, so the
                        # usual deferred chains would serialize behind the
                        # matmul stream.  Weave the pending head's finish
                        # into idx 0-3, halve the tree (level 1 only, the
                        # first half written into step NK-2's retired attB
                        # so the coarse dep-tracker sees no write to a tile
                        # the PE still reads), and let the PE itself do the
                        # final 8-way reduction as an accumulating
                        # ones-matmul group in idx 8-15.
                        attB_prev = atts[NK - 2][1]
                        if idx == 0:
                            pk, patt, ppsc = pending
                            p_psd = finish_mm(patt)
                        elif idx == 1:
                            p_lnd = finish_ln(p_psd)
                        elif idx == 2:
                            p_rec = finish_exp(p_lnd)
                        elif idx == 3:
                            finish_tt(pk, ppsc, p_rec)
                            pending = None
                        elif idx == 4:
                            nc.vector.tensor_tensor(
                                attB_prev[:, 0:4, :], attB[:, 0:4, :],
                                attA[:, 0:4, :], Alu.add)
                        elif idx == 7:
                            nc.vector.tensor_tensor(
                                attB[:, 4:8, :], attB[:, 4:8, :],
                                attA[:, 4:8, :], Alu.add)
                        if idx >= 8:
                            s = idx - 8
                            src = attB_prev[:, s, :] if s < 4 \
                                else attB[:, s, :]
                            if s == 0:
                                psd15 = psA.tile([P, ST], F32, tag="acc",
                                                 name="psd15")
                            nc.tensor.matmul(
                                psd15[:], ones_ap, src,
                                start=(s == 0), stop=(s == 7))
                if psq is not None and (h == HL - 1 or last):
                    qt, rf = qproj_fin(k + 2, psq)
                    qtiles.append(qt)
                else:
                    rf = None
                if pending is not None:
                    finish(*pending)
                    pending = None
                if h == HL - 1:
                    # last head of the block: finish NOW so the next
                    # block's interleaved out-projection reads final ctx
                    if last:
                        finish_tt(k, psc, finish_exp(finish_ln(psd15)))
                    else:
                        finish_tt(k, psc, finish_exp(now_lnd))
                    for jb in held:
                        cblock_mm(ci_prev, jb, cpst)
                else:
                    pending = (k, att, psc)
                if rf is not None:
                    rope_finish(*rf)

            # the last block's out-projection has no next block to hide in
            cpst = [None, None]
            for jb in range(4 * JT - 4):
                cblock_mm(blk(NK - 1), jb, cpst, drain=True)
            # final tile: accumulate the two column halves as separate
            # groups so the first half's copy+DMA overlaps the second
            # half's matmuls and the tail transfer is only [P, 256]
            ci = blk(NK - 1)
            sb = ci * (ST // P) + 3
            for hf in range(2):
                psf = psC.tile([P, ST // 2], F32, tag="ct", name="ctf")
                for ho in range(HL):
                    nc.tensor.matmul(
                        psf[:], ctxT_sb[:, ho, ds(sb * P, P)],
                        ow_sb[:, ho, ds(3 * ST + hf * (ST // 2), ST // 2)],
                        start=(ho == 0), stop=(ho == HL - 1))
                otf = oc.tile([P, ST // 2], BF16, tag="otf", name="otf")
                nc.vector.tensor_copy(otf[:], psf[:])
                eng = nc.scalar if hf else nc.sync
                eng.dma_start(
                    out[ds(sb * P, P), ds(3 * ST + hf * (ST // 2), ST // 2)],
                    otf[:])

    return nc


def _rope_tables():
    inv_freq = 1.0 / (10000.0 ** (np.arange(0, D, 2, dtype=np.float32) / D))
    t = np.arange(S, dtype=np.float32)
    freqs = np.einsum("s,f->sf", t, inv_freq)
    emb = np.concatenate([freqs, freqs], axis=-1)
    cosT = np.cos(emb).astype(np.float32).T.copy()
    sinT = np.sin(emb).astype(np.float32).T.copy()
    # rotate-half sign lives in the on-device permutation matrix
    return cosT.astype(BF), np.ascontiguousarray(sinT).astype(BF)


def _core_inputs(x, Wqkv_w, Wqkv_b, out_w, b, g, shared, xT_bf):
    # k-head columns first, then q-head columns (matches kernel layout)
    k_cols, q_cols, kb_rows, qb_rows = [], [], [], []
    for hl in range(HL):
        h = g * HL + hl
        q_cols.append(Wqkv_w[h * D:(h + 1) * D, :].T)
        k_cols.append(Wqkv_w[E + h * D:E + (h + 1) * D, :].T)
        qb_rows.append(Wqkv_b[h * D:(h + 1) * D])
        kb_rows.append(Wqkv_b[E + h * D:E + (h + 1) * D])
    def pack(wT):
        # [E, HL*D] -> [P, EO, HL*D]: per-partition contiguous rows so
        # the whole tensor moves in one (or few) large-descriptor DMAs
        return np.ascontiguousarray(
            wT.reshape(E // P, P, HL * D).transpose(1, 0, 2)).astype(BF)

    wkP = pack(np.concatenate(k_cols, axis=1))
    wqP = pack(np.concatenate(q_cols, axis=1))
    qkbT = np.stack(kb_rows + qb_rows).astype(np.float32).T    # [D, 2HL]
    v0 = 2 * E + g * HL * D
    wvP = pack(Wqkv_w[v0:v0 + HL * D, :].T)
    vb = Wqkv_b[v0:v0 + HL * D].astype(np.float32)
    owT = np.ascontiguousarray(
        out_w[:, g * HL * D:(g + 1) * HL * D].T).astype(BF)
    cosT, sinT, ones, perm = shared
    # qkb | vb | ones | perm | cos | sin (kernel's tbl layout); biases in
    # bf16 cost ~0.4% of their 0.01-scale values -- negligible
    tblP = np.ascontiguousarray(np.concatenate(
        [qkbT, np.broadcast_to(vb[None, :], (P, HL * D)), ones, perm,
         cosT, sinT], axis=1).astype(np.float32)).astype(BF)
    return {"xP": xT_bf, "wkP": wkP, "wqP": wqP, "wvP": wvP,
            "tblP": tblP, "owT": owT}


def kernel(x, Wqkv_w, Wqkv_b, out_w, out_b):
    global LAST_EXEC_NS
    _install_axon_ntff_shim()
    from concourse.bass_utils import run_bass_kernel_spmd

    x = np.asarray(x, dtype=np.float32)
    Wqkv_w = np.asarray(Wqkv_w, dtype=np.float32)
    Wqkv_b = np.asarray(Wqkv_b, dtype=np.float32)
    out_w = np.asarray(out_w, dtype=np.float32)
    out_b = np.asarray(out_b, dtype=np.float32)

    cosT, sinT = _rope_tables()
    # rotate-half permutation: out[d] = -q[d+64] (d<64), +q[d-64] (d>=64)
    perm = np.zeros((P, P), dtype=np.float32)
    for d in range(D // 2):
        perm[d + D // 2, d] = -1.0
        perm[d, d + D // 2] = 1.0
    shared = (cosT.astype(np.float32), sinT.astype(np.float32),
              np.ones((P, P), np.float32), perm)
    # x packed as [NS, P, EO, ST]: xP[i, p, eo, s] = x[b, i*ST+s, eo*P+p]
    NS, EO, ST = S // 512, E // P, 512
    xT_bf = [np.ascontiguousarray(
        x[b].reshape(NS, ST, EO, P).transpose(0, 3, 2, 1)).astype(BF)
        for b in range(2)]
    in_maps = []
    for core in range(8):
        b, g = core // 4, core % 4
        in_maps.append(
            _core_inputs(x, Wqkv_w, Wqkv_b, out_w, b, g, shared, xT_bf[b]))

    nc = bass.Bass()
    _build_mha(nc)
    _split_multi_waits(nc)

    trace = bool(os.environ.get("MHA_TRACE"))
    if trace:
        # dev-only profiling path; skip the S3 artifact upload
        import concourse.bass_utils as _bu
        _bu.upload_artifacts = lambda tmpdir: tmpdir
    res = run_bass_kernel_spmd(
        nc, in_maps, core_ids=list(range(8)), trace=trace)
    if trace:
        LAST_EXEC_NS = res.exec_time_ns

    out = np.empty((2, S, E), dtype=np.float32)
    for b in range(2):
        acc = res.results[b * 4 + 0]["out"].astype(np.float32)
        for g in range(1, 4):
            acc += res.results[b * 4 + g]["out"].astype(np.float32)
        out[b] = acc + out_b[None, :]
    return out


# revision 26
# speedup vs baseline: 1.0111x; 1.0018x over previous
"""Sharded MHA-with-RoPE Trainium2 kernel (nn_CustomTorchMHASelf).

Contract: kernel(**inputs) takes the FULL unsharded inputs of the
reference (x [2,2048,2048], Wqkv_w [6144,2048], Wqkv_b [6144],
out_w [2048,2048], out_b [2048]) and returns the full [2,2048,2048]
fp32 output, running the compute on 8 NeuronCores.

Sharding: core = b*4 + g handles batch b and head-group g (4 of the 16
heads). Each core computes q/k/v projections for its heads, RoPE,
softmax attention, and its slice of the out-projection; the host sums
the 4 partial outputs per batch and adds out_b.

Device data plane is bf16 (fp32 PSUM accumulation); the host
pre-transposes x and the weight slices into the layouts the TensorE
wants (contraction dim on partitions everywhere).

Schedule: pass 1 computes K+RoPE and V for all tokens (the last block
also hides the attention prologue under its V-projection); pass 2 is a
flat software pipeline over (block, head) steps -- at step k the PE
stream interleaves att@V(k), scores(k+1), q-projection(k+2) and a
quarter of the previous block's out-projection (65 matmuls/step), so
the ScalarE exp stream (16/step) is never on the critical path.
Key device tricks:
  - DMA triggers cost ~600ns of issuing-engine queue time EACH
    regardless of size, so tensors move with ONE trigger per tile
    (16KB-per-partition contiguous descriptor runs) instead of 16;
    the startup-critical wk/x0 pair is split into eighth/quarter
    chunks issued alternately so the first K matmuls start ~10.5us
    in, and block 0's K-projection walks chunk-major (4 live PSUM
    accumulators) to consume chunks as they land;
  - rotate-half for RoPE is a PE matmul with a signed permutation
    matrix (SBUF-SBUF DMA swaps are slow and their DIRECT2D triggers
    serialize on the sync sequencer);
  - the softmax denominator is a bf16 tree-add into the attB tile on
    VectorE plus ONE ones-matmul per (head, block) instead of 16 full
    PE ones-matmuls; att is split into two tiles (attA/attB) so the
    tree's writes never alias tiles the PE still reads (the dep
    tracker is coarse); the ones-matmul+normalize are deferred one
    step so the PE never waits on the tree;
  - 1/denominator = Exp(-Ln(d)) on ScalarE (ln and exp share an
    activation table, so no table reloads) because DVE reciprocal is
    slow and custom-DVE ops don't compile on this toolchain;
  - deferred finishes are woven into matmul slots 0-3 of the next step
    with the ones-matmul emitted BEFORE that slot's out-projection
    matmul, so the finish tile leads the psC bank ring (its reader is
    the fast Ln) and the Ln/Exp never lands at a step boundary where
    the score matmuls wait on psS;
  - warm-up matmuls on a memset tile (no DMA dependency) ramp the PE
    out of its 1.2GHz cold p-state while the first weight/x DMAs land,
    and filler matmuls after the first two K chunks keep the HAM
    activity window busy so the clock never re-throttles mid-startup;
  - output tiles are written bf16, one batched [P, E] DMA per token
    row-block; in the last step the tree is halved (level 1 written
    into a retired att tile), the PE itself does the final 8-way
    denominator reduction as an accumulating ones-matmul group, and
    the drain's final tile is split into column halves so the tail
    transfer is short.
"""

import math
import os
import sys
import types

import numpy as np
import ml_dtypes

import concourse.bass as bass
import concourse.mybir as mybir
import concourse.tile as tile
from concourse.bass import ds

F32 = mybir.dt.float32
BF16 = mybir.dt.bfloat16
Alu = mybir.AluOpType
Act = mybir.ActivationFunctionType
BF = ml_dtypes.bfloat16

S, E, HTOT, HL, D, P = 2048, 2048, 16, 4, 128, 128

# Filled with the profile exec time (ns) when MHA_TRACE=1; read by test.py.
LAST_EXEC_NS = None


def _install_axon_ntff_shim():
    """Provide antenv.axon_hooks so trace=True can reach the axon NTFF hook."""
    if "antenv.axon_hooks" in sys.modules:
        return
    mod = types.ModuleType("antenv.axon_hooks")
    holder = [None]
    mod.set_axon_ntff_profile_hook = lambda h: holder.__setitem__(0, h)
    mod.get_axon_ntff_profile_hook = lambda: holder[0]
    sys.modules["antenv.axon_hooks"] = mod
    try:
        import antenv
        antenv.axon_hooks = mod
    except ImportError:
        pass
    # boot() ran at interpreter start (sitecustomize), before this module
    # existed, so its NTFF-hook registration was silently skipped. Redo it.
    try:
        from trn_agent_boot.trn_boot import _ntff_profile_via_ctypes
        hook = _ntff_profile_via_ctypes("/opt/axon/libaxon_pjrt.so")
        if hook is not None:
            mod.set_axon_ntff_profile_hook(hook)
    except Exception:
        pass


def _split_multi_waits(nc):
    """Hoist extra sem-waits onto standalone NoOps (one wait per inst).

    This walrus build rejects any instruction carrying more than one
    sync-wait ("Too many sync wait commands"); Tile attaches one wait per
    outstanding semaphore to the consuming instruction. Splitting them
    across same-engine NoOps placed immediately before is equivalent:
    the engine executes serially, so all waits still precede the inst.
    """
    ctr = 0
    for fn in nc.m.functions:
        for blk in fn.blocks:
            out = []
            for inst in blk.instructions:
                si = getattr(inst, "sync_info", None)
                if si is not None and si.on_wait is not None \
                        and len(si.on_wait) > 1:
                    waits = list(si.on_wait)
                    si.on_wait = [waits[-1]]
                    for w in waits[:-1]:
                        ctr += 1
                        nop = mybir.InstNoOp(
                            name=f"I-wsplit-{ctr}", ins=[], outs=[])
                        nop.engine = inst.engine
                        nop.sync_info = mybir.SyncInfo(
                            on_wait=[w], on_update=[])
                        out.append(nop)
                out.append(inst)
            blk.instructions[:] = out


def _build_mha(nc: bass.Bass):
    """Emit the per-core MHA program (one shard) into `nc`."""
    EO = E // P            # contraction subtiles for the projections
    ST = 512               # free-dim tile (one PSUM bank of fp32)
    NS = S // ST
    SB = S // P
    JT = S // P            # key blocks per head
    ET = E // ST

    # packed layouts: [.., P, EO, ST] so each tensor is one long
    # per-partition-contiguous run -> ONE ~600ns DMA trigger moves it
    xP = nc.dram_tensor("xP", [NS, P, EO, ST], BF16, kind="ExternalInput")
    wkP = nc.dram_tensor("wkP", [P, EO, HL * D], BF16, kind="ExternalInput")
    wqP = nc.dram_tensor("wqP", [P, EO, HL * D], BF16, kind="ExternalInput")
    wvP = nc.dram_tensor("wvP", [P, EO, HL * D], BF16, kind="ExternalInput")
    # qkb | vb | ones | perm | cos | sin packed into one bf16 table;
    # the small head (biases + matrices) rides an early DMA slice, the
    # big cos/sin tail a later one
    TW = 2 * HL + HL * D + 2 * P + 2 * S
    tblP = nc.dram_tensor("tblP", [P, TW], BF16, kind="ExternalInput")
    owT = nc.dram_tensor("owT", [HL * D, E], BF16, kind="ExternalInput")
    out = nc.dram_tensor("out", [S, E], BF16, kind="ExternalOutput")

    isc = 1.0 / math.sqrt(D)

    from contextlib import ExitStack

    with tile.TileContext(nc) as tc, ExitStack() as stk:
        persist = stk.enter_context(tc.tile_pool(name="persist", bufs=1))
        kT_sb = persist.tile([P, HL, S], BF16)      # k post-RoPE [d, h, s]
        v_sb = persist.tile([P, SB, HL * D], BF16)  # v natural [s%128, s//128, hd]
        ctxT_sb = persist.tile([P, HL, S], BF16)    # [d, h, i]
        tbl_sb = persist.tile([P, TW], BF16)
        ow_sb = persist.tile([P, HL, E], BF16)
        wtile = persist.tile([P, P], BF16)          # memset warm-up operand

        QK0, VB0 = 0, 2 * HL
        ON0 = VB0 + HL * D
        PM0 = ON0 + P
        CS0, SN0 = PM0 + P, PM0 + P + S
        TSMALL = PM0 + P          # early slice: biases + ones + perm

        # tensor_scalar needs f32 scalars: widen the bf16 biases once
        qkbf = persist.tile([P, 2 * HL], F32)

        def qkb_ap(j):
            return qkbf[:, j, None]

        vb_ap = tbl_sb[:, ds(VB0, HL * D)]

        def cos_ap(s0):
            return tbl_sb[:, ds(CS0 + s0, ST)]

        def sin_ap(s0):
            return tbl_sb[:, ds(SN0 + s0, ST)]

        ones_ap = tbl_sb[:, ds(ON0, P)]
        perm_ap = tbl_sb[:, ds(PM0, P)]

        # x stream shared by both passes; rope temps likewise.  qb/rot are
        # still being read (by the rotate matmul / mults) when the next rope
        # starts, so they get extra bufs; t1/t2 are consumed immediately by
        # the in-order VectorE queue, so 1 buf suffices.
        xs = stk.enter_context(tc.tile_pool(name="xstream", bufs=2))
        rta = stk.enter_context(tc.tile_pool(name="ropetmpa", bufs=3))
        rtb = stk.enter_context(tc.tile_pool(name="ropetmpb", bufs=1))
        wqp = stk.enter_context(tc.tile_pool(name="wqpool", bufs=1))
        wq_sb = wqp.tile([P, EO, HL * D], BF16)

        psA = stk.enter_context(tc.tile_pool(name="psA", bufs=4, space="PSUM"))
        psS = stk.enter_context(tc.tile_pool(name="psS", bufs=2, space="PSUM"))
        psC = stk.enter_context(tc.tile_pool(name="psC", bufs=2, space="PSUM"))

        qp = stk.enter_context(tc.tile_pool(name="qpool", bufs=4))
        dp = stk.enter_context(tc.tile_pool(name="denp", bufs=1))
        oc = stk.enter_context(tc.tile_pool(name="ocopy", bufs=2))
        at0 = stk.enter_context(tc.tile_pool(name="att0p", bufs=1))

        # flat (block, head) schedule for the attention pass; blocks in
        # reverse order so the first one reuses pass 1's last x tile
        order = list(range(NS - 1, -1, -1))
        seq = [(i, h) for i in order for h in range(HL)]
        NK = len(seq)

        def blk(k):
            return seq[k][0]

        def rope_begin(ps, bias_ap):
            # qb = q + bias (bf16 so the rotate-half matmul runs full rate)
            qb = rta.tile([P, ST], BF16, tag="qb")
            nc.vector.tensor_scalar_add(qb[:], ps[:], bias_ap)
            return qb

        def rope_finish(qb, s0, out_ap):
            # rotate-half as a PE matmul with a signed permutation matrix
            # (cross-partition moves otherwise need a slow SBUF-SBUF DMA
            # whose trigger also serializes on the sync sequencer);
            # out = qb*cos + (perm.T @ qb)*sin.
            # rps lives in psC so the scores stream owns psS's two banks
            # outright (16 even allocations per step -> the exp reader is
            # always 2 full matmul-slots behind the next allocation)
            rps = psC.tile([P, ST], F32, tag="ct", name="rps")
            nc.tensor.matmul(rps[:], perm_ap, qb[:], start=True, stop=True)
            t1 = rtb.tile([P, ST], BF16, tag="t1")
            t2 = rtb.tile([P, ST], BF16, tag="t2")
            nc.vector.tensor_tensor(t1[:], qb[:], cos_ap(s0), Alu.mult)
            nc.vector.tensor_tensor(t2[:], rps[:], sin_ap(s0), Alu.mult)
            nc.vector.tensor_tensor(out_ap, t1[:], t2[:], Alu.add)

        def qproj_mm(k, psq, xt, eo):
            h = seq[k][1]
            nc.tensor.matmul(
                psq[:], wq_sb[:, eo, ds(h * D, D)], xt[:, eo, :],
                start=(eo == 0), stop=(eo == EO - 1))

        def qproj_fin(k, psq):
            qt = qp.tile([P, ST], BF16, tag="qt")
            qb = rope_begin(psq, qkb_ap(HL + seq[k][1]))
            return qt, (qb, blk(k) * ST, qt[:])

        def scores_mm(k, attab, qt, jb):
            h = seq[k][1]
            ps = psS.tile([P, ST], F32, tag="sc")
            nc.tensor.matmul(
                ps[:], kT_sb[:, h, ds(jb * P, P)], qt[:],
                start=True, stop=True)
            dst = attab[jb // 8][:, jb % 8, :]
            nc.scalar.activation(dst, ps[:], Act.Exp, scale=isc)

        # PE warm-up on a memset tile (no DMA dependency): the HAM clock
        # gate needs ~3.4us of sustained PE activity to release the cold
        # 1.2GHz p-state, and the first real matmul data lands ~10.5us in.
        nc.gpsimd.memset(wtile[:], 1.0)
        wsink = persist.tile([P, 1], F32)
        wps = psS.tile([P, ST], F32, tag="sc", name="warmps")
        NWARM = 44
        for w in range(NWARM):
            nc.tensor.matmul(
                wps[:, :P], wtile[:], wtile[:],
                start=(w == 0), stop=(w == NWARM - 1))
        nc.vector.tensor_copy(wsink[:], wps[:, :1])

        # ---- pass 1: K projection + RoPE, V projection ----
        # The last block additionally hides the attention pass's prologue
        # (q heads 0/1 + scores/exp for head 0) under its V-projection.
        xt_last = None
        att0 = None
        qtiles = []
        with tc.tile_pool(name="p1w", bufs=1) as p1:
            wk_sb = p1.tile([P, EO, HL * D], BF16)
            wv_sb = p1.tile([P, EO, HL * D], BF16)
            # Startup DMA plan.  Sync carries the startup-critical wk/x0
            # stream as interleaved eighth/quarter chunks (each trigger is
            # ~600ns of queue time, each chunk 0.7-2.9us of transfer);
            # Scalar (idle all of pass 1, first free ~8.8us after its
            # activation-table load) carries everything else in whole-tile
            # triggers, ordered by first use.
            # Each HWDGE queue streams ~200 B/ns, so the two startup-
            # critical tensors (wk, x0) ride DIFFERENT engines' queues in
            # matched chunks; everything else follows in first-use order.
            xt0 = xs.tile([P, EO, ST], BF16, tag="xt", name="xt0")
            xt1 = xs.tile([P, EO, ST], BF16, tag="xt", name="xt1")
            CH0 = [(0, 2), (2, 8), (8, 16)]
            nc.sync.dma_start(wk_sb[:, 0:2, :], wkP[:, 0:2, :])
            nc.scalar.dma_start(xt0[:, 0:2, :], xP[0][:, 0:2, :])
            nc.sync.dma_start(wk_sb[:, 2:8, :], wkP[:, 2:8, :])
            nc.scalar.dma_start(tbl_sb[:, :TSMALL], tblP[:, :TSMALL])
            nc.vector.tensor_copy(qkbf[:], tbl_sb[:, ds(QK0, 2 * HL)])
            nc.sync.dma_start(wk_sb[:, 8:16, :], wkP[:, 8:16, :])
            nc.scalar.dma_start(xt0[:, 2:8, :], xP[0][:, 2:8, :])
            nc.scalar.dma_start(xt0[:, 8:16, :], xP[0][:, 8:16, :])
            nc.sync.dma_start(wv_sb[:, 0:8, :], wvP[:, 0:8, :])
            nc.sync.dma_start(wv_sb[:, 8:16, :], wvP[:, 8:16, :])
            nc.scalar.dma_start(tbl_sb[:, TSMALL:], tblP[:, TSMALL:])
            nc.scalar.dma_start(xt1[:, 0:8, :], xP[1][:, 0:8, :])
            nc.scalar.dma_start(xt1[:, 8:16, :], xP[1][:, 8:16, :])

            for i in range(NS):
                if i == 0:
                    xt = xt0
                elif i == 1:
                    xt = xt1
                else:
                    xt = xs.tile([P, EO, ST], BF16, tag="xt")
                    nc.sync.dma_start(xt[:, 0:8, :], xP[i][:, 0:8, :])
                    nc.sync.dma_start(xt[:, 8:16, :], xP[i][:, 8:16, :])
                    if i == NS - 1:
                        # prefetch pass-2 weights behind this block's x:
                        # wq feeds the q-projections later in this block,
                        # ow the out-projection a block later
                        nc.sync.dma_start(wq_sb[:, 0:8, :], wqP[:, 0:8, :])
                        nc.sync.dma_start(wq_sb[:, 8:16, :], wqP[:, 8:16, :])
                        nc.sync.dma_start(
                            ow_sb[:],
                            owT[:].rearrange("(h p) e -> p h e", p=P))
                sl = ds(i * ST, ST)
                # K-projection: block 0 walks chunk-major with one live
                # PSUM accumulator per head so the PE consumes wk/x0
                # chunks as they land; later blocks' x is fully resident.
                chunks = CH0 if i == 0 else [(0, EO)]
                kps = [psA.tile([P, ST], F32, tag="acc", name=f"kps{jb}")
                       for jb in range(HL)]
                for ci, (a, b) in enumerate(chunks):
                    for jb in range(HL):
                        for eo in range(a, b):
                            nc.tensor.matmul(
                                kps[jb][:], wk_sb[:, eo, ds(jb * D, D)],
                                xt[:, eo, :],
                                start=(eo == 0), stop=(eo == EO - 1))
                    if i == 0 and ci < 2:
                        # keep the HAM clock-gate's activity window busy
                        # while the next wk/x0 chunk lands (psS is untouched
                        # through block 0's K, so wps is still the scores
                        # pool's most recent allocation)
                        for _ in range(34 + 6 * ci):
                            nc.tensor.matmul(
                                wps[:, :P], wtile[:], wtile[:],
                                start=True, stop=True)
                kropes = []
                for jb in range(HL):
                    qb = rope_begin(kps[jb], qkb_ap(jb))
                    kropes.append((qb, i * ST, kT_sb[:, jb, sl]))
                if i < NS - 1:
                    for sbl in range(ST // P):
                        sb = i * (ST // P) + sbl
                        ps = psS.tile([P, ST], F32, tag="sc")
                        for eo in range(EO):
                            nc.tensor.matmul(
                                ps[:, : HL * D], xt[:, eo, ds(sbl * P, P)],
                                wv_sb[:, eo, :],
                                start=(eo == 0), stop=(eo == EO - 1))
                        nc.vector.tensor_tensor(
                            v_sb[:, sb, :], ps[:, : HL * D], vb_ap, Alu.add)
                    for kr in kropes:
                        rope_finish(*kr)
                else:
                    for kr in kropes:
                        rope_finish(*kr)
                    # q-projections for the first two attention heads, then
                    # V-projection interleaved with scores/exp for head 0
                    psq0 = psA.tile([P, ST], F32, tag="acc")
                    for eo in range(EO):
                        qproj_mm(0, psq0, xt, eo)
                    qt0, rf0 = qproj_fin(0, psq0)
                    psq1 = psA.tile([P, ST], F32, tag="acc")
                    for eo in range(EO):
                        qproj_mm(1, psq1, xt, eo)
                    qt1, rf1 = qproj_fin(1, psq1)
                    qtiles.extend([qt0, qt1])
                    rope_finish(*rf0)
                    att0 = (at0.tile([P, 8, ST], BF16, name="att0A"),
                            at0.tile([P, 8, ST], BF16, name="att0B"))
                    sc_jb = 0
                    vps = None
                    for vi in range(4 * EO):
                        sbl, eo = vi // EO, vi % EO
                        if eo == 0:
                            vps = psA.tile([P, ST], F32, tag="acc")
                        nc.tensor.matmul(
                            vps[:, : HL * D], xt[:, eo, ds(sbl * P, P)],
                            wv_sb[:, eo, :],
                            start=(eo == 0), stop=(eo == EO - 1))
                        if eo == EO - 1:
                            nc.vector.tensor_tensor(
                                v_sb[:, i * (ST // P) + sbl, :],
                                vps[:, : HL * D], vb_ap, Alu.add)
                        if vi >= EO and (vi - EO) % 3 == 0 and sc_jb < JT:
                            scores_mm(0, att0, qt0, sc_jb)
                            sc_jb += 1
                        if vi == 40:
                            rope_finish(*rf1)
                if i == NS - 1:
                    xt_last = xt

        # ---- pass 2: flat software pipeline over (block, head) steps ----
        # Blocks run in reverse so the first one reuses pass 1's last x
        # tile.  At step k: att@V + denominator tree for head k, scores+exp
        # for head k+1, Q-projection for head k+2, one quarter of the
        # PREVIOUS block's out-projection, and the deferred denominator
        # ones-matmul + normalize for head k-1 -- all interleaved so the PE
        # stream (65 matmuls/step) hides the exp stream (16/step).
        with tc.tile_pool(name="attpa", bufs=3) as abA, \
             tc.tile_pool(name="attpb", bufs=3) as abB:

            xts = {order[0]: xt_last}

            def cblock_mm(ci, jb, pst, drain=False):
                # one of the 16x4 out-projection matmuls for token block ci;
                # jb runs 0..63 across the block's four steps.  Each row
                # block's four 512-col quarters collect in one oc tile and
                # leave in a single [P, E] DMA.
                tile_i, ho = jb // HL, jb % HL
                sb_loc, et = tile_i // ET, tile_i % ET
                sb = ci * (ST // P) + sb_loc
                if ho == 0:
                    pst[0] = psC.tile([P, ST], F32, tag="ct", name="ct")
                nc.tensor.matmul(
                    pst[0][:], ctxT_sb[:, ho, ds(sb * P, P)],
                    ow_sb[:, ho, ds(et * ST, ST)],
                    start=(ho == 0), stop=(ho == HL - 1))
                if ho == HL - 1:
                    if et == 0:
                        pst[1] = oc.tile([P, ET, ST], BF16, tag="ot",
                                         name="ot")
                    nc.vector.tensor_copy(pst[1][:, et, :], pst[0][:])
                    if drain and sb_loc == ST // P - 1:
                        # final row block: one small DMA per quarter right
                        # after its copy, alternating trigger engines, so
                        # the post-matmul tail transfer is short
                        eng = nc.scalar if (et % 2) else nc.sync
                        eng.dma_start(
                            out[ds(sb * P, P), ds(et * ST, ST)],
                            pst[1][:, et, :])
                    elif et == ET - 1:
                        if not drain:
                            nc.sync.dma_start(
                                out[ds(sb * P, P), :], pst[1][:])
                        else:
                            eng = nc.scalar if (sb_loc % 2) else nc.sync
                            eng.dma_start(out[ds(sb * P, P), :], pst[1][:])

            # denominator ones-matmul + normalize for head k; deferred one
            # step so the PE reaches the ones-matmul well after the VectorE
            # tree produced attB[:, 0, :].  1/d = Exp(-Ln(d)) on ScalarE
            # (ln/exp share an activation table: no reloads).  Split into
            # pieces so the last two steps can weave them into the matmul
            # stream instead of serializing after it.
            def finish_mm(attab):
                psd = psC.tile([P, ST], F32, tag="ct", name="psd")
                nc.tensor.matmul(
                    psd[:], ones_ap, attab[1][:, 0, :],
                    start=True, stop=True)
                return psd

            def finish_ln(psd):
                lnd = dp.tile([P, ST], F32, tag="lnd")
                nc.scalar.activation(lnd[:], psd[:], Act.Ln)
                return lnd

            def finish_exp(lnd):
                rec = dp.tile([P, ST], F32, tag="rec")
                nc.scalar.activation(rec[:], lnd[:], Act.Exp, scale=-1.0)
                return rec

            def finish_tt(k, psc, rec):
                nc.vector.tensor_tensor(
                    ctxT_sb[:, seq[k][1], ds(blk(k) * ST, ST)],
                    psc[:], rec[:], Alu.mult)

            def finish(k, attab, psc):
                finish_tt(k, psc, finish_exp(finish_ln(finish_mm(attab))))

            atts = [att0]

            cpst = [None, None]
            pending = None
            for k in range(NK):
                i, h = seq[k]
                if h == 0 and k + 4 < NK:
                    # prefetch the x tile for the NEXT block now; the DMA
                    # has a whole block (~55us) to land
                    nxt = blk(k + 4)
                    xtn = xs.tile([P, EO, ST], BF16, tag="xt")
                    nc.sync.dma_start(xtn[:], xP[nxt][:])
                    xts[nxt] = xtn
                att = atts[k]
                attA, attB = att
                ci_prev = blk(k - 4) if k >= 4 else None
                if k + 1 < NK:
                    attn = (abA.tile([P, 8, ST], BF16, tag="attA", name="attA"),
                            abB.tile([P, 8, ST], BF16, tag="attB", name="attB"))
                    atts.append(attn)
                else:
                    attn = None
                if k + 2 < NK:
                    psq = psA.tile([P, ST], F32, tag="acc")
                else:
                    psq = None
                psc = psA.tile([P, ST], F32, tag="acc")
                last = (k == NK - 1)
                # block-closing steps reduce the tree two slots early so
                # the finish-NOW chain lands sooner in the next step
                # tree two slots early everywhere: the deferred ones-
                # matmul reads attB[:, 0, :] at idx 2 of the NEXT step,
                # and level 4 at idx 15 + DVE queue lag just misses that
                tree_at = [7, 9, 11, 13]
                held = []
                psd15 = None
                for idx in range(JT):
                    if attn is not None:
                        scores_mm(k + 1, attn, qtiles[k + 1], idx)
                    jb = (idx + 8) % JT     # att@V: B half first
                    avs = (attA, attB)[jb // 8][:, jb % 8, :]
                    nc.tensor.matmul(
                        psc[:], v_sb[:, jb, ds(h * D, D)], avs,
                        start=(idx == 0), stop=(idx == JT - 1))
                    if psq is not None:
                        if h == HL - 1:
                            qproj_mm(k + 2, psq, xts[blk(k + 2)], idx)
                        else:
                            # compress: eo 14/15 ride slots 12/13 so the
                            # accumulation stops two slots early and the
                            # rope chain (idx 13/15 below) drains inside
                            # the step instead of at its boundary
                            if idx <= 13:
                                qproj_mm(k + 2, psq, xts[blk(k + 2)], idx)
                            if 12 <= idx <= 13:
                                qproj_mm(k + 2, psq, xts[blk(k + 2)],
                                         idx + 2)
                    if not last and pending is not None:
                        # pending finish at idx 0-3, the ones-matmul BEFORE
                        # this slot's cblock matmul: its psC tile then sits
                        # AHEAD of cblock group 1 in the bank ring and its
                        # reader is the fast Ln at idx 1, so every cblock
                        # group alloc gets >=4 slots of margin over its
                        # predecessor's copy (g2 no longer waits g1's CAST)
                        if idx == 0:
                            pk, patt, ppsc = pending
                            p_psd = finish_mm(patt)
                        elif idx == 1:
                            p_lnd = finish_ln(p_psd)
                        elif idx == 2:
                            p_rec = finish_exp(p_lnd)
                        elif idx == 3:
                            finish_tt(pk, ppsc, p_rec)
                            pending = None
                    if ci_prev is not None:
                        if k % 4 != 0:
                            if last and idx >= 4:
                                # hold the tail of the previous block's
                                # out-projection so it can hide the final
                                # normalize's Ln/Exp latency (below)
                                held.append((k % 4) * JT + idx)
                            else:
                                cblock_mm(ci_prev, (k % 4) * JT + idx, cpst)
                        elif idx >= 8:
                            # block-boundary step: the previous block's ctx
                            # normalize lands ~1us in, so weave its out-
                            # projection into the back half, two per slot
                            cblock_mm(ci_prev, (idx - 8) * 2, cpst)
                            cblock_mm(ci_prev, (idx - 8) * 2 + 1, cpst)
                    if not last:
                        if psq is not None and h < HL - 1:
                            if idx == 13:
                                qt, rf_early = qproj_fin(k + 2, psq)
                                qtiles.append(qt)
                            elif idx == 15:
                                rope_finish(*rf_early)
                        if h == HL - 1:
                            # block-closing step: own-head ones-matmul/Ln
                            # woven into idx 14-15 so ctx is final ~1us
                            # into the next (boundary) step
                            if idx == 14:
                                now_psd = finish_mm(att)
                            elif idx == 15:
                                now_lnd = finish_ln(now_psd)
                        # denominator tree levels woven into the matmul
                        # stream; they only ever write attB, whose att@V
                        # reads all finished at idx 7
                        if idx == tree_at[0]:
                            nc.vector.tensor_tensor(
                                attB[:], attB[:], attA[:], Alu.add)
                        elif idx == tree_at[1]:
                            nc.vector.tensor_tensor(
                                attB[:, 0:4, :], attB[:, 0:4, :],
                                attB[:, 4:8, :], Alu.add)
                        elif idx == tree_at[2]:
                            nc.vector.tensor_tensor(
                                attB[:, 0:2, :], attB[:, 0:2, :],
                                attB[:, 2:4, :], Alu.add)
                        elif idx == tree_at[3]:
                            nc.vector.tensor_tensor(
                                attB[:, 0, :], attB[:, 0, :], attB[:, 1, :],
                                Alu.add)
                    else:
                        # LAST step: nothing interleaves after it, so the
                        # usual deferred chains would serialize behind the
                        # matmul stream.  Weave the pending head's finish
                        # into idx 0-3, halve the tree (level 1 only, the
                        # first half written into step NK-2's retired attB
                        # so the coarse dep-tracker sees no write to a tile
                        # the PE still reads), and let the PE itself do the
                        # final 8-way reduction as an accumulating
                        # ones-matmul group in idx 8-15.
                        attB_prev = atts[NK - 2][1]
                        if idx == 0:
                            pk, patt, ppsc = pending
                            p_psd = finish_mm(patt)
                        elif idx == 1:
                            p_lnd = finish_ln(p_psd)
                        elif idx == 2:
                            p_rec = finish_exp(p_lnd)
                        elif idx == 3:
                            finish_tt(pk, ppsc, p_rec)
                            pending = None
                        elif idx == 4:
                            nc.vector.tensor_tensor(
                                attB_prev[:, 0:4, :], attB[:, 0:4, :],
                                attA[:, 0:4, :], Alu.add)
                        elif idx == 7:
                            nc.vector.tensor_tensor(
                                attB[:, 4:8, :], attB[:, 4:8, :],
                                attA[:, 4:8, :], Alu.add)
                        if idx >= 8:
                            s = idx - 8
                            src = attB_prev[:, s, :] if s < 4 \
                                else attB[:, s, :]
                            if s == 0:
                                psd15 = psA.tile([P, ST], F32, tag="acc",
                                                 name="psd15")
                            nc.tensor.matmul(
                                psd15[:], ones_ap, src,
                                start=(s == 0), stop=(s == 7))
                if psq is not None and (h == HL - 1 or last):
                    qt, rf = qproj_fin(k + 2, psq)
                    qtiles.append(qt)
                else:
                    rf = None
                if pending is not None:
                    finish(*pending)
                    pending = None
                if h == HL - 1:
                    # last head of the block: finish NOW so the next
                    # block's interleaved out-projection reads final ctx
                    if last:
                        finish_tt(k, psc, finish_exp(finish_ln(psd15)))
                    else:
                        finish_tt(k, psc, finish_exp(now_lnd))
                    for jb in held:
                        cblock_mm(ci_prev, jb, cpst)
                else:
                    pending = (k, att, psc)
                if rf is not None:
                    rope_finish(*rf)

            # the last block's out-projection has no next block to hide in
            cpst = [None, None]
            for jb in range(4 * JT - 4):
                cblock_mm(blk(NK - 1), jb, cpst, drain=True)
            # final tile: accumulate the two column halves as separate
            # groups so the first half's copy+DMA overlaps the second
            # half's matmuls and the tail transfer is only [P, 256]
            ci = blk(NK - 1)
            sb = ci * (ST // P) + 3
            for hf in range(2):
                psf = psC.tile([P, ST // 2], F32, tag="ct", name="ctf")
                for ho in range(HL):
                    nc.tensor.matmul(
                        psf[:], ctxT_sb[:, ho, ds(sb * P, P)],
                        ow_sb[:, ho, ds(3 * ST + hf * (ST // 2), ST // 2)],
                        start=(ho == 0), stop=(ho == HL - 1))
                otf = oc.tile([P, ST // 2], BF16, tag="otf", name="otf")
                nc.vector.tensor_copy(otf[:], psf[:])
                eng = nc.scalar if hf else nc.sync
                eng.dma_start(
                    out[ds(sb * P, P), ds(3 * ST + hf * (ST // 2), ST // 2)],
                    otf[:])

    return nc


def _rope_tables():
    inv_freq = 1.0 / (10000.0 ** (np.arange(0, D, 2, dtype=np.float32) / D))
    t = np.arange(S, dtype=np.float32)
    freqs = np.einsum("s,f->sf", t, inv_freq)
    emb = np.concatenate([freqs, freqs], axis=-1)
    cosT = np.cos(emb).astype(np.float32).T.copy()
    sinT = np.sin(emb).astype(np.float32).T.copy()
    # rotate-half sign lives in the on-device permutation matrix
    return cosT.astype(BF), np.ascontiguousarray(sinT).astype(BF)


def _core_inputs(x, Wqkv_w, Wqkv_b, out_w, b, g, shared, xT_bf):
    # k-head columns first, then q-head columns (matches kernel layout)
    k_cols, q_cols, kb_rows, qb_rows = [], [], [], []
    for hl in range(HL):
        h = g * HL + hl
        q_cols.append(Wqkv_w[h * D:(h + 1) * D, :].T)
        k_cols.append(Wqkv_w[E + h * D:E + (h + 1) * D, :].T)
        qb_rows.append(Wqkv_b[h * D:(h + 1) * D])
        kb_rows.append(Wqkv_b[E + h * D:E + (h + 1) * D])
    def pack(wT):
        # [E, HL*D] -> [P, EO, HL*D]: per-partition contiguous rows so
        # the whole tensor moves in one (or few) large-descriptor DMAs
        return np.ascontiguousarray(
            wT.reshape(E // P, P, HL * D).transpose(1, 0, 2)).astype(BF)

    wkP = pack(np.concatenate(k_cols, axis=1))
    wqP = pack(np.concatenate(q_cols, axis=1))
    qkbT = np.stack(kb_rows + qb_rows).astype(np.float32).T    # [D, 2HL]
    v0 = 2 * E + g * HL * D
    wvP = pack(Wqkv_w[v0:v0 + HL * D, :].T)
    vb = Wqkv_b[v0:v0 + HL * D].astype(np.float32)
    owT = np.ascontiguousarray(
        out_w[:, g * HL * D:(g + 1) * HL * D].T).astype(BF)
    cosT, sinT, ones, perm = shared
    # qkb | vb | ones | perm | cos | sin (kernel's tbl layout); biases in
    # bf16 cost ~0.4% of their 0.01-scale values -- negligible
    tblP = np.ascontiguousarray(np.concatenate(
        [qkbT, np.broadcast_to(vb[None, :], (P, HL * D)), ones, perm,
         cosT, sinT], axis=1).astype(np.float32)).astype(BF)
    return {"xP": xT_bf, "wkP": wkP, "wqP": wqP, "wvP": wvP,
            "tblP": tblP, "owT": owT}


def kernel(x, Wqkv_w, Wqkv_b, out_w, out_b):
    global LAST_EXEC_NS
    _install_axon_ntff_shim()
    from concourse.bass_utils import run_bass_kernel_spmd

    x = np.asarray(x, dtype=np.float32)
    Wqkv_w = np.asarray(Wqkv_w, dtype=np.float32)
    Wqkv_b = np.asarray(Wqkv_b, dtype=np.float32)
    out_w = np.asarray(out_w, dtype=np.float32)
    out_b = np.asarray(out_b, dtype=np.float32)

    cosT, sinT = _rope_tables()
    # rotate-half permutation: out[d] = -q[d+64] (d<64), +q[d-64] (d>=64)
    perm = np.zeros((P, P), dtype=np.float32)
    for d in range(D // 2):
        perm[d + D // 2, d] = -1.0
        perm[d, d + D // 2] = 1.0
    shared = (cosT.astype(np.float32), sinT.astype(np.float32),
              np.ones((P, P), np.float32), perm)
    # x packed as [NS, P, EO, ST]: xP[i, p, eo, s] = x[b, i*ST+s, eo*P+p]
    NS, EO, ST = S // 512, E // P, 512
    xT_bf = [np.ascontiguousarray(
        x[b].reshape(NS, ST, EO, P).transpose(0, 3, 2, 1)).astype(BF)
        for b in range(2)]
    in_maps = []
    for core in range(8):
        b, g = core // 4, core % 4
        in_maps.append(
            _core_inputs(x, Wqkv_w, Wqkv_b, out_w, b, g, shared, xT_bf[b]))

    nc = bass.Bass()
    _build_mha(nc)
    _split_multi_waits(nc)

    trace = bool(os.environ.get("MHA_TRACE"))
    if trace:
        # dev-only profiling path; skip the S3 artifact upload
        import concourse.bass_utils as _bu
        _bu.upload_artifacts = lambda tmpdir: tmpdir
    res = run_bass_kernel_spmd(
        nc, in_maps, core_ids=list(range(8)), trace=trace)
    if trace:
        LAST_EXEC_NS = res.exec_time_ns

    out = np.empty((2, S, E), dtype=np.float32)
    for b in range(2):
        acc = res.results[b * 4 + 0]["out"].astype(np.float32)
        for g in range(1, 4):
            acc += res.results[b * 4 + g]["out"].astype(np.float32)
        out[b] = acc + out_b[None, :]
    return out


# revision 27
# speedup vs baseline: 1.0185x; 1.0073x over previous
"""Sharded MHA-with-RoPE Trainium2 kernel (nn_CustomTorchMHASelf).

Contract: kernel(**inputs) takes the FULL unsharded inputs of the
reference (x [2,2048,2048], Wqkv_w [6144,2048], Wqkv_b [6144],
out_w [2048,2048], out_b [2048]) and returns the full [2,2048,2048]
fp32 output, running the compute on 8 NeuronCores.

Sharding: core = b*4 + g handles batch b and head-group g (4 of the 16
heads). Each core computes q/k/v projections for its heads, RoPE,
softmax attention, and its slice of the out-projection; the host sums
the 4 partial outputs per batch and adds out_b.

Device data plane is bf16 (fp32 PSUM accumulation); the host
pre-transposes x and the weight slices into the layouts the TensorE
wants (contraction dim on partitions everywhere).

Schedule: pass 1 computes K+RoPE and V for all tokens (the last block
also hides the attention prologue under its V-projection); pass 2 is a
flat software pipeline over (block, head) steps -- at step k the PE
stream interleaves att@V(k), scores(k+1), q-projection(k+2) and a
quarter of the previous block's out-projection (65 matmuls/step), so
the ScalarE exp stream (16/step) is never on the critical path.
Key device tricks:
  - DMA triggers cost ~600ns of issuing-engine queue time EACH
    regardless of size, so tensors move with ONE trigger per tile
    (16KB-per-partition contiguous descriptor runs) instead of 16;
    the startup-critical wk/x0 pair is split into eighth/quarter
    chunks issued alternately so the first K matmuls start ~10.5us
    in, and block 0's K-projection walks chunk-major (4 live PSUM
    accumulators) to consume chunks as they land;
  - rotate-half for RoPE is a PE matmul with a signed permutation
    matrix (SBUF-SBUF DMA swaps are slow and their DIRECT2D triggers
    serialize on the sync sequencer);
  - the softmax denominator is a bf16 tree-add into the attB tile on
    VectorE plus ONE ones-matmul per (head, block) instead of 16 full
    PE ones-matmuls; att is split into two tiles (attA/attB) so the
    tree's writes never alias tiles the PE still reads (the dep
    tracker is coarse); the ones-matmul+normalize are deferred one
    step so the PE never waits on the tree;
  - 1/denominator = Exp(-Ln(d)) on ScalarE (ln and exp share an
    activation table, so no table reloads) because DVE reciprocal is
    slow and custom-DVE ops don't compile on this toolchain;
  - deferred finishes are woven into matmul slots 0-3 of the next step
    with the ones-matmul emitted BEFORE that slot's out-projection
    matmul, so the finish tile leads the psC bank ring (its reader is
    the fast Ln) and the Ln/Exp never lands at a step boundary where
    the score matmuls wait on psS;
  - warm-up matmuls on a memset tile (no DMA dependency) ramp the PE
    out of its 1.2GHz cold p-state while the first weight/x DMAs land,
    and filler matmuls after the first two K chunks keep the HAM
    activity window busy so the clock never re-throttles mid-startup;
  - output tiles are written bf16, one batched [P, E] DMA per token
    row-block; in the last step the tree is halved (level 1 written
    into a retired att tile), the PE itself does the final 8-way
    denominator reduction as an accumulating ones-matmul group, and
    the drain's final tile is split into column halves so the tail
    transfer is short.
"""

import math
import os
import sys
import types

import numpy as np
import ml_dtypes

import concourse.bass as bass
import concourse.mybir as mybir
import concourse.tile as tile
from concourse.bass import ds

F32 = mybir.dt.float32
BF16 = mybir.dt.bfloat16
Alu = mybir.AluOpType
Act = mybir.ActivationFunctionType
BF = ml_dtypes.bfloat16

S, E, HTOT, HL, D, P = 2048, 2048, 16, 4, 128, 128

# Filled with the profile exec time (ns) when MHA_TRACE=1; read by test.py.
LAST_EXEC_NS = None


def _install_axon_ntff_shim():
    """Provide antenv.axon_hooks so trace=True can reach the axon NTFF hook."""
    if "antenv.axon_hooks" in sys.modules:
        return
    mod = types.ModuleType("antenv.axon_hooks")
    holder = [None]
    mod.set_axon_ntff_profile_hook = lambda h: holder.__setitem__(0, h)
    mod.get_axon_ntff_profile_hook = lambda: holder[0]
    sys.modules["antenv.axon_hooks"] = mod
    try:
        import antenv
        antenv.axon_hooks = mod
    except ImportError:
        pass
    # boot() ran at interpreter start (sitecustomize), before this module
    # existed, so its NTFF-hook registration was silently skipped. Redo it.
    try:
        from trn_agent_boot.trn_boot import _ntff_profile_via_ctypes
        hook = _ntff_profile_via_ctypes("/opt/axon/libaxon_pjrt.so")
        if hook is not None:
            mod.set_axon_ntff_profile_hook(hook)
    except Exception:
        pass


def _split_multi_waits(nc):
    """Hoist extra sem-waits onto standalone NoOps (one wait per inst).

    This walrus build rejects any instruction carrying more than one
    sync-wait ("Too many sync wait commands"); Tile attaches one wait per
    outstanding semaphore to the consuming instruction. Splitting them
    across same-engine NoOps placed immediately before is equivalent:
    the engine executes serially, so all waits still precede the inst.
    """
    ctr = 0
    for fn in nc.m.functions:
        for blk in fn.blocks:
            out = []
            for inst in blk.instructions:
                si = getattr(inst, "sync_info", None)
                if si is not None and si.on_wait is not None \
                        and len(si.on_wait) > 1:
                    waits = list(si.on_wait)
                    si.on_wait = [waits[-1]]
                    for w in waits[:-1]:
                        ctr += 1
                        nop = mybir.InstNoOp(
                            name=f"I-wsplit-{ctr}", ins=[], outs=[])
                        nop.engine = inst.engine
                        nop.sync_info = mybir.SyncInfo(
                            on_wait=[w], on_update=[])
                        out.append(nop)
                out.append(inst)
            blk.instructions[:] = out


def _build_mha(nc: bass.Bass):
    """Emit the per-core MHA program (one shard) into `nc`."""
    EO = E // P            # contraction subtiles for the projections
    ST = 512               # free-dim tile (one PSUM bank of fp32)
    NS = S // ST
    SB = S // P
    JT = S // P            # key blocks per head
    ET = E // ST

    # packed layouts: [.., P, EO, ST] so each tensor is one long
    # per-partition-contiguous run -> ONE ~600ns DMA trigger moves it
    xP = nc.dram_tensor("xP", [NS, P, EO, ST], BF16, kind="ExternalInput")
    wkP = nc.dram_tensor("wkP", [P, EO, HL * D], BF16, kind="ExternalInput")
    wqP = nc.dram_tensor("wqP", [P, EO, HL * D], BF16, kind="ExternalInput")
    wvP = nc.dram_tensor("wvP", [P, EO, HL * D], BF16, kind="ExternalInput")
    # qkb | vb | ones | perm | cos | sin packed into one bf16 table;
    # the small head (biases + matrices) rides an early DMA slice, the
    # big cos/sin tail a later one
    TW = 2 * HL + HL * D + 2 * P + 2 * S
    tblP = nc.dram_tensor("tblP", [P, TW], BF16, kind="ExternalInput")
    owT = nc.dram_tensor("owT", [HL * D, E], BF16, kind="ExternalInput")
    out = nc.dram_tensor("out", [S, E], BF16, kind="ExternalOutput")

    isc = 1.0 / math.sqrt(D)

    from contextlib import ExitStack

    with tile.TileContext(nc) as tc, ExitStack() as stk:
        persist = stk.enter_context(tc.tile_pool(name="persist", bufs=1))
        kT_sb = persist.tile([P, HL, S], BF16)      # k post-RoPE [d, h, s]
        v_sb = persist.tile([P, SB, HL * D], BF16)  # v natural [s%128, s//128, hd]
        ctxT_sb = persist.tile([P, HL, S], BF16)    # [d, h, i]
        tbl_sb = persist.tile([P, TW], BF16)
        ow_sb = persist.tile([P, HL, E], BF16)
        wtile = persist.tile([P, P], BF16)          # memset warm-up operand

        QK0, VB0 = 0, 2 * HL
        ON0 = VB0 + HL * D
        PM0 = ON0 + P
        CS0, SN0 = PM0 + P, PM0 + P + S
        TSMALL = PM0 + P          # early slice: biases + ones + perm

        # tensor_scalar needs f32 scalars: widen the bf16 biases once
        qkbf = persist.tile([P, 2 * HL], F32)

        def qkb_ap(j):
            return qkbf[:, j, None]

        vb_ap = tbl_sb[:, ds(VB0, HL * D)]

        def cos_ap(s0):
            return tbl_sb[:, ds(CS0 + s0, ST)]

        def sin_ap(s0):
            return tbl_sb[:, ds(SN0 + s0, ST)]

        ones_ap = tbl_sb[:, ds(ON0, P)]
        perm_ap = tbl_sb[:, ds(PM0, P)]

        # x stream shared by both passes; rope temps likewise.  qb/rot are
        # still being read (by the rotate matmul / mults) when the next rope
        # starts, so they get extra bufs; t1/t2 are consumed immediately by
        # the in-order VectorE queue, so 1 buf suffices.
        xs = stk.enter_context(tc.tile_pool(name="xstream", bufs=2))
        rta = stk.enter_context(tc.tile_pool(name="ropetmpa", bufs=3))
        rtb = stk.enter_context(tc.tile_pool(name="ropetmpb", bufs=1))
        wqp = stk.enter_context(tc.tile_pool(name="wqpool", bufs=1))
        wq_sb = wqp.tile([P, EO, HL * D], BF16)

        psA = stk.enter_context(tc.tile_pool(name="psA", bufs=4, space="PSUM"))
        psS = stk.enter_context(tc.tile_pool(name="psS", bufs=2, space="PSUM"))
        psC = stk.enter_context(tc.tile_pool(name="psC", bufs=2, space="PSUM"))

        qp = stk.enter_context(tc.tile_pool(name="qpool", bufs=4))
        dp = stk.enter_context(tc.tile_pool(name="denp", bufs=1))
        oc = stk.enter_context(tc.tile_pool(name="ocopy", bufs=2))
        at0 = stk.enter_context(tc.tile_pool(name="att0p", bufs=1))

        # flat (block, head) schedule for the attention pass; blocks in
        # reverse order so the first one reuses pass 1's last x tile
        order = list(range(NS - 1, -1, -1))
        seq = [(i, h) for i in order for h in range(HL)]
        NK = len(seq)

        def blk(k):
            return seq[k][0]

        def rope_begin(ps, bias_ap):
            # qb = q + bias (bf16 so the rotate-half matmul runs full rate)
            qb = rta.tile([P, ST], BF16, tag="qb")
            nc.vector.tensor_scalar_add(qb[:], ps[:], bias_ap)
            return qb

        def rope_finish(qb, s0, out_ap):
            # rotate-half as a PE matmul with a signed permutation matrix
            # (cross-partition moves otherwise need a slow SBUF-SBUF DMA
            # whose trigger also serializes on the sync sequencer);
            # out = qb*cos + (perm.T @ qb)*sin.
            # rps lives in psC so the scores stream owns psS's two banks
            # outright (16 even allocations per step -> the exp reader is
            # always 2 full matmul-slots behind the next allocation)
            rps = psC.tile([P, ST], F32, tag="ct", name="rps")
            nc.tensor.matmul(rps[:], perm_ap, qb[:], start=True, stop=True)
            t1 = rtb.tile([P, ST], BF16, tag="t1")
            t2 = rtb.tile([P, ST], BF16, tag="t2")
            nc.vector.tensor_tensor(t1[:], qb[:], cos_ap(s0), Alu.mult)
            nc.vector.tensor_tensor(t2[:], rps[:], sin_ap(s0), Alu.mult)
            nc.vector.tensor_tensor(out_ap, t1[:], t2[:], Alu.add)

        def qproj_mm(k, psq, xt, eo):
            h = seq[k][1]
            nc.tensor.matmul(
                psq[:], wq_sb[:, eo, ds(h * D, D)], xt[:, eo, :],
                start=(eo == 0), stop=(eo == EO - 1))

        def qproj_fin(k, psq):
            qt = qp.tile([P, ST], BF16, tag="qt")
            qb = rope_begin(psq, qkb_ap(HL + seq[k][1]))
            return qt, (qb, blk(k) * ST, qt[:])

        def scores_mm(k, attab, qt, jb):
            h = seq[k][1]
            ps = psS.tile([P, ST], F32, tag="sc")
            nc.tensor.matmul(
                ps[:], kT_sb[:, h, ds(jb * P, P)], qt[:],
                start=True, stop=True)
            dst = attab[jb // 8][:, jb % 8, :]
            nc.scalar.activation(dst, ps[:], Act.Exp, scale=isc)

        # PE warm-up on a memset tile (no DMA dependency): the HAM clock
        # gate needs ~3.4us of sustained PE activity to release the cold
        # 1.2GHz p-state, and the first real matmul data lands ~10.5us in.
        nc.gpsimd.memset(wtile[:], 1.0)
        wsink = persist.tile([P, 1], F32)
        wps = psS.tile([P, ST], F32, tag="sc", name="warmps")
        NWARM = 44
        for w in range(NWARM):
            nc.tensor.matmul(
                wps[:, :P], wtile[:], wtile[:],
                start=(w == 0), stop=(w == NWARM - 1))
        nc.vector.tensor_copy(wsink[:], wps[:, :1])

        # ---- pass 1: K projection + RoPE, V projection ----
        # The last block additionally hides the attention pass's prologue
        # (q heads 0/1 + scores/exp for head 0) under its V-projection.
        xt_last = None
        att0 = None
        qtiles = []
        with tc.tile_pool(name="p1w", bufs=1) as p1:
            wk_sb = p1.tile([P, EO, HL * D], BF16)
            wv_sb = p1.tile([P, EO, HL * D], BF16)
            # Startup DMA plan.  Sync carries the startup-critical wk/x0
            # stream as interleaved eighth/quarter chunks (each trigger is
            # ~600ns of queue time, each chunk 0.7-2.9us of transfer);
            # Scalar (idle all of pass 1, first free ~8.8us after its
            # activation-table load) carries everything else in whole-tile
            # triggers, ordered by first use.
            # Each HWDGE queue streams ~200 B/ns, so the two startup-
            # critical tensors (wk, x0) ride DIFFERENT engines' queues in
            # matched chunks; everything else follows in first-use order.
            xt0 = xs.tile([P, EO, ST], BF16, tag="xt", name="xt0")
            xt1 = xs.tile([P, EO, ST], BF16, tag="xt", name="xt1")
            CH0 = [(0, 2), (2, 8), (8, 16)]
            # pair 0 rides Sync alone: the Scalar queue crawls for its
            # first ~3us (activation-table loads share its DGE path), so
            # the first real matmuls start ~1us earlier this way
            nc.sync.dma_start(wk_sb[:, 0:2, :], wkP[:, 0:2, :])
            nc.sync.dma_start(xt0[:, 0:2, :], xP[0][:, 0:2, :])
            nc.scalar.dma_start(tbl_sb[:, :TSMALL], tblP[:, :TSMALL])
            nc.vector.tensor_copy(qkbf[:], tbl_sb[:, ds(QK0, 2 * HL)])
            nc.sync.dma_start(wk_sb[:, 2:8, :], wkP[:, 2:8, :])
            nc.scalar.dma_start(xt0[:, 2:8, :], xP[0][:, 2:8, :])
            nc.sync.dma_start(wk_sb[:, 8:16, :], wkP[:, 8:16, :])
            nc.scalar.dma_start(xt0[:, 8:16, :], xP[0][:, 8:16, :])
            nc.sync.dma_start(wv_sb[:, 0:8, :], wvP[:, 0:8, :])
            nc.sync.dma_start(wv_sb[:, 8:16, :], wvP[:, 8:16, :])
            nc.scalar.dma_start(tbl_sb[:, TSMALL:], tblP[:, TSMALL:])
            nc.scalar.dma_start(xt1[:, 0:8, :], xP[1][:, 0:8, :])
            nc.scalar.dma_start(xt1[:, 8:16, :], xP[1][:, 8:16, :])

            for i in range(NS):
                if i == 0:
                    xt = xt0
                elif i == 1:
                    xt = xt1
                else:
                    xt = xs.tile([P, EO, ST], BF16, tag="xt")
                    nc.sync.dma_start(xt[:, 0:8, :], xP[i][:, 0:8, :])
                    nc.sync.dma_start(xt[:, 8:16, :], xP[i][:, 8:16, :])
                    if i == NS - 1:
                        # prefetch pass-2 weights behind this block's x:
                        # wq feeds the q-projections later in this block,
                        # ow the out-projection a block later
                        nc.sync.dma_start(wq_sb[:, 0:8, :], wqP[:, 0:8, :])
                        nc.sync.dma_start(wq_sb[:, 8:16, :], wqP[:, 8:16, :])
                        nc.sync.dma_start(
                            ow_sb[:],
                            owT[:].rearrange("(h p) e -> p h e", p=P))
                sl = ds(i * ST, ST)
                # K-projection: block 0 walks chunk-major with one live
                # PSUM accumulator per head so the PE consumes wk/x0
                # chunks as they land; later blocks' x is fully resident.
                chunks = CH0 if i == 0 else [(0, EO)]
                kps = [psA.tile([P, ST], F32, tag="acc", name=f"kps{jb}")
                       for jb in range(HL)]
                for ci, (a, b) in enumerate(chunks):
                    for jb in range(HL):
                        for eo in range(a, b):
                            nc.tensor.matmul(
                                kps[jb][:], wk_sb[:, eo, ds(jb * D, D)],
                                xt[:, eo, :],
                                start=(eo == 0), stop=(eo == EO - 1))
                    if i == 0 and ci < 2:
                        # keep the HAM clock-gate's activity window busy
                        # while the next wk/x0 chunk lands (psS is untouched
                        # through block 0's K, so wps is still the scores
                        # pool's most recent allocation)
                        for _ in range(34 + 6 * ci):
                            nc.tensor.matmul(
                                wps[:, :P], wtile[:], wtile[:],
                                start=True, stop=True)
                kropes = []
                for jb in range(HL):
                    qb = rope_begin(kps[jb], qkb_ap(jb))
                    kropes.append((qb, i * ST, kT_sb[:, jb, sl]))
                if i < NS - 1:
                    for sbl in range(ST // P):
                        sb = i * (ST // P) + sbl
                        ps = psS.tile([P, ST], F32, tag="sc")
                        for eo in range(EO):
                            nc.tensor.matmul(
                                ps[:, : HL * D], xt[:, eo, ds(sbl * P, P)],
                                wv_sb[:, eo, :],
                                start=(eo == 0), stop=(eo == EO - 1))
                        nc.vector.tensor_tensor(
                            v_sb[:, sb, :], ps[:, : HL * D], vb_ap, Alu.add)
                    for kr in kropes:
                        rope_finish(*kr)
                else:
                    for kr in kropes:
                        rope_finish(*kr)
                    # q-projections for the first two attention heads, then
                    # V-projection interleaved with scores/exp for head 0
                    psq0 = psA.tile([P, ST], F32, tag="acc")
                    for eo in range(EO):
                        qproj_mm(0, psq0, xt, eo)
                    qt0, rf0 = qproj_fin(0, psq0)
                    psq1 = psA.tile([P, ST], F32, tag="acc")
                    for eo in range(EO):
                        qproj_mm(1, psq1, xt, eo)
                    qt1, rf1 = qproj_fin(1, psq1)
                    qtiles.extend([qt0, qt1])
                    rope_finish(*rf0)
                    att0 = (at0.tile([P, 8, ST], BF16, name="att0A"),
                            at0.tile([P, 8, ST], BF16, name="att0B"))
                    sc_jb = 0
                    vps = None
                    for vi in range(4 * EO):
                        sbl, eo = vi // EO, vi % EO
                        if eo == 0:
                            vps = psA.tile([P, ST], F32, tag="acc")
                        nc.tensor.matmul(
                            vps[:, : HL * D], xt[:, eo, ds(sbl * P, P)],
                            wv_sb[:, eo, :],
                            start=(eo == 0), stop=(eo == EO - 1))
                        if eo == EO - 1:
                            nc.vector.tensor_tensor(
                                v_sb[:, i * (ST // P) + sbl, :],
                                vps[:, : HL * D], vb_ap, Alu.add)
                        if vi >= EO and (vi - EO) % 3 == 0 and sc_jb < JT:
                            scores_mm(0, att0, qt0, sc_jb)
                            sc_jb += 1
                        if vi == 40:
                            rope_finish(*rf1)
                if i == NS - 1:
                    xt_last = xt

        # ---- pass 2: flat software pipeline over (block, head) steps ----
        # Blocks run in reverse so the first one reuses pass 1's last x
        # tile.  At step k: att@V + denominator tree for head k, scores+exp
        # for head k+1, Q-projection for head k+2, one quarter of the
        # PREVIOUS block's out-projection, and the deferred denominator
        # ones-matmul + normalize for head k-1 -- all interleaved so the PE
        # stream (65 matmuls/step) hides the exp stream (16/step).
        with tc.tile_pool(name="attpa", bufs=3) as abA, \
             tc.tile_pool(name="attpb", bufs=3) as abB:

            xts = {order[0]: xt_last}

            def cblock_mm(ci, jb, pst, drain=False):
                # one of the 16x4 out-projection matmuls for token block ci;
                # jb runs 0..63 across the block's four steps.  Each row
                # block's four 512-col quarters collect in one oc tile and
                # leave in a single [P, E] DMA.
                tile_i, ho = jb // HL, jb % HL
                sb_loc, et = tile_i // ET, tile_i % ET
                sb = ci * (ST // P) + sb_loc
                if ho == 0:
                    pst[0] = psC.tile([P, ST], F32, tag="ct", name="ct")
                nc.tensor.matmul(
                    pst[0][:], ctxT_sb[:, ho, ds(sb * P, P)],
                    ow_sb[:, ho, ds(et * ST, ST)],
                    start=(ho == 0), stop=(ho == HL - 1))
                if ho == HL - 1:
                    if et == 0:
                        pst[1] = oc.tile([P, ET, ST], BF16, tag="ot",
                                         name="ot")
                    nc.vector.tensor_copy(pst[1][:, et, :], pst[0][:])
                    if drain and sb_loc == ST // P - 1:
                        # final row block: one small DMA per quarter right
                        # after its copy, alternating trigger engines, so
                        # the post-matmul tail transfer is short
                        eng = nc.scalar if (et % 2) else nc.sync
                        eng.dma_start(
                            out[ds(sb * P, P), ds(et * ST, ST)],
                            pst[1][:, et, :])
                    elif et == ET - 1:
                        if not drain:
                            nc.sync.dma_start(
                                out[ds(sb * P, P), :], pst[1][:])
                        else:
                            eng = nc.scalar if (sb_loc % 2) else nc.sync
                            eng.dma_start(out[ds(sb * P, P), :], pst[1][:])

            # denominator ones-matmul + normalize for head k; deferred one
            # step so the PE reaches the ones-matmul well after the VectorE
            # tree produced attB[:, 0, :].  1/d = Exp(-Ln(d)) on ScalarE
            # (ln/exp share an activation table: no reloads).  Split into
            # pieces so the last two steps can weave them into the matmul
            # stream instead of serializing after it.
            def finish_mm(attab):
                psd = psC.tile([P, ST], F32, tag="ct", name="psd")
                nc.tensor.matmul(
                    psd[:], ones_ap, attab[1][:, 0, :],
                    start=True, stop=True)
                return psd

            def finish_ln(psd):
                lnd = dp.tile([P, ST], F32, tag="lnd")
                nc.scalar.activation(lnd[:], psd[:], Act.Ln)
                return lnd

            def finish_exp(lnd):
                rec = dp.tile([P, ST], F32, tag="rec")
                nc.scalar.activation(rec[:], lnd[:], Act.Exp, scale=-1.0)
                return rec

            def finish_tt(k, psc, rec):
                nc.vector.tensor_tensor(
                    ctxT_sb[:, seq[k][1], ds(blk(k) * ST, ST)],
                    psc[:], rec[:], Alu.mult)

            def finish(k, attab, psc):
                finish_tt(k, psc, finish_exp(finish_ln(finish_mm(attab))))

            atts = [att0]

            cpst = [None, None]
            pending = None
            for k in range(NK):
                i, h = seq[k]
                if h == 0 and k + 4 < NK:
                    # prefetch the x tile for the NEXT block now; the DMA
                    # has a whole block (~55us) to land
                    nxt = blk(k + 4)
                    xtn = xs.tile([P, EO, ST], BF16, tag="xt")
                    nc.sync.dma_start(xtn[:], xP[nxt][:])
                    xts[nxt] = xtn
                att = atts[k]
                attA, attB = att
                ci_prev = blk(k - 4) if k >= 4 else None
                if k + 1 < NK:
                    attn = (abA.tile([P, 8, ST], BF16, tag="attA", name="attA"),
                            abB.tile([P, 8, ST], BF16, tag="attB", name="attB"))
                    atts.append(attn)
                else:
                    attn = None
                if k + 2 < NK:
                    psq = psA.tile([P, ST], F32, tag="acc")
                else:
                    psq = None
                psc = psA.tile([P, ST], F32, tag="acc")
                last = (k == NK - 1)
                # block-closing steps reduce the tree two slots early so
                # the finish-NOW chain lands sooner in the next step
                # tree two slots early everywhere: the deferred ones-
                # matmul reads attB[:, 0, :] at idx 2 of the NEXT step,
                # and level 4 at idx 15 + DVE queue lag just misses that
                tree_at = [7, 9, 11, 13]
                held = []
                psd15 = None
                for idx in range(JT):
                    if attn is not None:
                        scores_mm(k + 1, attn, qtiles[k + 1], idx)
                    jb = (idx + 8) % JT     # att@V: B half first
                    avs = (attA, attB)[jb // 8][:, jb % 8, :]
                    nc.tensor.matmul(
                        psc[:], v_sb[:, jb, ds(h * D, D)], avs,
                        start=(idx == 0), stop=(idx == JT - 1))
                    if psq is not None:
                        if h == HL - 1:
                            qproj_mm(k + 2, psq, xts[blk(k + 2)], idx)
                        else:
                            # compress: eo 14/15 ride slots 12/13 so the
                            # accumulation stops two slots early and the
                            # rope chain (idx 13/15 below) drains inside
                            # the step instead of at its boundary
                            if idx <= 13:
                                qproj_mm(k + 2, psq, xts[blk(k + 2)], idx)
                            if 12 <= idx <= 13:
                                qproj_mm(k + 2, psq, xts[blk(k + 2)],
                                         idx + 2)
                    if not last and pending is not None:
                        # pending finish at idx 0-3, the ones-matmul BEFORE
                        # this slot's cblock matmul: its psC tile then sits
                        # AHEAD of cblock group 1 in the bank ring and its
                        # reader is the fast Ln at idx 1, so every cblock
                        # group alloc gets >=4 slots of margin over its
                        # predecessor's copy (g2 no longer waits g1's CAST)
                        if idx == 0:
                            pk, patt, ppsc = pending
                            p_psd = finish_mm(patt)
                        elif idx == 1:
                            p_lnd = finish_ln(p_psd)
                        elif idx == 2:
                            p_rec = finish_exp(p_lnd)
                        elif idx == 3:
                            finish_tt(pk, ppsc, p_rec)
                            pending = None
                    if ci_prev is not None:
                        if k % 4 != 0:
                            if last and idx >= 4:
                                # hold the tail of the previous block's
                                # out-projection so it can hide the final
                                # normalize's Ln/Exp latency (below)
                                held.append((k % 4) * JT + idx)
                            else:
                                cblock_mm(ci_prev, (k % 4) * JT + idx, cpst)
                        elif idx >= 8:
                            # block-boundary step: the previous block's ctx
                            # normalize lands ~1us in, so weave its out-
                            # projection into the back half, two per slot
                            cblock_mm(ci_prev, (idx - 8) * 2, cpst)
                            cblock_mm(ci_prev, (idx - 8) * 2 + 1, cpst)
                    if not last:
                        if psq is not None and h < HL - 1:
                            if idx == 13:
                                qt, rf_early = qproj_fin(k + 2, psq)
                                qtiles.append(qt)
                            elif idx == 15:
                                rope_finish(*rf_early)
                        if h == HL - 1:
                            # block-closing step: own-head ones-matmul/Ln
                            # woven into idx 14-15 so ctx is final ~1us
                            # into the next (boundary) step
                            if idx == 14:
                                now_psd = finish_mm(att)
                            elif idx == 15:
                                now_lnd = finish_ln(now_psd)
                        # denominator tree levels woven into the matmul
                        # stream; they only ever write attB, whose att@V
                        # reads all finished at idx 7
                        if idx == tree_at[0]:
                            nc.vector.tensor_tensor(
                                attB[:], attB[:], attA[:], Alu.add)
                        elif idx == tree_at[1]:
                            nc.vector.tensor_tensor(
                                attB[:, 0:4, :], attB[:, 0:4, :],
                                attB[:, 4:8, :], Alu.add)
                        elif idx == tree_at[2]:
                            nc.vector.tensor_tensor(
                                attB[:, 0:2, :], attB[:, 0:2, :],
                                attB[:, 2:4, :], Alu.add)
                        elif idx == tree_at[3]:
                            nc.vector.tensor_tensor(
                                attB[:, 0, :], attB[:, 0, :], attB[:, 1, :],
                                Alu.add)
                    else:
                        # LAST step: nothing interleaves after it, so the
                        # usual deferred chains would serialize behind the
                        # matmul stream.  Weave the pending head's finish
                        # into idx 0-3, halve the tree (level 1 only, the
                        # first half written into step NK-2's retired attB
                        # so the coarse dep-tracker sees no write to a tile
                        # the PE still reads), and let the PE itself do the
                        # final 8-way reduction as an accumulating
                        # ones-matmul group in idx 8-15.
                        attB_prev = atts[NK - 2][1]
                        if idx == 0:
                            pk, patt, ppsc = pending
                            p_psd = finish_mm(patt)
                        elif idx == 1:
                            p_lnd = finish_ln(p_psd)
                        elif idx == 2:
                            p_rec = finish_exp(p_lnd)
                        elif idx == 3:
                            finish_tt(pk, ppsc, p_rec)
                            pending = None
                        elif idx == 4:
                            nc.vector.tensor_tensor(
                                attB_prev[:, 0:4, :], attB[:, 0:4, :],
                                attA[:, 0:4, :], Alu.add)
                        elif idx == 7:
                            nc.vector.tensor_tensor(
                                attB[:, 4:8, :], attB[:, 4:8, :],
                                attA[:, 4:8, :], Alu.add)
                        if idx >= 8:
                            s = idx - 8
                            src = attB_prev[:, s, :] if s < 4 \
                                else attB[:, s, :]
                            if s == 0:
                                psd15 = psA.tile([P, ST], F32, tag="acc",
                                                 name="psd15")
                            nc.tensor.matmul(
                                psd15[:], ones_ap, src,
                                start=(s == 0), stop=(s == 7))
                if psq is not None and (h == HL - 1 or last):
                    qt, rf = qproj_fin(k + 2, psq)
                    qtiles.append(qt)
                else:
                    rf = None
                if pending is not None:
                    finish(*pending)
                    pending = None
                if h == HL - 1:
                    # last head of the block: finish NOW so the next
                    # block's interleaved out-projection reads final ctx
                    if last:
                        finish_tt(k, psc, finish_exp(finish_ln(psd15)))
                    else:
                        finish_tt(k, psc, finish_exp(now_lnd))
                    for jb in held:
                        cblock_mm(ci_prev, jb, cpst)
                else:
                    pending = (k, att, psc)
                if rf is not None:
                    rope_finish(*rf)

            # the last block's out-projection has no next block to hide in
            cpst = [None, None]
            for jb in range(4 * JT - 4):
                cblock_mm(blk(NK - 1), jb, cpst, drain=True)
            # final tile: accumulate the two column halves as separate
            # groups so the first half's copy+DMA overlaps the second
            # half's matmuls and the tail transfer is only [P, 256]
            ci = blk(NK - 1)
            sb = ci * (ST // P) + 3
            for hf in range(2):
                psf = psC.tile([P, ST // 2], F32, tag="ct", name="ctf")
                for ho in range(HL):
                    nc.tensor.matmul(
                        psf[:], ctxT_sb[:, ho, ds(sb * P, P)],
                        ow_sb[:, ho, ds(3 * ST + hf * (ST // 2), ST // 2)],
                        start=(ho == 0), stop=(ho == HL - 1))
                otf = oc.tile([P, ST // 2], BF16, tag="otf", name="otf")
                nc.vector.tensor_copy(otf[:], psf[:])
                eng = nc.scalar if hf else nc.sync
                eng.dma_start(
                    out[ds(sb * P, P), ds(3 * ST + hf * (ST // 2), ST // 2)],
                    otf[:])

    return nc


def _rope_tables():
    inv_freq = 1.0 / (10000.0 ** (np.arange(0, D, 2, dtype=np.float32) / D))
    t = np.arange(S, dtype=np.float32)
    freqs = np.einsum("s,f->sf", t, inv_freq)
    emb = np.concatenate([freqs, freqs], axis=-1)
    cosT = np.cos(emb).astype(np.float32).T.copy()
    sinT = np.sin(emb).astype(np.float32).T.copy()
    # rotate-half sign lives in the on-device permutation matrix
    return cosT.astype(BF), np.ascontiguousarray(sinT).astype(BF)


def _core_inputs(x, Wqkv_w, Wqkv_b, out_w, b, g, shared, xT_bf):
    # k-head columns first, then q-head columns (matches kernel layout)
    k_cols, q_cols, kb_rows, qb_rows = [], [], [], []
    for hl in range(HL):
        h = g * HL + hl
        q_cols.append(Wqkv_w[h * D:(h + 1) * D, :].T)
        k_cols.append(Wqkv_w[E + h * D:E + (h + 1) * D, :].T)
        qb_rows.append(Wqkv_b[h * D:(h + 1) * D])
        kb_rows.append(Wqkv_b[E + h * D:E + (h + 1) * D])
    def pack(wT):
        # [E, HL*D] -> [P, EO, HL*D]: per-partition contiguous rows so
        # the whole tensor moves in one (or few) large-descriptor DMAs
        return np.ascontiguousarray(
            wT.reshape(E // P, P, HL * D).transpose(1, 0, 2)).astype(BF)

    wkP = pack(np.concatenate(k_cols, axis=1))
    wqP = pack(np.concatenate(q_cols, axis=1))
    qkbT = np.stack(kb_rows + qb_rows).astype(np.float32).T    # [D, 2HL]
    v0 = 2 * E + g * HL * D
    wvP = pack(Wqkv_w[v0:v0 + HL * D, :].T)
    vb = Wqkv_b[v0:v0 + HL * D].astype(np.float32)
    owT = np.ascontiguousarray(
        out_w[:, g * HL * D:(g + 1) * HL * D].T).astype(BF)
    cosT, sinT, ones, perm = shared
    # qkb | vb | ones | perm | cos | sin (kernel's tbl layout); biases in
    # bf16 cost ~0.4% of their 0.01-scale values -- negligible
    tblP = np.ascontiguousarray(np.concatenate(
        [qkbT, np.broadcast_to(vb[None, :], (P, HL * D)), ones, perm,
         cosT, sinT], axis=1).astype(np.float32)).astype(BF)
    return {"xP": xT_bf, "wkP": wkP, "wqP": wqP, "wvP": wvP,
            "tblP": tblP, "owT": owT}


def kernel(x, Wqkv_w, Wqkv_b, out_w, out_b):
    global LAST_EXEC_NS
    _install_axon_ntff_shim()
    from concourse.bass_utils import run_bass_kernel_spmd

    x = np.asarray(x, dtype=np.float32)
    Wqkv_w = np.asarray(Wqkv_w, dtype=np.float32)
    Wqkv_b = np.asarray(Wqkv_b, dtype=np.float32)
    out_w = np.asarray(out_w, dtype=np.float32)
    out_b = np.asarray(out_b, dtype=np.float32)

    cosT, sinT = _rope_tables()
    # rotate-half permutation: out[d] = -q[d+64] (d<64), +q[d-64] (d>=64)
    perm = np.zeros((P, P), dtype=np.float32)
    for d in range(D // 2):
        perm[d + D // 2, d] = -1.0
        perm[d, d + D // 2] = 1.0
    shared = (cosT.astype(np.float32), sinT.astype(np.float32),
              np.ones((P, P), np.float32), perm)
    # x packed as [NS, P, EO, ST]: xP[i, p, eo, s] = x[b, i*ST+s, eo*P+p]
    NS, EO, ST = S // 512, E // P, 512
    xT_bf = [np.ascontiguousarray(
        x[b].reshape(NS, ST, EO, P).transpose(0, 3, 2, 1)).astype(BF)
        for b in range(2)]
    in_maps = []
    for core in range(8):
        b, g = core // 4, core % 4
        in_maps.append(
            _core_inputs(x, Wqkv_w, Wqkv_b, out_w, b, g, shared, xT_bf[b]))

    nc = bass.Bass()
    _build_mha(nc)
    _split_multi_waits(nc)

    trace = bool(os.environ.get("MHA_TRACE"))
    if trace:
        # dev-only profiling path; skip the S3 artifact upload
        import concourse.bass_utils as _bu
        _bu.upload_artifacts = lambda tmpdir: tmpdir
    res = run_bass_kernel_spmd(
        nc, in_maps, core_ids=list(range(8)), trace=trace)
    if trace:
        LAST_EXEC_NS = res.exec_time_ns

    out = np.empty((2, S, E), dtype=np.float32)
    for b in range(2):
        acc = res.results[b * 4 + 0]["out"].astype(np.float32)
        for g in range(1, 4):
            acc += res.results[b * 4 + g]["out"].astype(np.float32)
        out[b] = acc + out_b[None, :]
    return out
